# revision 11
# baseline (speedup 1.0000x reference)
"""Trainium2 Bass kernel for nn_ClustGeoNodeEncoder (segment_reduce).

v2 architecture (PE-accelerated moments):
  - Host sorts clusters by length, deals them round-robin to 8 cores x 32
    tiles of 128 clusters (one cluster per partition per tile), and stages
    TWO fp16 streams per core:
      * transposed stream: [128 element-slots, ncols] feature planes
        (x/16, y/16, z/16, v, ca, cb) where each column holds up to 128
        elements of one cluster chunk (2 chunks for tiles padded > 128).
        ca = oh1 + 512*oh2, cb = oh3 + 512*oh4 pack the semantic one-hots
        (exact in fp16; sums stay < 2^24 so fp32 PSUM accumulation is
        exact).  Columns are ordered partition-major so partition p's
        clusters occupy a contiguous 13*NT-column window.
      * cluster-major stream: [128 clusters, S] x/16, y/16, z/16 planes,
        feature-major per group of 4 equal-padded tiles (for pass B).
  - Device pass A: ACT squares the coordinate/value planes, DVE forms the
    three cross-product planes (2x fp16 mode), and the TensorEngine
    reduces all 13 moment planes per cluster with ones-column matmuls:
    a staircase window (ones only in absolute column 128 of a [128, 256]
    buffer) places partition p's sums into PSUM row p; 128 accumulating
    matmuls cover all partitions, long-tile second chunks accumulate into
    the same PSUM columns.  One [128, 13*NT] PSUM->SBUF copy evacuates
    every raw moment.
  - Cluster math on [128, NT] fp32 planes: centers, centered scatter
    matrix A (scale-free in /16 units), closed-form trig eigenvalues,
    principal eigenvector via spectral projector, B = A/w2, dirwt, value
    stats, semantic mode via int32-truncation unpack of ca/cb.
  - Pass B (cluster-major): ts-centering (4x fp16), per-tile stt dot with
    v0, ACT group squares + sqrt, stt-accum orientation statistic sc;
    padded-slot closed-form correction, sign flip, output DMA.
"""

import sys

for _p in ("/opt/trn_rl_repo",):
    if _p not in sys.path:
        sys.path.insert(0, _p)

import numpy as np

N = 2_000_000
C = 32768
L = 256
N_CORES = 8
P = 128
NT = C // (P * N_CORES)  # 32 tiles per core
NG = 8                   # pass-B tile groups (4 tiles each, shared pad)
f32 = np.float32
f16 = np.float16

_PI = float(np.pi)
SCL = 16.0               # coordinate pre-scale (powers of 2 are exact)


def _host_prep(data, clust_idx, clust_len):
    data = np.asarray(data, dtype=f32)
    clust_idx = np.asarray(clust_idx).astype(np.int64)
    lens = np.asarray(clust_len).astype(np.int64)

    # feature table: x/16, y/16, z/16, v, ca, cb ; row N = zeros for padding
    table = np.zeros((N + 1, 6), dtype=f32)
    table[:N, 0:3] = data[:, 0:3] / SCL
    table[:N, 3] = data[:, 4]
    sem = data[:N, 5].astype(np.int32)
    ca = (sem == 1).astype(f32) + 512.0 * (sem == 2)
    cb = (sem == 3).astype(f32) + 512.0 * (sem == 4)
    table[:N, 4] = ca
    table[:N, 5] = cb

    order = np.argsort(lens, kind="stable")
    # rank r: tile t = r // (P*N_CORES); slot s = r % (P*N_CORES)
    # core = s % N_CORES ; partition = s // N_CORES
    Lb = np.zeros(NT, dtype=np.int64)
    for t in range(NT):
        Lb[t] = lens[order[t * P * N_CORES:(t + 1) * P * N_CORES]].max()
    # pass-B groups of 4 tiles share a padded length
    Lg = np.zeros(NG, dtype=np.int64)
    for g in range(NG):
        Lg[g] = Lb[4 * g:4 * g + 4].max()
    Sg = int(Lg.sum() * 4)          # cluster-major columns per partition

    chunks = np.maximum(1, (Lb + 127) // 128)     # 1 or 2 per tile
    n2 = int((chunks == 2).sum())                 # trailing tiles (sorted)
    t2_start = NT - n2
    ncol_p = NT + n2                              # columns per partition
    NCOL = P * ncol_p

    ar = np.arange(L)[None, :]
    idx_pad = np.where(ar < lens[:, None], clust_idx, N)

    ids = np.zeros((N_CORES, NT, P), dtype=np.int64)
    nvecs = np.zeros((N_CORES, P, NT), dtype=f32)
    # transposed stream: [core][128 slots, 6 planes * NCOL] plane-major
    tstr = np.zeros((N_CORES, P, 6 * NCOL), dtype=f16)
    # cluster-major stream: [core][128, 3 * Sg] plane-major, group-padded
    cstr = np.zeros((N_CORES, P, 3 * Sg), dtype=f16)

    goff = np.zeros(NG, dtype=np.int64)
    off = 0
    for g in range(NG):
        goff[g] = off
        off += 4 * int(Lg[g])

    tv = tstr.reshape(N_CORES, P, 6, P, ncol_p)
    cv = cstr.reshape(N_CORES, P, 3, Sg)
    for t in range(NT):
        base = t * P * N_CORES
        g, tg = t // 4, t % 4
        lg = int(Lg[g])
        lb = int(Lb[t])
        c1 = min(lb, 128)
        for core in range(N_CORES):
            sel = order[base + core + N_CORES * np.arange(P)]
            ids[core, t] = sel
            nvecs[core, :, t] = lens[sel]
            feats = table[idx_pad[sel, :lb]]          # [P, lb, 6]
            # transposed: chunk 0 -> col t, chunk 1 -> col NT + (t - t2_start)
            blk = np.zeros((P, 128, 6), dtype=f32)
            blk[:, :c1] = feats[:, :c1]
            tv[core, :, :, :, t] = blk.transpose(1, 2, 0).astype(f16)
            if lb > 128:
                blk2 = np.zeros((P, 128, 6), dtype=f32)
                blk2[:, :lb - 128] = feats[:, 128:lb]
                tv[core, :, :, :, NT + (t - t2_start)] = (
                    blk2.transpose(1, 2, 0).astype(f16))
            # cluster-major x/16,y/16,z/16 planes, group layout
            s0 = int(goff[g]) + tg * lg
            cv[core, :, :, s0:s0 + lb] = (
                feats[:, :, 0:3].transpose(0, 2, 1).astype(f16))
    return dict(tstr=tstr, cstr=cstr, nvecs=nvecs, ids=ids, Lb=Lb, Lg=Lg,
                Sg=Sg, ncol_p=ncol_p, NCOL=NCOL, n2=n2, t2_start=t2_start,
                goff=goff)


def _build_program(meta):
    import concourse.bass as bass
    import concourse.bacc as bacc
    import concourse.mybir as mybir
    from concourse.tile import TileContext

    dt = mybir.dt
    Alu = mybir.AluOpType
    Act = mybir.ActivationFunctionType

    Lb = meta["Lb"]; Lg = meta["Lg"]; Sg = meta["Sg"]
    ncol_p = meta["ncol_p"]; NCOL = meta["NCOL"]
    n2 = meta["n2"]; t2s = meta["t2_start"]; goff = meta["goff"]
    NM = 13                       # moment planes

    nc = bacc.Bacc("TRN2", target_bir_lowering=False, debug=False,
                   enable_asserts=False)
    tstr = nc.dram_tensor("tstr", [P, 6 * NCOL], dt.float16,
                          kind="ExternalInput")
    cstr = nc.dram_tensor("cstr", [P, 3 * Sg], dt.float16,
                          kind="ExternalInput")
    nvec_d = nc.dram_tensor("nvec", [P, NT], dt.float32, kind="ExternalInput")
    res = nc.dram_tensor("res", [P, 19 * NT], dt.float32,
                         kind="ExternalOutput")
    resdbg = nc.dram_tensor("resdbg", [P, 13 * NT], dt.float32,
                            kind="ExternalOutput")

    TINY = 1e-30
    NBLK = 8
    BLK = NCOL // NBLK            # column split for SBUF residency

    with TileContext(nc) as tc:
        with tc.tile_pool(name="ret", bufs=1) as ret, \
             tc.tile_pool(name="ps", bufs=1, space="PSUM") as ps:

            def full_tile(tag, k=1):
                return ret.tile([P, k * NT], dt.float32, tag=tag, name=tag)

            NV = full_tile("NV")
            RN = full_tile("RN")
            RAWM = ret.tile([P, NM * NT], dt.float32, tag="RAWM", name="RAWM")
            SCRAW = full_tile("SCRAW")
            MEANV = full_tile("MEANV"); STDV = full_tile("STDV")
            MODE = full_tile("MODE")
            B6 = full_tile("B6", 6)
            V3 = full_tile("V3", 3)
            CEN = full_tile("CEN", 3)
            NCEN = full_tile("NCEN", 3)
            STAIR = ret.tile([P, 256], dt.float16, tag="STAIR", name="STAIR")

            nc.sync.dma_start(out=NV[:], in_=nvec_d[:, :])
            nc.vector.reciprocal(RN[:], NV[:])
            nc.vector.memset(STAIR[:], 0.0)
            nc.vector.memset(STAIR[:, 128:129], 1.0)

            def tt(op, out, a, b):
                nc.vector.tensor_tensor(out=out, in0=a, in1=b, op=op)

            def ts(out, in0, s, op, s2=None, op1=None):
                kw = {}
                if op1 is not None:
                    kw["op1"] = op1
                nc.vector.tensor_scalar(out=out, in0=in0, scalar1=s,
                                        scalar2=s2, op0=op, **kw)

            def stt(out, in0, s, op0, op1, in1, accum=None):
                nc.vector.scalar_tensor_tensor(out=out, in0=in0, scalar=s,
                                               in1=in1, op0=op0, op1=op1,
                                               accum_out=accum)

            def act(out, in_, func, bias=0.0, scale=1.0, accum=None):
                nc.scalar.activation(out, in_, func, bias=bias, scale=scale,
                                     accum_out=accum)

            # ---------------- Pass A: PE moment sums -----------------
            PS = ps.tile([P, NM * NT], dt.float32, tag="PS", name="PS")
            first = [True]

            def pass_a_blk(h, hp):
                c0, c1 = h * BLK, (h + 1) * BLK
                W = c1 - c0
                # combined tile: 6 raw planes + 7 product planes
                al = hp.tile([P, 13 * W], dt.float16, tag="al",
                             name=f"al{h}")
                alv = al[:].rearrange("k (f c) -> k f c", f=13)
                nc.sync.dma_start(
                    out=alv[:, 0:6, :],
                    in_=tstr[:, :].rearrange("k (f c) -> k f c", f=6)[:, :, c0:c1])
                # squares of x,y,z,v in one ACT instruction
                act(al[:, 6 * W:10 * W], al[:, 0:4 * W], Act.Square)
                # crosses xy, xz, yz on DVE (2x fp16)
                tt(Alu.mult, alv[:, 10, :], alv[:, 0, :], alv[:, 1, :])
                tt(Alu.mult, alv[:, 11, :], alv[:, 0, :], alv[:, 2, :])
                tt(Alu.mult, alv[:, 12, :], alv[:, 1, :], alv[:, 2, :])

                # per-partition matmuls (staircase window -> PSUM row p)
                p0, p1 = c0 // ncol_p, c1 // ncol_p
                psv = PS[:].rearrange("p (m t) -> p m t", m=NM)
                for p in range(p0, p1):
                    rb = p * ncol_p - c0
                    r13 = alv[:, :, rb:rb + ncol_p]
                    lhs = STAIR[:, 128 - p:256 - p]
                    st = first[0]
                    first[0] = False
                    last = (p == P - 1)
                    # start resets the WHOLE psum bank: only the very
                    # first matmul may carry start=True
                    nc.tensor.matmul(psv[:, :, 0:NT], lhs, r13[:, :, 0:NT],
                                     start=st, stop=False,
                                     skip_group_check=True)
                    if n2 > 0:
                        nc.tensor.matmul(psv[:, :, t2s:NT], lhs,
                                         r13[:, :, NT:ncol_p],
                                         start=False, stop=last,
                                         skip_group_check=True)
                    elif last:
                        nc.tensor.matmul(psv[:, 0:1, NT - 1:NT],
                                         STAIR[:, 0:1].broadcast_to((P, 1)),
                                         STAIR[:, 0:1], start=False, stop=True)

            with tc.tile_pool(name="blk", bufs=3) as hp:
                for h in range(NBLK):
                    pass_a_blk(h, hp)
            nc.vector.tensor_copy(out=RAWM[:], in_=PS[:])
            nc.sync.dma_start(out=resdbg[:, :], in_=RAWM[:])

            def msl(m):
                return RAWM[:, m * NT:(m + 1) * NT]

            # ---------------- cluster math ----------------------------
            def cluster_math():
                def tmp(tag, k=1):
                    return ret.tile([P, k * NT], dt.float32, tag=tag, name=tag)

                def sl(T, i):
                    return T[:, i * NT:(i + 1) * NT]

                SC1 = tmp("SC1"); SC2 = tmp("SC2"); SC3 = tmp("SC3")
                # centers (scaled units): c' = sum(x')/n
                for i in range(3):
                    tt(Alu.mult, sl(CEN, i), msl(i), RN[:])
                    ts(sl(NCEN, i), sl(CEN, i), -1.0, Alu.mult)
                # A' = prod - cen*sum  (xx,xy,xz,yy,yz,zz in cmap order)
                A = tmp("A", 6)
                # raw plane order: 6=xx,7=yy,8=zz,9=vv,10=xy,11=xz,12=yz
                pmap = [(0, 6, 0, 0), (1, 10, 0, 1), (2, 11, 0, 2),
                        (3, 7, 1, 1), (4, 12, 1, 2), (5, 8, 2, 2)]
                for q, pm, i, j in pmap:
                    tt(Alu.mult, SC1[:], sl(CEN, i), msl(j))
                    tt(Alu.subtract, sl(A, q), msl(pm), SC1[:])

                # value stats: meanv = sum(v)/n ; var = (sum(v^2)-mean*sum)/ (n-1)
                VAR = tmp("VAR"); NM1 = tmp("NM1")
                tt(Alu.mult, MEANV[:], msl(3), RN[:])
                tt(Alu.mult, VAR[:], MEANV[:], msl(3))
                tt(Alu.subtract, VAR[:], msl(9), VAR[:])
                ts(NM1[:], NV[:], 1.0, Alu.subtract)
                nc.vector.reciprocal(SC1[:], NM1[:])
                tt(Alu.mult, VAR[:], VAR[:], SC1[:])
                ts(VAR[:], VAR[:], 0.0, Alu.max)
                act(STDV[:], VAR[:], Act.Sqrt)

                # unpack semantic counts: ca -> c1 + 512*c2, cb -> c3 + 512*c4
                CNT = tmp("CNT", 4)
                HI_I = ret.tile([P, 2 * NT], dt.int32, tag="HI_I", name="HI_I")
                HIF = tmp("HIF", 2)
                for k, src in ((0, msl(4)), (1, msl(5))):
                    ts(sl(HIF, k), src, 1.0 / 512.0, Alu.mult)
                nc.vector.tensor_copy(out=HI_I[:], in_=HIF[:])
                nc.vector.tensor_copy(out=HIF[:], in_=HI_I[:])
                # c2 = floor(ca/512); c1 = ca - 512*c2
                for k, src in ((0, msl(4)), (1, msl(5))):
                    ts(SC1[:], sl(HIF, k), -512.0, Alu.mult)
                    tt(Alu.add, sl(CNT, 2 * k), src, SC1[:])
                    nc.vector.tensor_copy(out=sl(CNT, 2 * k + 1), in_=sl(HIF, k))

                BEST = tmp("BEST"); GT = tmp("GT"); KT = tmp("KT")
                tt(Alu.subtract, BEST[:], NV[:], sl(CNT, 0))
                for k in (1, 2, 3):
                    tt(Alu.subtract, BEST[:], BEST[:], sl(CNT, k))
                nc.vector.memset(MODE[:], 0.0)
                for k in range(1, 5):
                    ck = sl(CNT, k - 1)
                    tt(Alu.is_gt, GT[:], ck, BEST[:])
                    nc.vector.tensor_scalar(out=KT[:], in0=MODE[:],
                                            scalar1=-1.0, scalar2=float(k),
                                            op0=Alu.mult, op1=Alu.add)
                    tt(Alu.mult, KT[:], KT[:], GT[:])
                    tt(Alu.add, MODE[:], MODE[:], KT[:])
                    tt(Alu.max, BEST[:], BEST[:], ck)

                # eigenvalues: trig closed form on A'
                Q = tmp("Q"); P1 = tmp("P1"); P2 = tmp("P2"); PP = tmp("PP")
                RP = tmp("RP"); DET = tmp("DET"); RR = tmp("RR"); SS = tmp("SS")
                AT = tmp("AT"); PHI = tmp("PHI")
                W0 = tmp("W0"); W1 = tmp("W1"); W2 = tmp("W2"); RW2 = tmp("RW2")
                DIRWT = tmp("DIRWT")
                NB = tmp("NB", 6)

                tt(Alu.add, Q[:], sl(A, 0), sl(A, 3))
                tt(Alu.add, Q[:], Q[:], sl(A, 5))
                ts(Q[:], Q[:], 1.0 / 3.0, Alu.mult)

                tt(Alu.mult, P1[:], sl(A, 1), sl(A, 1))
                tt(Alu.mult, SC1[:], sl(A, 2), sl(A, 2))
                tt(Alu.add, P1[:], P1[:], SC1[:])
                tt(Alu.mult, SC1[:], sl(A, 4), sl(A, 4))
                tt(Alu.add, P1[:], P1[:], SC1[:])

                BD = tmp("BD", 3)
                tt(Alu.subtract, sl(BD, 0), sl(A, 0), Q[:])
                tt(Alu.subtract, sl(BD, 1), sl(A, 3), Q[:])
                tt(Alu.subtract, sl(BD, 2), sl(A, 5), Q[:])
                tt(Alu.mult, P2[:], sl(BD, 0), sl(BD, 0))
                tt(Alu.mult, SC1[:], sl(BD, 1), sl(BD, 1))
                tt(Alu.add, P2[:], P2[:], SC1[:])
                tt(Alu.mult, SC1[:], sl(BD, 2), sl(BD, 2))
                tt(Alu.add, P2[:], P2[:], SC1[:])
                stt(P2[:], P1[:], 2.0, Alu.mult, Alu.add, P2[:])
                ts(PP[:], P2[:], 1.0 / 6.0, Alu.mult)
                act(PP[:], PP[:], Act.Sqrt)
                ts(SC1[:], PP[:], TINY, Alu.max)
                nc.vector.reciprocal(RP[:], SC1[:])

                tt(Alu.mult, sl(NB, 0), sl(BD, 0), RP[:])
                tt(Alu.mult, sl(NB, 1), sl(A, 1), RP[:])
                tt(Alu.mult, sl(NB, 2), sl(A, 2), RP[:])
                tt(Alu.mult, sl(NB, 3), sl(BD, 1), RP[:])
                tt(Alu.mult, sl(NB, 4), sl(A, 4), RP[:])
                tt(Alu.mult, sl(NB, 5), sl(BD, 2), RP[:])

                tt(Alu.mult, SC1[:], sl(NB, 3), sl(NB, 5))
                tt(Alu.mult, SC2[:], sl(NB, 4), sl(NB, 4))
                tt(Alu.subtract, SC1[:], SC1[:], SC2[:])
                tt(Alu.mult, DET[:], sl(NB, 0), SC1[:])
                tt(Alu.mult, SC1[:], sl(NB, 1), sl(NB, 5))
                tt(Alu.mult, SC2[:], sl(NB, 4), sl(NB, 2))
                tt(Alu.subtract, SC1[:], SC1[:], SC2[:])
                tt(Alu.mult, SC1[:], sl(NB, 1), SC1[:])
                tt(Alu.subtract, DET[:], DET[:], SC1[:])
                tt(Alu.mult, SC1[:], sl(NB, 1), sl(NB, 4))
                tt(Alu.mult, SC2[:], sl(NB, 3), sl(NB, 2))
                tt(Alu.subtract, SC1[:], SC1[:], SC2[:])
                tt(Alu.mult, SC1[:], sl(NB, 2), SC1[:])
                tt(Alu.add, DET[:], DET[:], SC1[:])

                ts(RR[:], DET[:], 0.5, Alu.mult)
                ts(RR[:], RR[:], -1.0, Alu.max)
                ts(RR[:], RR[:], 1.0, Alu.min)
                tt(Alu.mult, SS[:], RR[:], RR[:])
                nc.vector.tensor_scalar(out=SS[:], in0=SS[:], scalar1=-1.0,
                                        scalar2=1.0, op0=Alu.mult, op1=Alu.add)
                ts(SS[:], SS[:], 0.0, Alu.max)
                act(SS[:], SS[:], Act.Sqrt)
                UA = tmp("UA"); UB = tmp("UB")
                ts(SC1[:], RR[:], -1.0, Alu.mult)
                tt(Alu.max, SC1[:], SC1[:], RR[:])
                ts(SS[:], SS[:], TINY, Alu.max)
                nc.vector.reciprocal(SC2[:], SS[:])
                tt(Alu.mult, UA[:], SC1[:], SC2[:])
                ts(SC1[:], UA[:], TINY, Alu.max)
                nc.vector.reciprocal(UB[:], SC1[:])
                tt(Alu.min, SC2[:], UA[:], UB[:])
                act(SC2[:], SC2[:], Act.Arctan)
                ts(SC1[:], UA[:], 1.0, Alu.is_gt)
                nc.vector.tensor_scalar(out=SC3[:], in0=SC2[:], scalar1=-2.0,
                                        scalar2=_PI / 2.0, op0=Alu.mult,
                                        op1=Alu.add)
                tt(Alu.mult, SC3[:], SC3[:], SC1[:])
                tt(Alu.add, SC2[:], SC2[:], SC3[:])
                ts(SC3[:], RR[:], 0.0, Alu.is_lt)
                nc.vector.tensor_scalar(out=SC3[:], in0=SC3[:], scalar1=-2.0,
                                        scalar2=1.0, op0=Alu.mult, op1=Alu.add)
                tt(Alu.mult, AT[:], SC2[:], SC3[:])
                nc.vector.tensor_scalar(out=PHI[:], in0=AT[:],
                                        scalar1=-1.0 / 3.0,
                                        scalar2=_PI / 6.0 + _PI / 2.0,
                                        op0=Alu.mult, op1=Alu.add)
                act(SC1[:], PHI[:], Act.Sin)
                tt(Alu.mult, SC1[:], SC1[:], PP[:])
                stt(W2[:], SC1[:], 2.0, Alu.mult, Alu.add, Q[:])
                nc.vector.tensor_scalar(out=PHI[:], in0=AT[:],
                                        scalar1=-1.0 / 3.0,
                                        scalar2=_PI / 6.0 + _PI / 6.0,
                                        op0=Alu.mult, op1=Alu.add)
                act(SC1[:], PHI[:], Act.Sin)
                tt(Alu.mult, SC1[:], SC1[:], PP[:])
                stt(W0[:], SC1[:], -2.0, Alu.mult, Alu.add, Q[:])
                ts(SC1[:], Q[:], 3.0, Alu.mult)
                tt(Alu.subtract, W1[:], SC1[:], W0[:])
                tt(Alu.subtract, W1[:], W1[:], W2[:])

                ts(SC1[:], W2[:], TINY, Alu.max)
                nc.vector.reciprocal(RW2[:], SC1[:])
                tt(Alu.mult, DIRWT[:], W1[:], RW2[:])
                nc.vector.tensor_scalar(out=DIRWT[:], in0=DIRWT[:],
                                        scalar1=-1.0, scalar2=1.0,
                                        op0=Alu.mult, op1=Alu.add)
                for q in range(6):
                    tt(Alu.mult, sl(B6, q), sl(A, q), RW2[:])

                CD = tmp("CD", 3)
                DD = tmp("DD", 3)
                for qi, ai in enumerate((0, 3, 5)):
                    tt(Alu.subtract, sl(CD, qi), sl(A, ai), W0[:])
                    tt(Alu.subtract, sl(DD, qi), sl(A, ai), W1[:])
                M9 = tmp("M9", 9)

                def mcol(colq, dv):
                    crow = [(sl(CD, 0), sl(A, 1), sl(A, 2)),
                            (sl(A, 1), sl(CD, 1), sl(A, 4)),
                            (sl(A, 2), sl(A, 4), sl(CD, 2))]
                    for r in range(3):
                        a0, a1, a2 = crow[r]
                        tt(Alu.mult, SC1[:], a0, dv[0])
                        tt(Alu.mult, SC2[:], a1, dv[1])
                        tt(Alu.add, SC1[:], SC1[:], SC2[:])
                        tt(Alu.mult, SC2[:], a2, dv[2])
                        tt(Alu.add, sl(M9, colq * 3 + r), SC1[:], SC2[:])

                mcol(0, (sl(DD, 0), sl(A, 1), sl(A, 2)))
                mcol(1, (sl(A, 1), sl(DD, 1), sl(A, 4)))
                mcol(2, (sl(A, 2), sl(A, 4), sl(DD, 2)))

                CN = tmp("CN", 3)
                for j in range(3):
                    tt(Alu.mult, sl(CN, j), sl(M9, j * 3), sl(M9, j * 3))
                    tt(Alu.mult, SC1[:], sl(M9, j * 3 + 1), sl(M9, j * 3 + 1))
                    tt(Alu.add, sl(CN, j), sl(CN, j), SC1[:])
                    tt(Alu.mult, SC1[:], sl(M9, j * 3 + 2), sl(M9, j * 3 + 2))
                    tt(Alu.add, sl(CN, j), sl(CN, j), SC1[:])
                NBEST = tmp("NBEST")
                for i in range(3):
                    nc.vector.tensor_copy(out=sl(V3, i), in_=sl(M9, i))
                nc.vector.tensor_copy(out=NBEST[:], in_=sl(CN, 0))
                for j in (1, 2):
                    tt(Alu.is_gt, GT[:], sl(CN, j), NBEST[:])
                    for i in range(3):
                        tt(Alu.subtract, SC1[:], sl(M9, j * 3 + i), sl(V3, i))
                        tt(Alu.mult, SC1[:], SC1[:], GT[:])
                        tt(Alu.add, sl(V3, i), sl(V3, i), SC1[:])
                    tt(Alu.max, NBEST[:], NBEST[:], sl(CN, j))
                ts(SC1[:], NBEST[:], 1e-37, Alu.max)
                act(SC2[:], SC1[:], Act.Sqrt)
                nc.vector.reciprocal(SC2[:], SC2[:])
                for i in range(3):
                    tt(Alu.mult, sl(V3, i), sl(V3, i), SC2[:])
                return DIRWT

            DIRWT = cluster_math()

            # ---------------- pass B (cluster-major, scaled units) ----
            from contextlib import ExitStack
            _pb_stack = ExitStack()
            pbp = _pb_stack.enter_context(tc.tile_pool(name="pbp", bufs=1))
            pb = _pb_stack.enter_context(tc.tile_pool(name="pb", bufs=2))
            CSTR = pbp.tile([P, 3 * Sg], dt.float16, tag="CSTR", name="CSTR")
            nc.sync.dma_start(out=CSTR[:], in_=cstr[:, :])
            TP = pbp.tile([P, Sg], dt.float16, tag="TP", name="TP")

            def cm_plane(i):   # cluster-major input plane i
                return CSTR[:, i * Sg:(i + 1) * Sg]

            # NK = -(c . v0)  (also reused as T0 in sign phase)
            NK = full_tile("NK")
            SCX = full_tile("SCX")
            tt(Alu.mult, NK[:], CEN[:, 0:NT], V3[:, 0:NT])
            tt(Alu.mult, SCX[:], CEN[:, NT:2 * NT], V3[:, NT:2 * NT])
            tt(Alu.add, NK[:], NK[:], SCX[:])
            tt(Alu.mult, SCX[:], CEN[:, 2 * NT:3 * NT], V3[:, 2 * NT:3 * NT])
            tt(Alu.add, NK[:], NK[:], SCX[:])
            ts(NK[:], NK[:], -1.0, Alu.mult)

            for g in range(NG):
                lg = int(Lg[g]); s0 = int(goff[g]); w = 4 * lg
                SQ3 = pb.tile([P, 3 * w], dt.float16, tag="SQ3", name=f"SQ3{g}")
                QA = pb.tile([P, w], dt.float16, tag="QA", name=f"QA{g}")
                QB = pb.tile([P, w], dt.float16, tag="QB", name=f"QB{g}")
                T2 = pb.tile([P, w], dt.float16, tag="T2", name=f"T2{g}")
                R2 = pb.tile([P, w], dt.float16, tag="R2", name=f"R2{g}")
                RPL = pb.tile([P, w], dt.float16, tag="RPL", name=f"RPL{g}")
                for tg in range(4):
                    t = 4 * g + tg
                    sl0 = s0 + tg * lg
                    # centered squares straight from raw x via ACT bias
                    for i in range(3):
                        act(SQ3[:, i * w + tg * lg:i * w + (tg + 1) * lg],
                            cm_plane(i)[:, sl0:sl0 + lg], Act.Square,
                            bias=NCEN[:, i * NT + t:i * NT + t + 1])
                    # T = x*v0x + NK, += y*v0y, += z*v0z
                    nc.vector.tensor_scalar(
                        out=TP[:, sl0:sl0 + lg],
                        in0=cm_plane(0)[:, sl0:sl0 + lg],
                        scalar1=V3[:, 0 * NT + t:0 * NT + t + 1],
                        scalar2=NK[:, t:t + 1],
                        op0=Alu.mult, op1=Alu.add)
                    stt(TP[:, sl0:sl0 + lg], cm_plane(1)[:, sl0:sl0 + lg],
                        V3[:, 1 * NT + t:1 * NT + t + 1], Alu.mult, Alu.add,
                        TP[:, sl0:sl0 + lg])
                    stt(TP[:, sl0:sl0 + lg], cm_plane(2)[:, sl0:sl0 + lg],
                        V3[:, 2 * NT + t:2 * NT + t + 1], Alu.mult, Alu.add,
                        TP[:, sl0:sl0 + lg])
                # q and r per group; first add on gpsimd
                nc.gpsimd.tensor_tensor(out=QA[:], in0=SQ3[:, 0:w],
                                        in1=SQ3[:, w:2 * w], op=Alu.add)
                tt(Alu.add, QB[:], QA[:], SQ3[:, 2 * w:3 * w])
                nc.gpsimd.tensor_tensor(out=T2[:], in0=TP[:, s0:s0 + w],
                                        in1=TP[:, s0:s0 + w], op=Alu.mult)
                nc.gpsimd.tensor_tensor(out=R2[:], in0=QB[:], in1=T2[:],
                                        op=Alu.subtract)
                ts(R2[:], R2[:], 0.0, Alu.max)
                act(RPL[:], R2[:], Act.Sqrt)
                for tg in range(4):
                    t = 4 * g + tg
                    stt(T2[:, tg * lg:(tg + 1) * lg],
                        TP[:, s0 + tg * lg:s0 + (tg + 1) * lg], 1.0,
                        Alu.mult, Alu.mult,
                        RPL[:, tg * lg:(tg + 1) * lg],
                        accum=SCRAW[:, t:t + 1])

            # ---------------- sign + output --------------------------
            def sign_phase():
                def tmp(tag, k=1):
                    return ret.tile([P, k * NT], dt.float32, tag=tag, name=tag)

                def sl(T, i):
                    return T[:, i * NT:(i + 1) * NT]

                CC = tmp("CC"); R0 = tmp("R0")
                SCV = tmp("SCV"); FAC = tmp("FAC"); SC9 = tmp("SC9")
                GT9 = tmp("GT9"); NPAD = tmp("NPAD")
                T0 = NK
                tt(Alu.mult, CC[:], sl(CEN, 0), sl(CEN, 0))
                tt(Alu.mult, SC9[:], sl(CEN, 1), sl(CEN, 1))
                tt(Alu.add, CC[:], CC[:], SC9[:])
                tt(Alu.mult, SC9[:], sl(CEN, 2), sl(CEN, 2))
                tt(Alu.add, CC[:], CC[:], SC9[:])
                tt(Alu.mult, SC9[:], T0[:], T0[:])
                tt(Alu.subtract, R0[:], CC[:], SC9[:])
                ts(R0[:], R0[:], 0.0, Alu.max)
                act(R0[:], R0[:], Act.Sqrt)
                # padded slots use the group padded length Lg
                for t in range(NT):
                    lg = int(Lg[t // 4])
                    nc.vector.tensor_scalar(
                        out=NPAD[:, t:t + 1],
                        in0=NV[:, t:t + 1], scalar1=-1.0,
                        scalar2=float(lg), op0=Alu.mult, op1=Alu.add)
                tt(Alu.mult, SC9[:], T0[:], R0[:])
                tt(Alu.mult, SC9[:], SC9[:], NPAD[:])
                tt(Alu.subtract, SCV[:], SCRAW[:], SC9[:])
                ts(GT9[:], SCV[:], 0.0, Alu.is_lt)
                nc.vector.tensor_scalar(out=GT9[:], in0=GT9[:], scalar1=-2.0,
                                        scalar2=1.0, op0=Alu.mult, op1=Alu.add)
                tt(Alu.mult, FAC[:], DIRWT[:], GT9[:])
                for i in range(3):
                    tt(Alu.mult, sl(V3, i), sl(V3, i), FAC[:])
                # unscale centers: x16
                for i in range(3):
                    ts(sl(CEN, i), sl(CEN, i), SCL, Alu.mult)
                OUTST = ret.tile([P, 19 * NT], dt.float32, tag="OUTST",
                                 name="OUTST")
                for j, pl in [(0, sl(CEN, 0)), (1, sl(CEN, 1)), (2, sl(CEN, 2)),
                              (3, sl(B6, 0)), (4, sl(B6, 1)), (5, sl(B6, 2)),
                              (6, sl(B6, 1)), (7, sl(B6, 3)), (8, sl(B6, 4)),
                              (9, sl(B6, 2)), (10, sl(B6, 4)), (11, sl(B6, 5)),
                              (12, sl(V3, 0)), (13, sl(V3, 1)), (14, sl(V3, 2)),
                              (15, NV[:]), (16, MEANV[:]), (17, STDV[:]),
                              (18, MODE[:])]:
                    nc.vector.tensor_copy(
                        out=OUTST[:, j * NT:(j + 1) * NT], in_=pl)
                nc.sync.dma_start(out=res[:, :], in_=OUTST[:])

            sign_phase()
            _pb_stack.close()

    nc.compile()
    return nc


_cache = {}
_last = None


def kernel(data, clust_idx, clust_len):
    global N, C, L, NT, NG
    data = np.asarray(data)
    clust_idx = np.asarray(clust_idx)
    N = int(data.shape[0])
    C, L = int(clust_idx.shape[0]), int(clust_idx.shape[1])
    assert C % (P * N_CORES) == 0
    NT = C // (P * N_CORES)
    NG = NT // 4
    meta = _host_prep(data, clust_idx, clust_len)

    key = (tuple(int(x) for x in meta["Lb"]), N, C)
    if key not in _cache:
        _cache[key] = _build_program(meta)
    nc = _cache[key]

    from concourse.bass_utils import run_bass_kernel_spmd
    in_maps = [{"tstr": meta["tstr"][c], "cstr": meta["cstr"][c],
                "nvec": meta["nvecs"][c]} for c in range(N_CORES)]
    global _last
    _last = (nc, in_maps)
    res = run_bass_kernel_spmd(nc, in_maps, list(range(N_CORES)))

    ids = meta["ids"]
    out = np.zeros((C, 19), dtype=f32)
    for core in range(N_CORES):
        r = res.results[core]["res"].reshape(P, 19, NT)
        for t in range(NT):
            out[ids[core, t]] = r[:, :, t]
    return out


# revision 12
# speedup vs baseline: 1.1497x; 1.1497x over previous
"""Trainium2 Bass kernel for nn_ClustGeoNodeEncoder (segment_reduce).

v2 architecture (PE-accelerated moments):
  - Host sorts clusters by length, deals them round-robin to 8 cores x 32
    tiles of 128 clusters (one cluster per partition per tile), and stages
    TWO fp16 streams per core:
      * transposed stream: [128 element-slots, ncols] feature planes
        (x/16, y/16, z/16, v, ca, cb) where each column holds up to 128
        elements of one cluster chunk (2 chunks for tiles padded > 128).
        ca = oh1 + 512*oh2, cb = oh3 + 512*oh4 pack the semantic one-hots
        (exact in fp16; sums stay < 2^24 so fp32 PSUM accumulation is
        exact).  Columns are ordered partition-major so partition p's
        clusters occupy a contiguous 13*NT-column window.
      * cluster-major stream: [128 clusters, S] x/16, y/16, z/16 planes,
        feature-major per group of 4 equal-padded tiles (for pass B).
  - Device pass A: ACT squares the coordinate/value planes, DVE forms the
    three cross-product planes (2x fp16 mode), and the TensorEngine
    reduces all 13 moment planes per cluster with ones-column matmuls:
    a staircase window (ones only in absolute column 128 of a [128, 256]
    buffer) places partition p's sums into PSUM row p; 128 accumulating
    matmuls cover all partitions, long-tile second chunks accumulate into
    the same PSUM columns.  One [128, 13*NT] PSUM->SBUF copy evacuates
    every raw moment.
  - Cluster math on [128, NT] fp32 planes: centers, centered scatter
    matrix A (scale-free in /16 units), closed-form trig eigenvalues,
    principal eigenvector via spectral projector, B = A/w2, dirwt, value
    stats, semantic mode via int32-truncation unpack of ca/cb.
  - Pass B (cluster-major): ts-centering (4x fp16), per-tile stt dot with
    v0, ACT group squares + sqrt, stt-accum orientation statistic sc;
    padded-slot closed-form correction, sign flip, output DMA.
"""

import sys

for _p in ("/opt/trn_rl_repo",):
    if _p not in sys.path:
        sys.path.insert(0, _p)

import numpy as np

N = 2_000_000
C = 32768
L = 256
N_CORES = 8
P = 128
NT = C // (P * N_CORES)  # 32 tiles per core
NG = 8                   # pass-B tile groups (4 tiles each, shared pad)
f32 = np.float32
f16 = np.float16

_PI = float(np.pi)
SCL = 16.0               # coordinate pre-scale (powers of 2 are exact)


def _host_prep(data, clust_idx, clust_len):
    data = np.asarray(data, dtype=f32)
    clust_idx = np.asarray(clust_idx).astype(np.int64)
    lens = np.asarray(clust_len).astype(np.int64)

    # feature table: x/16, y/16, z/16, v, ca, cb ; row N = zeros for padding
    table = np.zeros((N + 1, 6), dtype=f32)
    table[:N, 0:3] = data[:, 0:3] / SCL
    table[:N, 3] = data[:, 4]
    sem = data[:N, 5].astype(np.int32)
    ca = (sem == 1).astype(f32) + 512.0 * (sem == 2)
    cb = (sem == 3).astype(f32) + 512.0 * (sem == 4)
    table[:N, 4] = ca
    table[:N, 5] = cb

    order = np.argsort(lens, kind="stable")
    # rank r: tile t = r // (P*N_CORES); slot s = r % (P*N_CORES)
    # core = s % N_CORES ; partition = s // N_CORES
    Lb = np.zeros(NT, dtype=np.int64)
    for t in range(NT):
        Lb[t] = lens[order[t * P * N_CORES:(t + 1) * P * N_CORES]].max()
    # pass-B groups of 4 tiles share a padded length
    Lg = np.zeros(NG, dtype=np.int64)
    for g in range(NG):
        Lg[g] = Lb[4 * g:4 * g + 4].max()
    Sg = int(Lg.sum() * 4)          # cluster-major columns per partition

    chunks = np.maximum(1, (Lb + 127) // 128)     # 1 or 2 per tile
    n2 = int((chunks == 2).sum())                 # trailing tiles (sorted)
    t2_start = NT - n2
    ncol_p = NT + n2                              # columns per partition
    NCOL = P * ncol_p

    ar = np.arange(L)[None, :]
    idx_pad = np.where(ar < lens[:, None], clust_idx, N)

    ids = np.zeros((N_CORES, NT, P), dtype=np.int64)
    nvecs = np.zeros((N_CORES, P, NT), dtype=f32)
    # transposed stream: [core][128 slots, 6 planes * NCOL] plane-major
    tstr = np.zeros((N_CORES, P, 6 * NCOL), dtype=f16)
    # cluster-major stream: [core][128, 3 * Sg] plane-major, group-padded
    cstr = np.zeros((N_CORES, P, 3 * Sg), dtype=f16)

    goff = np.zeros(NG, dtype=np.int64)
    off = 0
    for g in range(NG):
        goff[g] = off
        off += 4 * int(Lg[g])

    tv = tstr.reshape(N_CORES, P, 6, P, ncol_p)
    cv = cstr.reshape(N_CORES, P, 3, Sg)
    for t in range(NT):
        base = t * P * N_CORES
        g, tg = t // 4, t % 4
        lg = int(Lg[g])
        lb = int(Lb[t])
        c1 = min(lb, 128)
        for core in range(N_CORES):
            sel = order[base + core + N_CORES * np.arange(P)]
            ids[core, t] = sel
            nvecs[core, :, t] = lens[sel]
            feats = table[idx_pad[sel, :lb]]          # [P, lb, 6]
            # transposed: chunk 0 -> col t, chunk 1 -> col NT + (t - t2_start)
            blk = np.zeros((P, 128, 6), dtype=f32)
            blk[:, :c1] = feats[:, :c1]
            tv[core, :, :, :, t] = blk.transpose(1, 2, 0).astype(f16)
            if lb > 128:
                blk2 = np.zeros((P, 128, 6), dtype=f32)
                blk2[:, :lb - 128] = feats[:, 128:lb]
                tv[core, :, :, :, NT + (t - t2_start)] = (
                    blk2.transpose(1, 2, 0).astype(f16))
            # cluster-major x/16,y/16,z/16 planes, group layout
            s0 = int(goff[g]) + tg * lg
            cv[core, :, :, s0:s0 + lb] = (
                feats[:, :, 0:3].transpose(0, 2, 1).astype(f16))
    return dict(tstr=tstr, cstr=cstr, nvecs=nvecs, ids=ids, Lb=Lb, Lg=Lg,
                Sg=Sg, ncol_p=ncol_p, NCOL=NCOL, n2=n2, t2_start=t2_start,
                goff=goff)


def _build_program(meta):
    import concourse.bass as bass
    import concourse.bacc as bacc
    import concourse.mybir as mybir
    from concourse.tile import TileContext

    dt = mybir.dt
    Alu = mybir.AluOpType
    Act = mybir.ActivationFunctionType

    Lb = meta["Lb"]; Lg = meta["Lg"]; Sg = meta["Sg"]
    ncol_p = meta["ncol_p"]; NCOL = meta["NCOL"]
    n2 = meta["n2"]; t2s = meta["t2_start"]; goff = meta["goff"]
    NM = 13                       # moment planes

    nc = bacc.Bacc("TRN2", target_bir_lowering=False, debug=False,
                   enable_asserts=False)
    tstr = nc.dram_tensor("tstr", [P, 6 * NCOL], dt.float16,
                          kind="ExternalInput")
    cstr = nc.dram_tensor("cstr", [P, 3 * Sg], dt.float16,
                          kind="ExternalInput")
    nvec_d = nc.dram_tensor("nvec", [P, NT], dt.float32, kind="ExternalInput")
    res = nc.dram_tensor("res", [P, 19 * NT], dt.float32,
                         kind="ExternalOutput")
    resdbg = nc.dram_tensor("resdbg", [P, 13 * NT], dt.float32,
                            kind="ExternalOutput")

    TINY = 1e-30
    NBLK = 2
    BLK = NCOL // NBLK            # column split for SBUF residency

    with TileContext(nc) as tc:
        with tc.tile_pool(name="ret", bufs=1) as ret, \
             tc.tile_pool(name="ps", bufs=1, space="PSUM") as ps:

            def full_tile(tag, k=1):
                return ret.tile([P, k * NT], dt.float32, tag=tag, name=tag)

            NV = full_tile("NV")
            RN = full_tile("RN")
            RAWM = ret.tile([P, NM * NT], dt.float32, tag="RAWM", name="RAWM")
            SCRAW = full_tile("SCRAW")
            MEANV = full_tile("MEANV"); STDV = full_tile("STDV")
            MODE = full_tile("MODE")
            B6 = full_tile("B6", 6)
            V3 = full_tile("V3", 3)
            CEN = full_tile("CEN", 3)
            NCEN = full_tile("NCEN", 3)
            STAIR = ret.tile([P, 256], dt.float16, tag="STAIR", name="STAIR")

            nc.sync.dma_start(out=NV[:], in_=nvec_d[:, :])
            nc.vector.reciprocal(RN[:], NV[:])
            nc.vector.memset(STAIR[:], 0.0)
            nc.vector.memset(STAIR[:, 128:129], 1.0)

            def tt(op, out, a, b):
                nc.vector.tensor_tensor(out=out, in0=a, in1=b, op=op)

            def ts(out, in0, s, op, s2=None, op1=None):
                kw = {}
                if op1 is not None:
                    kw["op1"] = op1
                nc.vector.tensor_scalar(out=out, in0=in0, scalar1=s,
                                        scalar2=s2, op0=op, **kw)

            def stt(out, in0, s, op0, op1, in1, accum=None):
                nc.vector.scalar_tensor_tensor(out=out, in0=in0, scalar=s,
                                               in1=in1, op0=op0, op1=op1,
                                               accum_out=accum)

            def act(out, in_, func, bias=0.0, scale=1.0, accum=None):
                nc.scalar.activation(out, in_, func, bias=bias, scale=scale,
                                     accum_out=accum)

            # ---------------- Pass A: PE moment sums -----------------
            PS = ps.tile([P, NM * NT], dt.float32, tag="PS", name="PS")
            first = [True]

            def pass_a_blk(h, hp):
                c0, c1 = h * BLK, (h + 1) * BLK
                W = c1 - c0
                # combined tile: 6 raw planes + 7 product planes
                al = hp.tile([P, 13 * W], dt.float16, tag="al",
                             name=f"al{h}")
                alv = al[:].rearrange("k (f c) -> k f c", f=13)
                nc.sync.dma_start(
                    out=alv[:, 0:6, :],
                    in_=tstr[:, :].rearrange("k (f c) -> k f c", f=6)[:, :, c0:c1])
                # squares of x,y,z,v in one ACT instruction
                act(al[:, 6 * W:10 * W], al[:, 0:4 * W], Act.Square)
                # crosses xy, xz, yz on DVE (2x fp16)
                tt(Alu.mult, alv[:, 10, :], alv[:, 0, :], alv[:, 1, :])
                tt(Alu.mult, alv[:, 11, :], alv[:, 0, :], alv[:, 2, :])
                tt(Alu.mult, alv[:, 12, :], alv[:, 1, :], alv[:, 2, :])

                # per-partition matmuls (staircase window -> PSUM row p)
                p0, p1 = c0 // ncol_p, c1 // ncol_p
                psv = PS[:].rearrange("p (m t) -> p m t", m=NM)
                for p in range(p0, p1):
                    rb = p * ncol_p - c0
                    r13 = alv[:, :, rb:rb + ncol_p]
                    lhs = STAIR[:, 128 - p:256 - p]
                    st = first[0]
                    first[0] = False
                    last = (p == P - 1)
                    # start resets the WHOLE psum bank: only the very
                    # first matmul may carry start=True
                    nc.tensor.matmul(psv[:, :, 0:NT], lhs, r13[:, :, 0:NT],
                                     start=st, stop=False,
                                     skip_group_check=True)
                    if n2 > 0:
                        nc.tensor.matmul(psv[:, :, t2s:NT], lhs,
                                         r13[:, :, NT:ncol_p],
                                         start=False, stop=last,
                                         skip_group_check=True)
                    elif last:
                        nc.tensor.matmul(psv[:, 0:1, NT - 1:NT],
                                         STAIR[:, 0:1].broadcast_to((P, 1)),
                                         STAIR[:, 0:1], start=False, stop=True)

            with tc.tile_pool(name="blk", bufs=2) as hp:
                for h in range(NBLK):
                    pass_a_blk(h, hp)
            nc.vector.tensor_copy(out=RAWM[:], in_=PS[:])
            nc.sync.dma_start(out=resdbg[:, :], in_=RAWM[:])

            def msl(m):
                return RAWM[:, m * NT:(m + 1) * NT]

            # ---------------- cluster math ----------------------------
            def cluster_math():
                def tmp(tag, k=1):
                    return ret.tile([P, k * NT], dt.float32, tag=tag, name=tag)

                def sl(T, i):
                    return T[:, i * NT:(i + 1) * NT]

                SC1 = tmp("SC1"); SC2 = tmp("SC2"); SC3 = tmp("SC3")
                # centers (scaled units): c' = sum(x')/n
                for i in range(3):
                    tt(Alu.mult, sl(CEN, i), msl(i), RN[:])
                    ts(sl(NCEN, i), sl(CEN, i), -1.0, Alu.mult)
                # A' = prod - cen*sum  (xx,xy,xz,yy,yz,zz in cmap order)
                A = tmp("A", 6)
                # raw plane order: 6=xx,7=yy,8=zz,9=vv,10=xy,11=xz,12=yz
                pmap = [(0, 6, 0, 0), (1, 10, 0, 1), (2, 11, 0, 2),
                        (3, 7, 1, 1), (4, 12, 1, 2), (5, 8, 2, 2)]
                for q, pm, i, j in pmap:
                    tt(Alu.mult, SC1[:], sl(CEN, i), msl(j))
                    tt(Alu.subtract, sl(A, q), msl(pm), SC1[:])

                # value stats: meanv = sum(v)/n ; var = (sum(v^2)-mean*sum)/ (n-1)
                VAR = tmp("VAR"); NM1 = tmp("NM1")
                tt(Alu.mult, MEANV[:], msl(3), RN[:])
                tt(Alu.mult, VAR[:], MEANV[:], msl(3))
                tt(Alu.subtract, VAR[:], msl(9), VAR[:])
                ts(NM1[:], NV[:], 1.0, Alu.subtract)
                nc.vector.reciprocal(SC1[:], NM1[:])
                tt(Alu.mult, VAR[:], VAR[:], SC1[:])
                ts(VAR[:], VAR[:], 0.0, Alu.max)
                act(STDV[:], VAR[:], Act.Sqrt)

                # unpack semantic counts: ca -> c1 + 512*c2, cb -> c3 + 512*c4
                CNT = tmp("CNT", 4)
                HI_I = ret.tile([P, 2 * NT], dt.int32, tag="HI_I", name="HI_I")
                HIF = tmp("HIF", 2)
                for k, src in ((0, msl(4)), (1, msl(5))):
                    ts(sl(HIF, k), src, 1.0 / 512.0, Alu.mult)
                nc.vector.tensor_copy(out=HI_I[:], in_=HIF[:])
                nc.vector.tensor_copy(out=HIF[:], in_=HI_I[:])
                # c2 = floor(ca/512); c1 = ca - 512*c2
                for k, src in ((0, msl(4)), (1, msl(5))):
                    ts(SC1[:], sl(HIF, k), -512.0, Alu.mult)
                    tt(Alu.add, sl(CNT, 2 * k), src, SC1[:])
                    nc.vector.tensor_copy(out=sl(CNT, 2 * k + 1), in_=sl(HIF, k))

                BEST = tmp("BEST"); GT = tmp("GT"); KT = tmp("KT")
                tt(Alu.subtract, BEST[:], NV[:], sl(CNT, 0))
                for k in (1, 2, 3):
                    tt(Alu.subtract, BEST[:], BEST[:], sl(CNT, k))
                nc.vector.memset(MODE[:], 0.0)
                for k in range(1, 5):
                    ck = sl(CNT, k - 1)
                    tt(Alu.is_gt, GT[:], ck, BEST[:])
                    nc.vector.tensor_scalar(out=KT[:], in0=MODE[:],
                                            scalar1=-1.0, scalar2=float(k),
                                            op0=Alu.mult, op1=Alu.add)
                    tt(Alu.mult, KT[:], KT[:], GT[:])
                    tt(Alu.add, MODE[:], MODE[:], KT[:])
                    tt(Alu.max, BEST[:], BEST[:], ck)

                # eigenvalues: trig closed form on A'
                Q = tmp("Q"); P1 = tmp("P1"); P2 = tmp("P2"); PP = tmp("PP")
                RP = tmp("RP"); DET = tmp("DET"); RR = tmp("RR"); SS = tmp("SS")
                AT = tmp("AT"); PHI = tmp("PHI")
                W0 = tmp("W0"); W1 = tmp("W1"); W2 = tmp("W2"); RW2 = tmp("RW2")
                DIRWT = tmp("DIRWT")
                NB = tmp("NB", 6)

                tt(Alu.add, Q[:], sl(A, 0), sl(A, 3))
                tt(Alu.add, Q[:], Q[:], sl(A, 5))
                ts(Q[:], Q[:], 1.0 / 3.0, Alu.mult)

                tt(Alu.mult, P1[:], sl(A, 1), sl(A, 1))
                tt(Alu.mult, SC1[:], sl(A, 2), sl(A, 2))
                tt(Alu.add, P1[:], P1[:], SC1[:])
                tt(Alu.mult, SC1[:], sl(A, 4), sl(A, 4))
                tt(Alu.add, P1[:], P1[:], SC1[:])

                BD = tmp("BD", 3)
                tt(Alu.subtract, sl(BD, 0), sl(A, 0), Q[:])
                tt(Alu.subtract, sl(BD, 1), sl(A, 3), Q[:])
                tt(Alu.subtract, sl(BD, 2), sl(A, 5), Q[:])
                tt(Alu.mult, P2[:], sl(BD, 0), sl(BD, 0))
                tt(Alu.mult, SC1[:], sl(BD, 1), sl(BD, 1))
                tt(Alu.add, P2[:], P2[:], SC1[:])
                tt(Alu.mult, SC1[:], sl(BD, 2), sl(BD, 2))
                tt(Alu.add, P2[:], P2[:], SC1[:])
                stt(P2[:], P1[:], 2.0, Alu.mult, Alu.add, P2[:])
                ts(PP[:], P2[:], 1.0 / 6.0, Alu.mult)
                act(PP[:], PP[:], Act.Sqrt)
                ts(SC1[:], PP[:], TINY, Alu.max)
                nc.vector.reciprocal(RP[:], SC1[:])

                tt(Alu.mult, sl(NB, 0), sl(BD, 0), RP[:])
                tt(Alu.mult, sl(NB, 1), sl(A, 1), RP[:])
                tt(Alu.mult, sl(NB, 2), sl(A, 2), RP[:])
                tt(Alu.mult, sl(NB, 3), sl(BD, 1), RP[:])
                tt(Alu.mult, sl(NB, 4), sl(A, 4), RP[:])
                tt(Alu.mult, sl(NB, 5), sl(BD, 2), RP[:])

                tt(Alu.mult, SC1[:], sl(NB, 3), sl(NB, 5))
                tt(Alu.mult, SC2[:], sl(NB, 4), sl(NB, 4))
                tt(Alu.subtract, SC1[:], SC1[:], SC2[:])
                tt(Alu.mult, DET[:], sl(NB, 0), SC1[:])
                tt(Alu.mult, SC1[:], sl(NB, 1), sl(NB, 5))
                tt(Alu.mult, SC2[:], sl(NB, 4), sl(NB, 2))
                tt(Alu.subtract, SC1[:], SC1[:], SC2[:])
                tt(Alu.mult, SC1[:], sl(NB, 1), SC1[:])
                tt(Alu.subtract, DET[:], DET[:], SC1[:])
                tt(Alu.mult, SC1[:], sl(NB, 1), sl(NB, 4))
                tt(Alu.mult, SC2[:], sl(NB, 3), sl(NB, 2))
                tt(Alu.subtract, SC1[:], SC1[:], SC2[:])
                tt(Alu.mult, SC1[:], sl(NB, 2), SC1[:])
                tt(Alu.add, DET[:], DET[:], SC1[:])

                ts(RR[:], DET[:], 0.5, Alu.mult)
                ts(RR[:], RR[:], -1.0, Alu.max)
                ts(RR[:], RR[:], 1.0, Alu.min)
                tt(Alu.mult, SS[:], RR[:], RR[:])
                nc.vector.tensor_scalar(out=SS[:], in0=SS[:], scalar1=-1.0,
                                        scalar2=1.0, op0=Alu.mult, op1=Alu.add)
                ts(SS[:], SS[:], 0.0, Alu.max)
                act(SS[:], SS[:], Act.Sqrt)
                UA = tmp("UA"); UB = tmp("UB")
                ts(SC1[:], RR[:], -1.0, Alu.mult)
                tt(Alu.max, SC1[:], SC1[:], RR[:])
                ts(SS[:], SS[:], TINY, Alu.max)
                nc.vector.reciprocal(SC2[:], SS[:])
                tt(Alu.mult, UA[:], SC1[:], SC2[:])
                ts(SC1[:], UA[:], TINY, Alu.max)
                nc.vector.reciprocal(UB[:], SC1[:])
                tt(Alu.min, SC2[:], UA[:], UB[:])
                act(SC2[:], SC2[:], Act.Arctan)
                ts(SC1[:], UA[:], 1.0, Alu.is_gt)
                nc.vector.tensor_scalar(out=SC3[:], in0=SC2[:], scalar1=-2.0,
                                        scalar2=_PI / 2.0, op0=Alu.mult,
                                        op1=Alu.add)
                tt(Alu.mult, SC3[:], SC3[:], SC1[:])
                tt(Alu.add, SC2[:], SC2[:], SC3[:])
                ts(SC3[:], RR[:], 0.0, Alu.is_lt)
                nc.vector.tensor_scalar(out=SC3[:], in0=SC3[:], scalar1=-2.0,
                                        scalar2=1.0, op0=Alu.mult, op1=Alu.add)
                tt(Alu.mult, AT[:], SC2[:], SC3[:])
                nc.vector.tensor_scalar(out=PHI[:], in0=AT[:],
                                        scalar1=-1.0 / 3.0,
                                        scalar2=_PI / 6.0 + _PI / 2.0,
                                        op0=Alu.mult, op1=Alu.add)
                act(SC1[:], PHI[:], Act.Sin)
                tt(Alu.mult, SC1[:], SC1[:], PP[:])
                stt(W2[:], SC1[:], 2.0, Alu.mult, Alu.add, Q[:])
                nc.vector.tensor_scalar(out=PHI[:], in0=AT[:],
                                        scalar1=-1.0 / 3.0,
                                        scalar2=_PI / 6.0 + _PI / 6.0,
                                        op0=Alu.mult, op1=Alu.add)
                act(SC1[:], PHI[:], Act.Sin)
                tt(Alu.mult, SC1[:], SC1[:], PP[:])
                stt(W0[:], SC1[:], -2.0, Alu.mult, Alu.add, Q[:])
                ts(SC1[:], Q[:], 3.0, Alu.mult)
                tt(Alu.subtract, W1[:], SC1[:], W0[:])
                tt(Alu.subtract, W1[:], W1[:], W2[:])

                ts(SC1[:], W2[:], TINY, Alu.max)
                nc.vector.reciprocal(RW2[:], SC1[:])
                tt(Alu.mult, DIRWT[:], W1[:], RW2[:])
                nc.vector.tensor_scalar(out=DIRWT[:], in0=DIRWT[:],
                                        scalar1=-1.0, scalar2=1.0,
                                        op0=Alu.mult, op1=Alu.add)
                for q in range(6):
                    tt(Alu.mult, sl(B6, q), sl(A, q), RW2[:])

                CD = tmp("CD", 3)
                DD = tmp("DD", 3)
                for qi, ai in enumerate((0, 3, 5)):
                    tt(Alu.subtract, sl(CD, qi), sl(A, ai), W0[:])
                    tt(Alu.subtract, sl(DD, qi), sl(A, ai), W1[:])
                M9 = tmp("M9", 9)

                def mcol(colq, dv):
                    crow = [(sl(CD, 0), sl(A, 1), sl(A, 2)),
                            (sl(A, 1), sl(CD, 1), sl(A, 4)),
                            (sl(A, 2), sl(A, 4), sl(CD, 2))]
                    for r in range(3):
                        a0, a1, a2 = crow[r]
                        tt(Alu.mult, SC1[:], a0, dv[0])
                        tt(Alu.mult, SC2[:], a1, dv[1])
                        tt(Alu.add, SC1[:], SC1[:], SC2[:])
                        tt(Alu.mult, SC2[:], a2, dv[2])
                        tt(Alu.add, sl(M9, colq * 3 + r), SC1[:], SC2[:])

                mcol(0, (sl(DD, 0), sl(A, 1), sl(A, 2)))
                mcol(1, (sl(A, 1), sl(DD, 1), sl(A, 4)))
                mcol(2, (sl(A, 2), sl(A, 4), sl(DD, 2)))

                CN = tmp("CN", 3)
                for j in range(3):
                    tt(Alu.mult, sl(CN, j), sl(M9, j * 3), sl(M9, j * 3))
                    tt(Alu.mult, SC1[:], sl(M9, j * 3 + 1), sl(M9, j * 3 + 1))
                    tt(Alu.add, sl(CN, j), sl(CN, j), SC1[:])
                    tt(Alu.mult, SC1[:], sl(M9, j * 3 + 2), sl(M9, j * 3 + 2))
                    tt(Alu.add, sl(CN, j), sl(CN, j), SC1[:])
                NBEST = tmp("NBEST")
                for i in range(3):
                    nc.vector.tensor_copy(out=sl(V3, i), in_=sl(M9, i))
                nc.vector.tensor_copy(out=NBEST[:], in_=sl(CN, 0))
                for j in (1, 2):
                    tt(Alu.is_gt, GT[:], sl(CN, j), NBEST[:])
                    for i in range(3):
                        tt(Alu.subtract, SC1[:], sl(M9, j * 3 + i), sl(V3, i))
                        tt(Alu.mult, SC1[:], SC1[:], GT[:])
                        tt(Alu.add, sl(V3, i), sl(V3, i), SC1[:])
                    tt(Alu.max, NBEST[:], NBEST[:], sl(CN, j))
                ts(SC1[:], NBEST[:], 1e-37, Alu.max)
                act(SC2[:], SC1[:], Act.Sqrt)
                nc.vector.reciprocal(SC2[:], SC2[:])
                for i in range(3):
                    tt(Alu.mult, sl(V3, i), sl(V3, i), SC2[:])
                return DIRWT

            DIRWT = cluster_math()

            # ---------------- pass B (cluster-major, scaled units) ----
            from contextlib import ExitStack
            _pb_stack = ExitStack()
            pbp = _pb_stack.enter_context(tc.tile_pool(name="pbp", bufs=1))
            pb = _pb_stack.enter_context(tc.tile_pool(name="pb", bufs=2))
            CSTR = pbp.tile([P, 3 * Sg], dt.float16, tag="CSTR", name="CSTR")
            nc.sync.dma_start(out=CSTR[:], in_=cstr[:, :])
            TP = pbp.tile([P, Sg], dt.float16, tag="TP", name="TP")

            def cm_plane(i):   # cluster-major input plane i
                return CSTR[:, i * Sg:(i + 1) * Sg]

            # NK = -(c . v0)  (also reused as T0 in sign phase)
            NK = full_tile("NK")
            SCX = full_tile("SCX")
            tt(Alu.mult, NK[:], CEN[:, 0:NT], V3[:, 0:NT])
            tt(Alu.mult, SCX[:], CEN[:, NT:2 * NT], V3[:, NT:2 * NT])
            tt(Alu.add, NK[:], NK[:], SCX[:])
            tt(Alu.mult, SCX[:], CEN[:, 2 * NT:3 * NT], V3[:, 2 * NT:3 * NT])
            tt(Alu.add, NK[:], NK[:], SCX[:])
            ts(NK[:], NK[:], -1.0, Alu.mult)

            for g in range(NG):
                lg = int(Lg[g]); s0 = int(goff[g]); w = 4 * lg
                SQ3 = pb.tile([P, 3 * w], dt.float16, tag="SQ3", name=f"SQ3{g}")
                QA = pb.tile([P, w], dt.float16, tag="QA", name=f"QA{g}")
                QB = pb.tile([P, w], dt.float16, tag="QB", name=f"QB{g}")
                T2 = pb.tile([P, w], dt.float16, tag="T2", name=f"T2{g}")
                R2 = pb.tile([P, w], dt.float16, tag="R2", name=f"R2{g}")
                RPL = pb.tile([P, w], dt.float16, tag="RPL", name=f"RPL{g}")
                for tg in range(4):
                    t = 4 * g + tg
                    sl0 = s0 + tg * lg
                    # centered squares straight from raw x via ACT bias
                    for i in range(3):
                        act(SQ3[:, i * w + tg * lg:i * w + (tg + 1) * lg],
                            cm_plane(i)[:, sl0:sl0 + lg], Act.Square,
                            bias=NCEN[:, i * NT + t:i * NT + t + 1])
                    # T = x*v0x + NK, += y*v0y, += z*v0z
                    nc.vector.tensor_scalar(
                        out=TP[:, sl0:sl0 + lg],
                        in0=cm_plane(0)[:, sl0:sl0 + lg],
                        scalar1=V3[:, 0 * NT + t:0 * NT + t + 1],
                        scalar2=NK[:, t:t + 1],
                        op0=Alu.mult, op1=Alu.add)
                    stt(TP[:, sl0:sl0 + lg], cm_plane(1)[:, sl0:sl0 + lg],
                        V3[:, 1 * NT + t:1 * NT + t + 1], Alu.mult, Alu.add,
                        TP[:, sl0:sl0 + lg])
                    stt(TP[:, sl0:sl0 + lg], cm_plane(2)[:, sl0:sl0 + lg],
                        V3[:, 2 * NT + t:2 * NT + t + 1], Alu.mult, Alu.add,
                        TP[:, sl0:sl0 + lg])
                # q and r per group; first add on gpsimd
                nc.gpsimd.tensor_tensor(out=QA[:], in0=SQ3[:, 0:w],
                                        in1=SQ3[:, w:2 * w], op=Alu.add)
                tt(Alu.add, QB[:], QA[:], SQ3[:, 2 * w:3 * w])
                tt(Alu.mult, T2[:], TP[:, s0:s0 + w], TP[:, s0:s0 + w])
                tt(Alu.subtract, R2[:], QB[:], T2[:])
                ts(R2[:], R2[:], 0.0, Alu.max)
                act(RPL[:], R2[:], Act.Sqrt)
                for tg in range(4):
                    t = 4 * g + tg
                    stt(T2[:, tg * lg:(tg + 1) * lg],
                        TP[:, s0 + tg * lg:s0 + (tg + 1) * lg], 1.0,
                        Alu.mult, Alu.mult,
                        RPL[:, tg * lg:(tg + 1) * lg],
                        accum=SCRAW[:, t:t + 1])

            # ---------------- sign + output --------------------------
            def sign_phase():
                def tmp(tag, k=1):
                    return ret.tile([P, k * NT], dt.float32, tag=tag, name=tag)

                def sl(T, i):
                    return T[:, i * NT:(i + 1) * NT]

                CC = tmp("CC"); R0 = tmp("R0")
                SCV = tmp("SCV"); FAC = tmp("FAC"); SC9 = tmp("SC9")
                GT9 = tmp("GT9"); NPAD = tmp("NPAD")
                T0 = NK
                tt(Alu.mult, CC[:], sl(CEN, 0), sl(CEN, 0))
                tt(Alu.mult, SC9[:], sl(CEN, 1), sl(CEN, 1))
                tt(Alu.add, CC[:], CC[:], SC9[:])
                tt(Alu.mult, SC9[:], sl(CEN, 2), sl(CEN, 2))
                tt(Alu.add, CC[:], CC[:], SC9[:])
                tt(Alu.mult, SC9[:], T0[:], T0[:])
                tt(Alu.subtract, R0[:], CC[:], SC9[:])
                ts(R0[:], R0[:], 0.0, Alu.max)
                act(R0[:], R0[:], Act.Sqrt)
                # padded slots use the group padded length Lg
                for t in range(NT):
                    lg = int(Lg[t // 4])
                    nc.vector.tensor_scalar(
                        out=NPAD[:, t:t + 1],
                        in0=NV[:, t:t + 1], scalar1=-1.0,
                        scalar2=float(lg), op0=Alu.mult, op1=Alu.add)
                tt(Alu.mult, SC9[:], T0[:], R0[:])
                tt(Alu.mult, SC9[:], SC9[:], NPAD[:])
                tt(Alu.subtract, SCV[:], SCRAW[:], SC9[:])
                ts(GT9[:], SCV[:], 0.0, Alu.is_lt)
                nc.vector.tensor_scalar(out=GT9[:], in0=GT9[:], scalar1=-2.0,
                                        scalar2=1.0, op0=Alu.mult, op1=Alu.add)
                tt(Alu.mult, FAC[:], DIRWT[:], GT9[:])
                for i in range(3):
                    tt(Alu.mult, sl(V3, i), sl(V3, i), FAC[:])
                # unscale centers: x16
                for i in range(3):
                    ts(sl(CEN, i), sl(CEN, i), SCL, Alu.mult)
                OUTST = ret.tile([P, 19 * NT], dt.float32, tag="OUTST",
                                 name="OUTST")
                for j, pl in [(0, sl(CEN, 0)), (1, sl(CEN, 1)), (2, sl(CEN, 2)),
                              (3, sl(B6, 0)), (4, sl(B6, 1)), (5, sl(B6, 2)),
                              (6, sl(B6, 1)), (7, sl(B6, 3)), (8, sl(B6, 4)),
                              (9, sl(B6, 2)), (10, sl(B6, 4)), (11, sl(B6, 5)),
                              (12, sl(V3, 0)), (13, sl(V3, 1)), (14, sl(V3, 2)),
                              (15, NV[:]), (16, MEANV[:]), (17, STDV[:]),
                              (18, MODE[:])]:
                    nc.vector.tensor_copy(
                        out=OUTST[:, j * NT:(j + 1) * NT], in_=pl)
                nc.sync.dma_start(out=res[:, :], in_=OUTST[:])

            sign_phase()
            _pb_stack.close()

    nc.compile()
    return nc


_cache = {}
_last = None


def kernel(data, clust_idx, clust_len):
    global N, C, L, NT, NG
    data = np.asarray(data)
    clust_idx = np.asarray(clust_idx)
    N = int(data.shape[0])
    C, L = int(clust_idx.shape[0]), int(clust_idx.shape[1])
    assert C % (P * N_CORES) == 0
    NT = C // (P * N_CORES)
    NG = NT // 4
    meta = _host_prep(data, clust_idx, clust_len)

    key = (tuple(int(x) for x in meta["Lb"]), N, C)
    if key not in _cache:
        _cache[key] = _build_program(meta)
    nc = _cache[key]

    from concourse.bass_utils import run_bass_kernel_spmd
    in_maps = [{"tstr": meta["tstr"][c], "cstr": meta["cstr"][c],
                "nvec": meta["nvecs"][c]} for c in range(N_CORES)]
    global _last
    _last = (nc, in_maps)
    res = run_bass_kernel_spmd(nc, in_maps, list(range(N_CORES)))

    ids = meta["ids"]
    out = np.zeros((C, 19), dtype=f32)
    for core in range(N_CORES):
        r = res.results[core]["res"].reshape(P, 19, NT)
        for t in range(NT):
            out[ids[core, t]] = r[:, :, t]
    return out


# revision 13
# speedup vs baseline: 1.2185x; 1.0599x over previous
"""Trainium2 Bass kernel for nn_ClustGeoNodeEncoder (segment_reduce).

v2 architecture (PE-accelerated moments):
  - Host sorts clusters by length, deals them round-robin to 8 cores x 32
    tiles of 128 clusters (one cluster per partition per tile), and stages
    TWO fp16 streams per core:
      * transposed stream: [128 element-slots, ncols] feature planes
        (x/16, y/16, z/16, v, ca, cb) where each column holds up to 128
        elements of one cluster chunk (2 chunks for tiles padded > 128).
        ca = oh1 + 512*oh2, cb = oh3 + 512*oh4 pack the semantic one-hots
        (exact in fp16; sums stay < 2^24 so fp32 PSUM accumulation is
        exact).  Columns are ordered partition-major so partition p's
        clusters occupy a contiguous 13*NT-column window.
      * cluster-major stream: [128 clusters, S] x/16, y/16, z/16 planes,
        feature-major per group of 4 equal-padded tiles (for pass B).
  - Device pass A: ACT squares the coordinate/value planes, DVE forms the
    three cross-product planes (2x fp16 mode), and the TensorEngine
    reduces all 13 moment planes per cluster with ones-column matmuls:
    a staircase window (ones only in absolute column 128 of a [128, 256]
    buffer) places partition p's sums into PSUM row p; 128 accumulating
    matmuls cover all partitions, long-tile second chunks accumulate into
    the same PSUM columns.  One [128, 13*NT] PSUM->SBUF copy evacuates
    every raw moment.
  - Cluster math on [128, NT] fp32 planes: centers, centered scatter
    matrix A (scale-free in /16 units), closed-form trig eigenvalues,
    principal eigenvector via spectral projector, B = A/w2, dirwt, value
    stats, semantic mode via int32-truncation unpack of ca/cb.
  - Pass B (cluster-major): ts-centering (4x fp16), per-tile stt dot with
    v0, ACT group squares + sqrt, stt-accum orientation statistic sc;
    padded-slot closed-form correction, sign flip, output DMA.
"""

import sys

for _p in ("/opt/trn_rl_repo",):
    if _p not in sys.path:
        sys.path.insert(0, _p)

import numpy as np

N = 2_000_000
C = 32768
L = 256
N_CORES = 8
P = 128
NT = C // (P * N_CORES)  # 32 tiles per core
NG = 8                   # pass-B tile groups (4 tiles each, shared pad)
f32 = np.float32
f16 = np.float16

_PI = float(np.pi)
SCL = 16.0               # coordinate pre-scale (powers of 2 are exact)


def _host_prep(data, clust_idx, clust_len):
    data = np.asarray(data, dtype=f32)
    clust_idx = np.asarray(clust_idx).astype(np.int64)
    lens = np.asarray(clust_len).astype(np.int64)

    # feature table: x/16, y/16, z/16, v, ca, cb ; row N = zeros for padding
    table = np.zeros((N + 1, 6), dtype=f32)
    table[:N, 0:3] = data[:, 0:3] / SCL
    table[:N, 3] = data[:, 4]
    sem = data[:N, 5].astype(np.int32)
    ca = (sem == 1).astype(f32) + 512.0 * (sem == 2)
    cb = (sem == 3).astype(f32) + 512.0 * (sem == 4)
    table[:N, 4] = ca
    table[:N, 5] = cb

    order = np.argsort(lens, kind="stable")
    # rank r: tile t = r // (P*N_CORES); slot s = r % (P*N_CORES)
    # core = s % N_CORES ; partition = s // N_CORES
    Lb = np.zeros(NT, dtype=np.int64)
    for t in range(NT):
        Lb[t] = lens[order[t * P * N_CORES:(t + 1) * P * N_CORES]].max()
    # pass-B groups of 4 tiles share a padded length
    Lg = np.zeros(NG, dtype=np.int64)
    for g in range(NG):
        Lg[g] = Lb[4 * g:4 * g + 4].max()
    Sg = int(Lg.sum() * 4)          # cluster-major columns per partition

    chunks = np.maximum(1, (Lb + 127) // 128)     # 1 or 2 per tile
    n2 = int((chunks == 2).sum())                 # trailing tiles (sorted)
    t2_start = NT - n2
    ncol_p = NT + n2                              # columns per partition
    NCOL = P * ncol_p

    ar = np.arange(L)[None, :]
    idx_pad = np.where(ar < lens[:, None], clust_idx, N)

    ids = np.zeros((N_CORES, NT, P), dtype=np.int64)
    nvecs = np.zeros((N_CORES, P, NT), dtype=f32)
    # transposed stream: [core][128 slots, 6 planes * NCOL] plane-major
    tstr = np.zeros((N_CORES, P, 6 * NCOL), dtype=f16)
    # cluster-major stream: [core][128, 3 * Sg] plane-major, group-padded
    cstr = np.zeros((N_CORES, P, 3 * Sg), dtype=f16)

    goff = np.zeros(NG, dtype=np.int64)
    off = 0
    for g in range(NG):
        goff[g] = off
        off += 4 * int(Lg[g])

    tv = tstr.reshape(N_CORES, P, 6, P, ncol_p)
    cv = cstr.reshape(N_CORES, P, 3, Sg)
    for t in range(NT):
        base = t * P * N_CORES
        g, tg = t // 4, t % 4
        lg = int(Lg[g])
        lb = int(Lb[t])
        c1 = min(lb, 128)
        for core in range(N_CORES):
            sel = order[base + core + N_CORES * np.arange(P)]
            ids[core, t] = sel
            nvecs[core, :, t] = lens[sel]
            feats = table[idx_pad[sel, :lb]]          # [P, lb, 6]
            # transposed: chunk 0 -> col t, chunk 1 -> col NT + (t - t2_start)
            blk = np.zeros((P, 128, 6), dtype=f32)
            blk[:, :c1] = feats[:, :c1]
            tv[core, :, :, :, t] = blk.transpose(1, 2, 0).astype(f16)
            if lb > 128:
                blk2 = np.zeros((P, 128, 6), dtype=f32)
                blk2[:, :lb - 128] = feats[:, 128:lb]
                tv[core, :, :, :, NT + (t - t2_start)] = (
                    blk2.transpose(1, 2, 0).astype(f16))
            # cluster-major x/16,y/16,z/16 planes, group layout
            s0 = int(goff[g]) + tg * lg
            cv[core, :, :, s0:s0 + lb] = (
                feats[:, :, 0:3].transpose(0, 2, 1).astype(f16))
    return dict(tstr=tstr, cstr=cstr, nvecs=nvecs, ids=ids, Lb=Lb, Lg=Lg,
                Sg=Sg, ncol_p=ncol_p, NCOL=NCOL, n2=n2, t2_start=t2_start,
                goff=goff)


def _build_program(meta):
    import concourse.bass as bass
    import concourse.bacc as bacc
    import concourse.mybir as mybir
    from concourse.tile import TileContext

    dt = mybir.dt
    Alu = mybir.AluOpType
    Act = mybir.ActivationFunctionType

    Lb = meta["Lb"]; Lg = meta["Lg"]; Sg = meta["Sg"]
    ncol_p = meta["ncol_p"]; NCOL = meta["NCOL"]
    n2 = meta["n2"]; t2s = meta["t2_start"]; goff = meta["goff"]
    NM = 13                       # moment planes

    nc = bacc.Bacc("TRN2", target_bir_lowering=False, debug=False,
                   enable_asserts=False)
    tstr = nc.dram_tensor("tstr", [P, 6 * NCOL], dt.float16,
                          kind="ExternalInput")
    cstr = nc.dram_tensor("cstr", [P, 3 * Sg], dt.float16,
                          kind="ExternalInput")
    nvec_d = nc.dram_tensor("nvec", [P, NT], dt.float32, kind="ExternalInput")
    res = nc.dram_tensor("res", [P, 19 * NT], dt.float32,
                         kind="ExternalOutput")
    resdbg = nc.dram_tensor("resdbg", [P, 13 * NT], dt.float32,
                            kind="ExternalOutput")

    TINY = 1e-30
    NBLK = 2
    BLK = NCOL // NBLK            # column split for SBUF residency

    with TileContext(nc) as tc:
        with tc.tile_pool(name="ret", bufs=1) as ret, \
             tc.tile_pool(name="ps", bufs=1, space="PSUM") as ps:

            def full_tile(tag, k=1):
                return ret.tile([P, k * NT], dt.float32, tag=tag, name=tag)

            NV = full_tile("NV")
            RN = full_tile("RN")
            RAWM = ret.tile([P, NM * NT], dt.float32, tag="RAWM", name="RAWM")
            SCRAW = full_tile("SCRAW")
            MEANV = full_tile("MEANV"); STDV = full_tile("STDV")
            MODE = full_tile("MODE")
            B6 = full_tile("B6", 6)
            V3 = full_tile("V3", 3)
            CEN = full_tile("CEN", 3)
            NCEN = full_tile("NCEN", 3)
            STAIR = ret.tile([P, 256], dt.float16, tag="STAIR", name="STAIR")

            nc.sync.dma_start(out=NV[:], in_=nvec_d[:, :])
            nc.vector.reciprocal(RN[:], NV[:])
            nc.vector.memset(STAIR[:], 0.0)
            nc.vector.memset(STAIR[:, 128:129], 1.0)

            def tt(op, out, a, b):
                nc.vector.tensor_tensor(out=out, in0=a, in1=b, op=op)

            def ts(out, in0, s, op, s2=None, op1=None):
                kw = {}
                if op1 is not None:
                    kw["op1"] = op1
                nc.vector.tensor_scalar(out=out, in0=in0, scalar1=s,
                                        scalar2=s2, op0=op, **kw)

            def stt(out, in0, s, op0, op1, in1, accum=None):
                nc.vector.scalar_tensor_tensor(out=out, in0=in0, scalar=s,
                                               in1=in1, op0=op0, op1=op1,
                                               accum_out=accum)

            def act(out, in_, func, bias=0.0, scale=1.0, accum=None):
                nc.scalar.activation(out, in_, func, bias=bias, scale=scale,
                                     accum_out=accum)

            # ---------------- Pass A: PE moment sums -----------------
            PS = ps.tile([P, NM * NT], dt.float32, tag="PS", name="PS")
            first = [True]

            def pass_a_blk(h, hp):
                c0, c1 = h * BLK, (h + 1) * BLK
                W = c1 - c0
                # combined tile: 6 raw planes + 7 product planes
                al = hp.tile([P, 13 * W], dt.float16, tag="al",
                             name=f"al{h}")
                alv = al[:].rearrange("k (f c) -> k f c", f=13)
                nc.sync.dma_start(
                    out=alv[:, 0:6, :],
                    in_=tstr[:, :].rearrange("k (f c) -> k f c", f=6)[:, :, c0:c1])
                # squares + crosses on DVE (2x fp16, idle during pass A);
                # chunked so per-partition matmuls can start early
                NCK = 4
                CW = W // NCK
                for ck in range(NCK):
                    a0, a1 = ck * CW, (ck + 1) * CW
                    tt(Alu.mult, alv[:, 6:10, a0:a1], alv[:, 0:4, a0:a1],
                       alv[:, 0:4, a0:a1])
                    tt(Alu.mult, alv[:, 10, a0:a1], alv[:, 0, a0:a1],
                       alv[:, 1, a0:a1])
                    tt(Alu.mult, alv[:, 11, a0:a1], alv[:, 0, a0:a1],
                       alv[:, 2, a0:a1])
                    tt(Alu.mult, alv[:, 12, a0:a1], alv[:, 1, a0:a1],
                       alv[:, 2, a0:a1])

                # per-partition matmuls (staircase window -> PSUM row p)
                p0, p1 = c0 // ncol_p, c1 // ncol_p
                psv = PS[:].rearrange("p (m t) -> p m t", m=NM)
                for p in range(p0, p1):
                    rb = p * ncol_p - c0
                    r13 = alv[:, :, rb:rb + ncol_p]
                    lhs = STAIR[:, 128 - p:256 - p]
                    st = first[0]
                    first[0] = False
                    last = (p == P - 1)
                    # start resets the WHOLE psum bank: only the very
                    # first matmul may carry start=True
                    nc.tensor.matmul(psv[:, :, 0:NT], lhs, r13[:, :, 0:NT],
                                     start=st, stop=False,
                                     skip_group_check=True)
                    if n2 > 0:
                        nc.tensor.matmul(psv[:, :, t2s:NT], lhs,
                                         r13[:, :, NT:ncol_p],
                                         start=False, stop=last,
                                         skip_group_check=True)
                    elif last:
                        nc.tensor.matmul(psv[:, 0:1, NT - 1:NT],
                                         STAIR[:, 0:1].broadcast_to((P, 1)),
                                         STAIR[:, 0:1], start=False, stop=True)

            with tc.tile_pool(name="blk", bufs=2) as hp:
                for h in range(NBLK):
                    pass_a_blk(h, hp)
            nc.vector.tensor_copy(out=RAWM[:], in_=PS[:])
            nc.sync.dma_start(out=resdbg[:, :], in_=RAWM[:])

            def msl(m):
                return RAWM[:, m * NT:(m + 1) * NT]

            # ---------------- cluster math ----------------------------
            def cluster_math():
                def tmp(tag, k=1):
                    return ret.tile([P, k * NT], dt.float32, tag=tag, name=tag)

                def sl(T, i):
                    return T[:, i * NT:(i + 1) * NT]

                SC1 = tmp("SC1"); SC2 = tmp("SC2"); SC3 = tmp("SC3")
                # centers (scaled units): c' = sum(x')/n
                for i in range(3):
                    tt(Alu.mult, sl(CEN, i), msl(i), RN[:])
                    ts(sl(NCEN, i), sl(CEN, i), -1.0, Alu.mult)
                # A' = prod - cen*sum  (xx,xy,xz,yy,yz,zz in cmap order)
                A = tmp("A", 6)
                # raw plane order: 6=xx,7=yy,8=zz,9=vv,10=xy,11=xz,12=yz
                pmap = [(0, 6, 0, 0), (1, 10, 0, 1), (2, 11, 0, 2),
                        (3, 7, 1, 1), (4, 12, 1, 2), (5, 8, 2, 2)]
                for q, pm, i, j in pmap:
                    tt(Alu.mult, SC1[:], sl(CEN, i), msl(j))
                    tt(Alu.subtract, sl(A, q), msl(pm), SC1[:])

                # value stats: meanv = sum(v)/n ; var = (sum(v^2)-mean*sum)/ (n-1)
                VAR = tmp("VAR"); NM1 = tmp("NM1")
                tt(Alu.mult, MEANV[:], msl(3), RN[:])
                tt(Alu.mult, VAR[:], MEANV[:], msl(3))
                tt(Alu.subtract, VAR[:], msl(9), VAR[:])
                ts(NM1[:], NV[:], 1.0, Alu.subtract)
                nc.vector.reciprocal(SC1[:], NM1[:])
                tt(Alu.mult, VAR[:], VAR[:], SC1[:])
                ts(VAR[:], VAR[:], 0.0, Alu.max)
                act(STDV[:], VAR[:], Act.Sqrt)

                # unpack semantic counts: ca -> c1 + 512*c2, cb -> c3 + 512*c4
                CNT = tmp("CNT", 4)
                HI_I = ret.tile([P, 2 * NT], dt.int32, tag="HI_I", name="HI_I")
                HIF = tmp("HIF", 2)
                for k, src in ((0, msl(4)), (1, msl(5))):
                    ts(sl(HIF, k), src, 1.0 / 512.0, Alu.mult)
                nc.vector.tensor_copy(out=HI_I[:], in_=HIF[:])
                nc.vector.tensor_copy(out=HIF[:], in_=HI_I[:])
                # c2 = floor(ca/512); c1 = ca - 512*c2
                for k, src in ((0, msl(4)), (1, msl(5))):
                    ts(SC1[:], sl(HIF, k), -512.0, Alu.mult)
                    tt(Alu.add, sl(CNT, 2 * k), src, SC1[:])
                    nc.vector.tensor_copy(out=sl(CNT, 2 * k + 1), in_=sl(HIF, k))

                BEST = tmp("BEST"); GT = tmp("GT"); KT = tmp("KT")
                tt(Alu.subtract, BEST[:], NV[:], sl(CNT, 0))
                for k in (1, 2, 3):
                    tt(Alu.subtract, BEST[:], BEST[:], sl(CNT, k))
                nc.vector.memset(MODE[:], 0.0)
                for k in range(1, 5):
                    ck = sl(CNT, k - 1)
                    tt(Alu.is_gt, GT[:], ck, BEST[:])
                    nc.vector.tensor_scalar(out=KT[:], in0=MODE[:],
                                            scalar1=-1.0, scalar2=float(k),
                                            op0=Alu.mult, op1=Alu.add)
                    tt(Alu.mult, KT[:], KT[:], GT[:])
                    tt(Alu.add, MODE[:], MODE[:], KT[:])
                    tt(Alu.max, BEST[:], BEST[:], ck)

                # eigenvalues: trig closed form on A'
                Q = tmp("Q"); P1 = tmp("P1"); P2 = tmp("P2"); PP = tmp("PP")
                RP = tmp("RP"); DET = tmp("DET"); RR = tmp("RR"); SS = tmp("SS")
                AT = tmp("AT"); PHI = tmp("PHI")
                W0 = tmp("W0"); W1 = tmp("W1"); W2 = tmp("W2"); RW2 = tmp("RW2")
                DIRWT = tmp("DIRWT")
                NB = tmp("NB", 6)

                tt(Alu.add, Q[:], sl(A, 0), sl(A, 3))
                tt(Alu.add, Q[:], Q[:], sl(A, 5))
                ts(Q[:], Q[:], 1.0 / 3.0, Alu.mult)

                tt(Alu.mult, P1[:], sl(A, 1), sl(A, 1))
                tt(Alu.mult, SC1[:], sl(A, 2), sl(A, 2))
                tt(Alu.add, P1[:], P1[:], SC1[:])
                tt(Alu.mult, SC1[:], sl(A, 4), sl(A, 4))
                tt(Alu.add, P1[:], P1[:], SC1[:])

                BD = tmp("BD", 3)
                tt(Alu.subtract, sl(BD, 0), sl(A, 0), Q[:])
                tt(Alu.subtract, sl(BD, 1), sl(A, 3), Q[:])
                tt(Alu.subtract, sl(BD, 2), sl(A, 5), Q[:])
                tt(Alu.mult, P2[:], sl(BD, 0), sl(BD, 0))
                tt(Alu.mult, SC1[:], sl(BD, 1), sl(BD, 1))
                tt(Alu.add, P2[:], P2[:], SC1[:])
                tt(Alu.mult, SC1[:], sl(BD, 2), sl(BD, 2))
                tt(Alu.add, P2[:], P2[:], SC1[:])
                stt(P2[:], P1[:], 2.0, Alu.mult, Alu.add, P2[:])
                ts(PP[:], P2[:], 1.0 / 6.0, Alu.mult)
                act(PP[:], PP[:], Act.Sqrt)
                ts(SC1[:], PP[:], TINY, Alu.max)
                nc.vector.reciprocal(RP[:], SC1[:])

                tt(Alu.mult, sl(NB, 0), sl(BD, 0), RP[:])
                tt(Alu.mult, sl(NB, 1), sl(A, 1), RP[:])
                tt(Alu.mult, sl(NB, 2), sl(A, 2), RP[:])
                tt(Alu.mult, sl(NB, 3), sl(BD, 1), RP[:])
                tt(Alu.mult, sl(NB, 4), sl(A, 4), RP[:])
                tt(Alu.mult, sl(NB, 5), sl(BD, 2), RP[:])

                tt(Alu.mult, SC1[:], sl(NB, 3), sl(NB, 5))
                tt(Alu.mult, SC2[:], sl(NB, 4), sl(NB, 4))
                tt(Alu.subtract, SC1[:], SC1[:], SC2[:])
                tt(Alu.mult, DET[:], sl(NB, 0), SC1[:])
                tt(Alu.mult, SC1[:], sl(NB, 1), sl(NB, 5))
                tt(Alu.mult, SC2[:], sl(NB, 4), sl(NB, 2))
                tt(Alu.subtract, SC1[:], SC1[:], SC2[:])
                tt(Alu.mult, SC1[:], sl(NB, 1), SC1[:])
                tt(Alu.subtract, DET[:], DET[:], SC1[:])
                tt(Alu.mult, SC1[:], sl(NB, 1), sl(NB, 4))
                tt(Alu.mult, SC2[:], sl(NB, 3), sl(NB, 2))
                tt(Alu.subtract, SC1[:], SC1[:], SC2[:])
                tt(Alu.mult, SC1[:], sl(NB, 2), SC1[:])
                tt(Alu.add, DET[:], DET[:], SC1[:])

                ts(RR[:], DET[:], 0.5, Alu.mult)
                ts(RR[:], RR[:], -1.0, Alu.max)
                ts(RR[:], RR[:], 1.0, Alu.min)
                tt(Alu.mult, SS[:], RR[:], RR[:])
                nc.vector.tensor_scalar(out=SS[:], in0=SS[:], scalar1=-1.0,
                                        scalar2=1.0, op0=Alu.mult, op1=Alu.add)
                ts(SS[:], SS[:], 0.0, Alu.max)
                act(SS[:], SS[:], Act.Sqrt)
                UA = tmp("UA"); UB = tmp("UB")
                ts(SC1[:], RR[:], -1.0, Alu.mult)
                tt(Alu.max, SC1[:], SC1[:], RR[:])
                ts(SS[:], SS[:], TINY, Alu.max)
                nc.vector.reciprocal(SC2[:], SS[:])
                tt(Alu.mult, UA[:], SC1[:], SC2[:])
                ts(SC1[:], UA[:], TINY, Alu.max)
                nc.vector.reciprocal(UB[:], SC1[:])
                tt(Alu.min, SC2[:], UA[:], UB[:])
                act(SC2[:], SC2[:], Act.Arctan)
                ts(SC1[:], UA[:], 1.0, Alu.is_gt)
                nc.vector.tensor_scalar(out=SC3[:], in0=SC2[:], scalar1=-2.0,
                                        scalar2=_PI / 2.0, op0=Alu.mult,
                                        op1=Alu.add)
                tt(Alu.mult, SC3[:], SC3[:], SC1[:])
                tt(Alu.add, SC2[:], SC2[:], SC3[:])
                ts(SC3[:], RR[:], 0.0, Alu.is_lt)
                nc.vector.tensor_scalar(out=SC3[:], in0=SC3[:], scalar1=-2.0,
                                        scalar2=1.0, op0=Alu.mult, op1=Alu.add)
                tt(Alu.mult, AT[:], SC2[:], SC3[:])
                nc.vector.tensor_scalar(out=PHI[:], in0=AT[:],
                                        scalar1=-1.0 / 3.0,
                                        scalar2=_PI / 6.0 + _PI / 2.0,
                                        op0=Alu.mult, op1=Alu.add)
                act(SC1[:], PHI[:], Act.Sin)
                tt(Alu.mult, SC1[:], SC1[:], PP[:])
                stt(W2[:], SC1[:], 2.0, Alu.mult, Alu.add, Q[:])
                nc.vector.tensor_scalar(out=PHI[:], in0=AT[:],
                                        scalar1=-1.0 / 3.0,
                                        scalar2=_PI / 6.0 + _PI / 6.0,
                                        op0=Alu.mult, op1=Alu.add)
                act(SC1[:], PHI[:], Act.Sin)
                tt(Alu.mult, SC1[:], SC1[:], PP[:])
                stt(W0[:], SC1[:], -2.0, Alu.mult, Alu.add, Q[:])
                ts(SC1[:], Q[:], 3.0, Alu.mult)
                tt(Alu.subtract, W1[:], SC1[:], W0[:])
                tt(Alu.subtract, W1[:], W1[:], W2[:])

                ts(SC1[:], W2[:], TINY, Alu.max)
                nc.vector.reciprocal(RW2[:], SC1[:])
                tt(Alu.mult, DIRWT[:], W1[:], RW2[:])
                nc.vector.tensor_scalar(out=DIRWT[:], in0=DIRWT[:],
                                        scalar1=-1.0, scalar2=1.0,
                                        op0=Alu.mult, op1=Alu.add)
                for q in range(6):
                    tt(Alu.mult, sl(B6, q), sl(A, q), RW2[:])

                CD = tmp("CD", 3)
                DD = tmp("DD", 3)
                for qi, ai in enumerate((0, 3, 5)):
                    tt(Alu.subtract, sl(CD, qi), sl(A, ai), W0[:])
                    tt(Alu.subtract, sl(DD, qi), sl(A, ai), W1[:])
                M9 = tmp("M9", 9)

                def mcol(colq, dv):
                    crow = [(sl(CD, 0), sl(A, 1), sl(A, 2)),
                            (sl(A, 1), sl(CD, 1), sl(A, 4)),
                            (sl(A, 2), sl(A, 4), sl(CD, 2))]
                    for r in range(3):
                        a0, a1, a2 = crow[r]
                        tt(Alu.mult, SC1[:], a0, dv[0])
                        tt(Alu.mult, SC2[:], a1, dv[1])
                        tt(Alu.add, SC1[:], SC1[:], SC2[:])
                        tt(Alu.mult, SC2[:], a2, dv[2])
                        tt(Alu.add, sl(M9, colq * 3 + r), SC1[:], SC2[:])

                mcol(0, (sl(DD, 0), sl(A, 1), sl(A, 2)))
                mcol(1, (sl(A, 1), sl(DD, 1), sl(A, 4)))
                mcol(2, (sl(A, 2), sl(A, 4), sl(DD, 2)))

                CN = tmp("CN", 3)
                for j in range(3):
                    tt(Alu.mult, sl(CN, j), sl(M9, j * 3), sl(M9, j * 3))
                    tt(Alu.mult, SC1[:], sl(M9, j * 3 + 1), sl(M9, j * 3 + 1))
                    tt(Alu.add, sl(CN, j), sl(CN, j), SC1[:])
                    tt(Alu.mult, SC1[:], sl(M9, j * 3 + 2), sl(M9, j * 3 + 2))
                    tt(Alu.add, sl(CN, j), sl(CN, j), SC1[:])
                NBEST = tmp("NBEST")
                for i in range(3):
                    nc.vector.tensor_copy(out=sl(V3, i), in_=sl(M9, i))
                nc.vector.tensor_copy(out=NBEST[:], in_=sl(CN, 0))
                for j in (1, 2):
                    tt(Alu.is_gt, GT[:], sl(CN, j), NBEST[:])
                    for i in range(3):
                        tt(Alu.subtract, SC1[:], sl(M9, j * 3 + i), sl(V3, i))
                        tt(Alu.mult, SC1[:], SC1[:], GT[:])
                        tt(Alu.add, sl(V3, i), sl(V3, i), SC1[:])
                    tt(Alu.max, NBEST[:], NBEST[:], sl(CN, j))
                ts(SC1[:], NBEST[:], 1e-37, Alu.max)
                act(SC2[:], SC1[:], Act.Sqrt)
                nc.vector.reciprocal(SC2[:], SC2[:])
                for i in range(3):
                    tt(Alu.mult, sl(V3, i), sl(V3, i), SC2[:])
                return DIRWT

            DIRWT = cluster_math()

            # ---------------- pass B (cluster-major, scaled units) ----
            from contextlib import ExitStack
            _pb_stack = ExitStack()
            pbp = _pb_stack.enter_context(tc.tile_pool(name="pbp", bufs=1))
            pb = _pb_stack.enter_context(tc.tile_pool(name="pb", bufs=2))
            CSTR = pbp.tile([P, 3 * Sg], dt.float16, tag="CSTR", name="CSTR")
            nc.sync.dma_start(out=CSTR[:], in_=cstr[:, :])
            TP = pbp.tile([P, Sg], dt.float16, tag="TP", name="TP")

            def cm_plane(i):   # cluster-major input plane i
                return CSTR[:, i * Sg:(i + 1) * Sg]

            # NK = -(c . v0)  (also reused as T0 in sign phase)
            NK = full_tile("NK")
            SCX = full_tile("SCX")
            tt(Alu.mult, NK[:], CEN[:, 0:NT], V3[:, 0:NT])
            tt(Alu.mult, SCX[:], CEN[:, NT:2 * NT], V3[:, NT:2 * NT])
            tt(Alu.add, NK[:], NK[:], SCX[:])
            tt(Alu.mult, SCX[:], CEN[:, 2 * NT:3 * NT], V3[:, 2 * NT:3 * NT])
            tt(Alu.add, NK[:], NK[:], SCX[:])
            ts(NK[:], NK[:], -1.0, Alu.mult)

            for g in range(NG):
                lg = int(Lg[g]); s0 = int(goff[g]); w = 4 * lg
                SQ3 = pb.tile([P, 3 * w], dt.float16, tag="SQ3", name=f"SQ3{g}")
                QA = pb.tile([P, w], dt.float16, tag="QA", name=f"QA{g}")
                QB = pb.tile([P, w], dt.float16, tag="QB", name=f"QB{g}")
                T2 = pb.tile([P, w], dt.float16, tag="T2", name=f"T2{g}")
                R2 = pb.tile([P, w], dt.float16, tag="R2", name=f"R2{g}")
                RPL = pb.tile([P, w], dt.float16, tag="RPL", name=f"RPL{g}")
                for tg in range(4):
                    t = 4 * g + tg
                    sl0 = s0 + tg * lg
                    # centered squares straight from raw x via ACT bias
                    for i in range(3):
                        act(SQ3[:, i * w + tg * lg:i * w + (tg + 1) * lg],
                            cm_plane(i)[:, sl0:sl0 + lg], Act.Square,
                            bias=NCEN[:, i * NT + t:i * NT + t + 1])
                    # T = x*v0x + NK, += y*v0y, += z*v0z
                    nc.vector.tensor_scalar(
                        out=TP[:, sl0:sl0 + lg],
                        in0=cm_plane(0)[:, sl0:sl0 + lg],
                        scalar1=V3[:, 0 * NT + t:0 * NT + t + 1],
                        scalar2=NK[:, t:t + 1],
                        op0=Alu.mult, op1=Alu.add)
                    stt(TP[:, sl0:sl0 + lg], cm_plane(1)[:, sl0:sl0 + lg],
                        V3[:, 1 * NT + t:1 * NT + t + 1], Alu.mult, Alu.add,
                        TP[:, sl0:sl0 + lg])
                    stt(TP[:, sl0:sl0 + lg], cm_plane(2)[:, sl0:sl0 + lg],
                        V3[:, 2 * NT + t:2 * NT + t + 1], Alu.mult, Alu.add,
                        TP[:, sl0:sl0 + lg])
                # q and r per group; first add on gpsimd
                nc.gpsimd.tensor_tensor(out=QA[:], in0=SQ3[:, 0:w],
                                        in1=SQ3[:, w:2 * w], op=Alu.add)
                tt(Alu.add, QB[:], QA[:], SQ3[:, 2 * w:3 * w])
                tt(Alu.mult, T2[:], TP[:, s0:s0 + w], TP[:, s0:s0 + w])
                tt(Alu.subtract, R2[:], QB[:], T2[:])
                ts(R2[:], R2[:], 0.0, Alu.max)
                act(RPL[:], R2[:], Act.Sqrt)
                for tg in range(4):
                    t = 4 * g + tg
                    stt(T2[:, tg * lg:(tg + 1) * lg],
                        TP[:, s0 + tg * lg:s0 + (tg + 1) * lg], 1.0,
                        Alu.mult, Alu.mult,
                        RPL[:, tg * lg:(tg + 1) * lg],
                        accum=SCRAW[:, t:t + 1])

            # ---------------- sign + output --------------------------
            def sign_phase():
                def tmp(tag, k=1):
                    return ret.tile([P, k * NT], dt.float32, tag=tag, name=tag)

                def sl(T, i):
                    return T[:, i * NT:(i + 1) * NT]

                CC = tmp("CC"); R0 = tmp("R0")
                SCV = tmp("SCV"); FAC = tmp("FAC"); SC9 = tmp("SC9")
                GT9 = tmp("GT9"); NPAD = tmp("NPAD")
                T0 = NK
                tt(Alu.mult, CC[:], sl(CEN, 0), sl(CEN, 0))
                tt(Alu.mult, SC9[:], sl(CEN, 1), sl(CEN, 1))
                tt(Alu.add, CC[:], CC[:], SC9[:])
                tt(Alu.mult, SC9[:], sl(CEN, 2), sl(CEN, 2))
                tt(Alu.add, CC[:], CC[:], SC9[:])
                tt(Alu.mult, SC9[:], T0[:], T0[:])
                tt(Alu.subtract, R0[:], CC[:], SC9[:])
                ts(R0[:], R0[:], 0.0, Alu.max)
                act(R0[:], R0[:], Act.Sqrt)
                # padded slots use the group padded length Lg
                for t in range(NT):
                    lg = int(Lg[t // 4])
                    nc.vector.tensor_scalar(
                        out=NPAD[:, t:t + 1],
                        in0=NV[:, t:t + 1], scalar1=-1.0,
                        scalar2=float(lg), op0=Alu.mult, op1=Alu.add)
                tt(Alu.mult, SC9[:], T0[:], R0[:])
                tt(Alu.mult, SC9[:], SC9[:], NPAD[:])
                tt(Alu.subtract, SCV[:], SCRAW[:], SC9[:])
                ts(GT9[:], SCV[:], 0.0, Alu.is_lt)
                nc.vector.tensor_scalar(out=GT9[:], in0=GT9[:], scalar1=-2.0,
                                        scalar2=1.0, op0=Alu.mult, op1=Alu.add)
                tt(Alu.mult, FAC[:], DIRWT[:], GT9[:])
                for i in range(3):
                    tt(Alu.mult, sl(V3, i), sl(V3, i), FAC[:])
                # unscale centers: x16
                for i in range(3):
                    ts(sl(CEN, i), sl(CEN, i), SCL, Alu.mult)
                OUTST = ret.tile([P, 19 * NT], dt.float32, tag="OUTST",
                                 name="OUTST")
                for j, pl in [(0, sl(CEN, 0)), (1, sl(CEN, 1)), (2, sl(CEN, 2)),
                              (3, sl(B6, 0)), (4, sl(B6, 1)), (5, sl(B6, 2)),
                              (6, sl(B6, 1)), (7, sl(B6, 3)), (8, sl(B6, 4)),
                              (9, sl(B6, 2)), (10, sl(B6, 4)), (11, sl(B6, 5)),
                              (12, sl(V3, 0)), (13, sl(V3, 1)), (14, sl(V3, 2)),
                              (15, NV[:]), (16, MEANV[:]), (17, STDV[:]),
                              (18, MODE[:])]:
                    nc.vector.tensor_copy(
                        out=OUTST[:, j * NT:(j + 1) * NT], in_=pl)
                nc.sync.dma_start(out=res[:, :], in_=OUTST[:])

            sign_phase()
            _pb_stack.close()

    nc.compile()
    return nc


_cache = {}
_last = None


def kernel(data, clust_idx, clust_len):
    global N, C, L, NT, NG
    data = np.asarray(data)
    clust_idx = np.asarray(clust_idx)
    N = int(data.shape[0])
    C, L = int(clust_idx.shape[0]), int(clust_idx.shape[1])
    assert C % (P * N_CORES) == 0
    NT = C // (P * N_CORES)
    NG = NT // 4
    meta = _host_prep(data, clust_idx, clust_len)

    key = (tuple(int(x) for x in meta["Lb"]), N, C)
    if key not in _cache:
        _cache[key] = _build_program(meta)
    nc = _cache[key]

    from concourse.bass_utils import run_bass_kernel_spmd
    in_maps = [{"tstr": meta["tstr"][c], "cstr": meta["cstr"][c],
                "nvec": meta["nvecs"][c]} for c in range(N_CORES)]
    global _last
    _last = (nc, in_maps)
    res = run_bass_kernel_spmd(nc, in_maps, list(range(N_CORES)))

    ids = meta["ids"]
    out = np.zeros((C, 19), dtype=f32)
    for core in range(N_CORES):
        r = res.results[core]["res"].reshape(P, 19, NT)
        for t in range(NT):
            out[ids[core, t]] = r[:, :, t]
    return out


# revision 14
# speedup vs baseline: 1.2800x; 1.0504x over previous
"""Trainium2 Bass kernel for nn_ClustGeoNodeEncoder (segment_reduce).

v2 architecture (PE-accelerated moments):
  - Host sorts clusters by length, deals them round-robin to 8 cores x 32
    tiles of 128 clusters (one cluster per partition per tile), and stages
    TWO fp16 streams per core:
      * transposed stream: [128 element-slots, ncols] feature planes
        (x/16, y/16, z/16, v, ca, cb) where each column holds up to 128
        elements of one cluster chunk (2 chunks for tiles padded > 128).
        ca = oh1 + 512*oh2, cb = oh3 + 512*oh4 pack the semantic one-hots
        (exact in fp16; sums stay < 2^24 so fp32 PSUM accumulation is
        exact).  Columns are ordered partition-major so partition p's
        clusters occupy a contiguous 13*NT-column window.
      * cluster-major stream: [128 clusters, S] x/16, y/16, z/16 planes,
        feature-major per group of 4 equal-padded tiles (for pass B).
  - Device pass A: ACT squares the coordinate/value planes, DVE forms the
    three cross-product planes (2x fp16 mode), and the TensorEngine
    reduces all 13 moment planes per cluster with ones-column matmuls:
    a staircase window (ones only in absolute column 128 of a [128, 256]
    buffer) places partition p's sums into PSUM row p; 128 accumulating
    matmuls cover all partitions, long-tile second chunks accumulate into
    the same PSUM columns.  One [128, 13*NT] PSUM->SBUF copy evacuates
    every raw moment.
  - Cluster math on [128, NT] fp32 planes: centers, centered scatter
    matrix A (scale-free in /16 units), closed-form trig eigenvalues,
    principal eigenvector via spectral projector, B = A/w2, dirwt, value
    stats, semantic mode via int32-truncation unpack of ca/cb.
  - Pass B (cluster-major): ts-centering (4x fp16), per-tile stt dot with
    v0, ACT group squares + sqrt, stt-accum orientation statistic sc;
    padded-slot closed-form correction, sign flip, output DMA.
"""

import sys

for _p in ("/opt/trn_rl_repo",):
    if _p not in sys.path:
        sys.path.insert(0, _p)

import numpy as np

N = 2_000_000
C = 32768
L = 256
N_CORES = 8
P = 128
NT = C // (P * N_CORES)  # 32 tiles per core
NG = 8                   # pass-B tile groups (4 tiles each, shared pad)
f32 = np.float32
f16 = np.float16

_PI = float(np.pi)
SCL = 16.0               # coordinate pre-scale (powers of 2 are exact)


def _host_prep(data, clust_idx, clust_len):
    data = np.asarray(data, dtype=f32)
    clust_idx = np.asarray(clust_idx).astype(np.int64)
    lens = np.asarray(clust_len).astype(np.int64)

    # feature table: x/16, y/16, z/16, v, ca, cb ; row N = zeros for padding
    table = np.zeros((N + 1, 6), dtype=f32)
    table[:N, 0:3] = data[:, 0:3] / SCL
    table[:N, 3] = data[:, 4]
    sem = data[:N, 5].astype(np.int32)
    ca = (sem == 1).astype(f32) + 512.0 * (sem == 2)
    cb = (sem == 3).astype(f32) + 512.0 * (sem == 4)
    table[:N, 4] = ca
    table[:N, 5] = cb

    order = np.argsort(lens, kind="stable")
    # rank r: tile t = r // (P*N_CORES); slot s = r % (P*N_CORES)
    # core = s % N_CORES ; partition = s // N_CORES
    Lb = np.zeros(NT, dtype=np.int64)
    for t in range(NT):
        Lb[t] = lens[order[t * P * N_CORES:(t + 1) * P * N_CORES]].max()
    # pass-B groups of 4 tiles share a padded length
    Lg = np.zeros(NG, dtype=np.int64)
    for g in range(NG):
        Lg[g] = Lb[4 * g:4 * g + 4].max()
    Sg = int(Lg.sum() * 4)          # cluster-major columns per partition

    chunks = np.maximum(1, (Lb + 127) // 128)     # 1 or 2 per tile
    n2 = int((chunks == 2).sum())                 # trailing tiles (sorted)
    t2_start = NT - n2
    ncol_p = NT + n2                              # columns per partition
    NCOL = P * ncol_p

    ar = np.arange(L)[None, :]
    idx_pad = np.where(ar < lens[:, None], clust_idx, N)

    ids = np.zeros((N_CORES, NT, P), dtype=np.int64)
    nvecs = np.zeros((N_CORES, P, NT), dtype=f32)
    # transposed stream: [core][128 slots, 6 planes * NCOL] plane-major
    tstr = np.zeros((N_CORES, P, 6 * NCOL), dtype=f16)
    # cluster-major stream: [core][128, 3 * Sg] plane-major, group-padded
    cstr = np.zeros((N_CORES, P, 3 * Sg), dtype=f16)

    goff = np.zeros(NG, dtype=np.int64)
    off = 0
    for g in range(NG):
        goff[g] = off
        off += 4 * int(Lg[g])

    tv = tstr.reshape(N_CORES, P, 6, P, ncol_p)
    cv = cstr.reshape(N_CORES, P, 3, Sg)
    for t in range(NT):
        base = t * P * N_CORES
        g, tg = t // 4, t % 4
        lg = int(Lg[g])
        lb = int(Lb[t])
        c1 = min(lb, 128)
        for core in range(N_CORES):
            sel = order[base + core + N_CORES * np.arange(P)]
            ids[core, t] = sel
            nvecs[core, :, t] = lens[sel]
            feats = table[idx_pad[sel, :lb]]          # [P, lb, 6]
            # transposed: chunk 0 -> col t, chunk 1 -> col NT + (t - t2_start)
            blk = np.zeros((P, 128, 6), dtype=f32)
            blk[:, :c1] = feats[:, :c1]
            tv[core, :, :, :, t] = blk.transpose(1, 2, 0).astype(f16)
            if lb > 128:
                blk2 = np.zeros((P, 128, 6), dtype=f32)
                blk2[:, :lb - 128] = feats[:, 128:lb]
                tv[core, :, :, :, NT + (t - t2_start)] = (
                    blk2.transpose(1, 2, 0).astype(f16))
            # cluster-major x/16,y/16,z/16 planes, group layout
            s0 = int(goff[g]) + tg * lg
            cv[core, :, :, s0:s0 + lb] = (
                feats[:, :, 0:3].transpose(0, 2, 1).astype(f16))
    return dict(tstr=tstr, cstr=cstr, nvecs=nvecs, ids=ids, Lb=Lb, Lg=Lg,
                Sg=Sg, ncol_p=ncol_p, NCOL=NCOL, n2=n2, t2_start=t2_start,
                goff=goff)


def _build_program(meta):
    import concourse.bass as bass
    import concourse.bacc as bacc
    import concourse.mybir as mybir
    from concourse.tile import TileContext

    dt = mybir.dt
    Alu = mybir.AluOpType
    Act = mybir.ActivationFunctionType

    Lb = meta["Lb"]; Lg = meta["Lg"]; Sg = meta["Sg"]
    ncol_p = meta["ncol_p"]; NCOL = meta["NCOL"]
    n2 = meta["n2"]; t2s = meta["t2_start"]; goff = meta["goff"]
    NM = 13                       # moment planes

    nc = bacc.Bacc("TRN2", target_bir_lowering=False, debug=False,
                   enable_asserts=False)
    tstr = nc.dram_tensor("tstr", [P, 6 * NCOL], dt.float16,
                          kind="ExternalInput")
    cstr = nc.dram_tensor("cstr", [P, 3 * Sg], dt.float16,
                          kind="ExternalInput")
    nvec_d = nc.dram_tensor("nvec", [P, NT], dt.float32, kind="ExternalInput")
    res = nc.dram_tensor("res", [P, 19 * NT], dt.float32,
                         kind="ExternalOutput")
    resdbg = nc.dram_tensor("resdbg", [P, 13 * NT], dt.float32,
                            kind="ExternalOutput")

    TINY = 1e-30
    NBLK = 4
    BLK = NCOL // NBLK            # column split for SBUF residency

    with TileContext(nc) as tc:
        with tc.tile_pool(name="ret", bufs=1) as ret, \
             tc.tile_pool(name="ps", bufs=1, space="PSUM") as ps:

            def full_tile(tag, k=1):
                return ret.tile([P, k * NT], dt.float32, tag=tag, name=tag)

            NV = full_tile("NV")
            RN = full_tile("RN")
            RAWM = ret.tile([P, NM * NT], dt.float32, tag="RAWM", name="RAWM")
            SCRAW = full_tile("SCRAW")
            MEANV = full_tile("MEANV"); STDV = full_tile("STDV")
            MODE = full_tile("MODE")
            B6 = full_tile("B6", 6)
            V3 = full_tile("V3", 3)
            CEN = full_tile("CEN", 3)
            NCEN = full_tile("NCEN", 3)
            STAIR = ret.tile([P, 256], dt.float16, tag="STAIR", name="STAIR")

            nc.sync.dma_start(out=NV[:], in_=nvec_d[:, :])
            nc.vector.reciprocal(RN[:], NV[:])
            nc.vector.memset(STAIR[:], 0.0)
            nc.vector.memset(STAIR[:, 128:129], 1.0)

            def tt(op, out, a, b):
                nc.vector.tensor_tensor(out=out, in0=a, in1=b, op=op)

            def ts(out, in0, s, op, s2=None, op1=None):
                kw = {}
                if op1 is not None:
                    kw["op1"] = op1
                nc.vector.tensor_scalar(out=out, in0=in0, scalar1=s,
                                        scalar2=s2, op0=op, **kw)

            def stt(out, in0, s, op0, op1, in1, accum=None):
                nc.vector.scalar_tensor_tensor(out=out, in0=in0, scalar=s,
                                               in1=in1, op0=op0, op1=op1,
                                               accum_out=accum)

            def act(out, in_, func, bias=0.0, scale=1.0, accum=None):
                nc.scalar.activation(out, in_, func, bias=bias, scale=scale,
                                     accum_out=accum)

            # ---------------- Pass A: PE moment sums -----------------
            PS = ps.tile([P, NM * NT], dt.float32, tag="PS", name="PS")
            first = [True]

            def pass_a_blk(h, hp):
                c0, c1 = h * BLK, (h + 1) * BLK
                W = c1 - c0
                # combined tile: 6 raw planes + 7 product planes
                al = hp.tile([P, 13 * W], dt.float16, tag="al",
                             name=f"al{h}")
                alv = al[:].rearrange("k (f c) -> k f c", f=13)
                nc.sync.dma_start(
                    out=alv[:, 0:6, :],
                    in_=tstr[:, :].rearrange("k (f c) -> k f c", f=6)[:, :, c0:c1])
                # squares on ACT, crosses on DVE; chunked so the
                # per-partition matmuls can start early
                NCK = 4
                CW = W // NCK
                for ck in range(NCK):
                    a0, a1 = ck * CW, (ck + 1) * CW
                    act(alv[:, 6:10, a0:a1], alv[:, 0:4, a0:a1], Act.Square)
                    tt(Alu.mult, alv[:, 10, a0:a1], alv[:, 0, a0:a1],
                       alv[:, 1, a0:a1])
                    tt(Alu.mult, alv[:, 11, a0:a1], alv[:, 0, a0:a1],
                       alv[:, 2, a0:a1])
                    tt(Alu.mult, alv[:, 12, a0:a1], alv[:, 1, a0:a1],
                       alv[:, 2, a0:a1])

                # per-partition matmuls (staircase window -> PSUM row p)
                p0, p1 = c0 // ncol_p, c1 // ncol_p
                psv = PS[:].rearrange("p (m t) -> p m t", m=NM)
                for p in range(p0, p1):
                    rb = p * ncol_p - c0
                    r13 = alv[:, :, rb:rb + ncol_p]
                    lhs = STAIR[:, 128 - p:256 - p]
                    st = first[0]
                    first[0] = False
                    last = (p == P - 1)
                    # start resets the WHOLE psum bank: only the very
                    # first matmul may carry start=True
                    nc.tensor.matmul(psv[:, :, 0:NT], lhs, r13[:, :, 0:NT],
                                     start=st, stop=False,
                                     skip_group_check=True)
                    if n2 > 0:
                        nc.tensor.matmul(psv[:, :, t2s:NT], lhs,
                                         r13[:, :, NT:ncol_p],
                                         start=False, stop=last,
                                         skip_group_check=True)
                    elif last:
                        nc.tensor.matmul(psv[:, 0:1, NT - 1:NT],
                                         STAIR[:, 0:1].broadcast_to((P, 1)),
                                         STAIR[:, 0:1], start=False, stop=True)

            with tc.tile_pool(name="blk", bufs=3) as hp:
                for h in range(NBLK):
                    pass_a_blk(h, hp)
            nc.vector.tensor_copy(out=RAWM[:], in_=PS[:])
            nc.sync.dma_start(out=resdbg[:, :], in_=RAWM[:])

            def msl(m):
                return RAWM[:, m * NT:(m + 1) * NT]

            # ---------------- cluster math ----------------------------
            def cluster_math():
                def tmp(tag, k=1):
                    return ret.tile([P, k * NT], dt.float32, tag=tag, name=tag)

                def sl(T, i):
                    return T[:, i * NT:(i + 1) * NT]

                SC1 = tmp("SC1"); SC2 = tmp("SC2"); SC3 = tmp("SC3")
                # centers (scaled units): c' = sum(x')/n
                for i in range(3):
                    tt(Alu.mult, sl(CEN, i), msl(i), RN[:])
                    ts(sl(NCEN, i), sl(CEN, i), -1.0, Alu.mult)
                # A' = prod - cen*sum  (xx,xy,xz,yy,yz,zz in cmap order)
                A = tmp("A", 6)
                # raw plane order: 6=xx,7=yy,8=zz,9=vv,10=xy,11=xz,12=yz
                pmap = [(0, 6, 0, 0), (1, 10, 0, 1), (2, 11, 0, 2),
                        (3, 7, 1, 1), (4, 12, 1, 2), (5, 8, 2, 2)]
                for q, pm, i, j in pmap:
                    tt(Alu.mult, SC1[:], sl(CEN, i), msl(j))
                    tt(Alu.subtract, sl(A, q), msl(pm), SC1[:])

                # value stats: meanv = sum(v)/n ; var = (sum(v^2)-mean*sum)/ (n-1)
                VAR = tmp("VAR"); NM1 = tmp("NM1")
                tt(Alu.mult, MEANV[:], msl(3), RN[:])
                tt(Alu.mult, VAR[:], MEANV[:], msl(3))
                tt(Alu.subtract, VAR[:], msl(9), VAR[:])
                ts(NM1[:], NV[:], 1.0, Alu.subtract)
                nc.vector.reciprocal(SC1[:], NM1[:])
                tt(Alu.mult, VAR[:], VAR[:], SC1[:])
                ts(VAR[:], VAR[:], 0.0, Alu.max)
                act(STDV[:], VAR[:], Act.Sqrt)

                # unpack semantic counts: ca -> c1 + 512*c2, cb -> c3 + 512*c4
                CNT = tmp("CNT", 4)
                HI_I = ret.tile([P, 2 * NT], dt.int32, tag="HI_I", name="HI_I")
                HIF = tmp("HIF", 2)
                for k, src in ((0, msl(4)), (1, msl(5))):
                    ts(sl(HIF, k), src, 1.0 / 512.0, Alu.mult)
                nc.vector.tensor_copy(out=HI_I[:], in_=HIF[:])
                nc.vector.tensor_copy(out=HIF[:], in_=HI_I[:])
                # c2 = floor(ca/512); c1 = ca - 512*c2
                for k, src in ((0, msl(4)), (1, msl(5))):
                    ts(SC1[:], sl(HIF, k), -512.0, Alu.mult)
                    tt(Alu.add, sl(CNT, 2 * k), src, SC1[:])
                    nc.vector.tensor_copy(out=sl(CNT, 2 * k + 1), in_=sl(HIF, k))

                BEST = tmp("BEST"); GT = tmp("GT"); KT = tmp("KT")
                tt(Alu.subtract, BEST[:], NV[:], sl(CNT, 0))
                for k in (1, 2, 3):
                    tt(Alu.subtract, BEST[:], BEST[:], sl(CNT, k))
                nc.vector.memset(MODE[:], 0.0)
                for k in range(1, 5):
                    ck = sl(CNT, k - 1)
                    tt(Alu.is_gt, GT[:], ck, BEST[:])
                    nc.vector.tensor_scalar(out=KT[:], in0=MODE[:],
                                            scalar1=-1.0, scalar2=float(k),
                                            op0=Alu.mult, op1=Alu.add)
                    tt(Alu.mult, KT[:], KT[:], GT[:])
                    tt(Alu.add, MODE[:], MODE[:], KT[:])
                    tt(Alu.max, BEST[:], BEST[:], ck)

                # eigenvalues: trig closed form on A'
                Q = tmp("Q"); P1 = tmp("P1"); P2 = tmp("P2"); PP = tmp("PP")
                RP = tmp("RP"); DET = tmp("DET"); RR = tmp("RR"); SS = tmp("SS")
                AT = tmp("AT"); PHI = tmp("PHI")
                W0 = tmp("W0"); W1 = tmp("W1"); W2 = tmp("W2"); RW2 = tmp("RW2")
                DIRWT = tmp("DIRWT")
                NB = tmp("NB", 6)

                tt(Alu.add, Q[:], sl(A, 0), sl(A, 3))
                tt(Alu.add, Q[:], Q[:], sl(A, 5))
                ts(Q[:], Q[:], 1.0 / 3.0, Alu.mult)

                tt(Alu.mult, P1[:], sl(A, 1), sl(A, 1))
                tt(Alu.mult, SC1[:], sl(A, 2), sl(A, 2))
                tt(Alu.add, P1[:], P1[:], SC1[:])
                tt(Alu.mult, SC1[:], sl(A, 4), sl(A, 4))
                tt(Alu.add, P1[:], P1[:], SC1[:])

                BD = tmp("BD", 3)
                tt(Alu.subtract, sl(BD, 0), sl(A, 0), Q[:])
                tt(Alu.subtract, sl(BD, 1), sl(A, 3), Q[:])
                tt(Alu.subtract, sl(BD, 2), sl(A, 5), Q[:])
                tt(Alu.mult, P2[:], sl(BD, 0), sl(BD, 0))
                tt(Alu.mult, SC1[:], sl(BD, 1), sl(BD, 1))
                tt(Alu.add, P2[:], P2[:], SC1[:])
                tt(Alu.mult, SC1[:], sl(BD, 2), sl(BD, 2))
                tt(Alu.add, P2[:], P2[:], SC1[:])
                stt(P2[:], P1[:], 2.0, Alu.mult, Alu.add, P2[:])
                ts(PP[:], P2[:], 1.0 / 6.0, Alu.mult)
                act(PP[:], PP[:], Act.Sqrt)
                ts(SC1[:], PP[:], TINY, Alu.max)
                nc.vector.reciprocal(RP[:], SC1[:])

                tt(Alu.mult, sl(NB, 0), sl(BD, 0), RP[:])
                tt(Alu.mult, sl(NB, 1), sl(A, 1), RP[:])
                tt(Alu.mult, sl(NB, 2), sl(A, 2), RP[:])
                tt(Alu.mult, sl(NB, 3), sl(BD, 1), RP[:])
                tt(Alu.mult, sl(NB, 4), sl(A, 4), RP[:])
                tt(Alu.mult, sl(NB, 5), sl(BD, 2), RP[:])

                tt(Alu.mult, SC1[:], sl(NB, 3), sl(NB, 5))
                tt(Alu.mult, SC2[:], sl(NB, 4), sl(NB, 4))
                tt(Alu.subtract, SC1[:], SC1[:], SC2[:])
                tt(Alu.mult, DET[:], sl(NB, 0), SC1[:])
                tt(Alu.mult, SC1[:], sl(NB, 1), sl(NB, 5))
                tt(Alu.mult, SC2[:], sl(NB, 4), sl(NB, 2))
                tt(Alu.subtract, SC1[:], SC1[:], SC2[:])
                tt(Alu.mult, SC1[:], sl(NB, 1), SC1[:])
                tt(Alu.subtract, DET[:], DET[:], SC1[:])
                tt(Alu.mult, SC1[:], sl(NB, 1), sl(NB, 4))
                tt(Alu.mult, SC2[:], sl(NB, 3), sl(NB, 2))
                tt(Alu.subtract, SC1[:], SC1[:], SC2[:])
                tt(Alu.mult, SC1[:], sl(NB, 2), SC1[:])
                tt(Alu.add, DET[:], DET[:], SC1[:])

                ts(RR[:], DET[:], 0.5, Alu.mult)
                ts(RR[:], RR[:], -1.0, Alu.max)
                ts(RR[:], RR[:], 1.0, Alu.min)
                tt(Alu.mult, SS[:], RR[:], RR[:])
                nc.vector.tensor_scalar(out=SS[:], in0=SS[:], scalar1=-1.0,
                                        scalar2=1.0, op0=Alu.mult, op1=Alu.add)
                ts(SS[:], SS[:], 0.0, Alu.max)
                act(SS[:], SS[:], Act.Sqrt)
                UA = tmp("UA"); UB = tmp("UB")
                ts(SC1[:], RR[:], -1.0, Alu.mult)
                tt(Alu.max, SC1[:], SC1[:], RR[:])
                ts(SS[:], SS[:], TINY, Alu.max)
                nc.vector.reciprocal(SC2[:], SS[:])
                tt(Alu.mult, UA[:], SC1[:], SC2[:])
                ts(SC1[:], UA[:], TINY, Alu.max)
                nc.vector.reciprocal(UB[:], SC1[:])
                tt(Alu.min, SC2[:], UA[:], UB[:])
                act(SC2[:], SC2[:], Act.Arctan)
                ts(SC1[:], UA[:], 1.0, Alu.is_gt)
                nc.vector.tensor_scalar(out=SC3[:], in0=SC2[:], scalar1=-2.0,
                                        scalar2=_PI / 2.0, op0=Alu.mult,
                                        op1=Alu.add)
                tt(Alu.mult, SC3[:], SC3[:], SC1[:])
                tt(Alu.add, SC2[:], SC2[:], SC3[:])
                ts(SC3[:], RR[:], 0.0, Alu.is_lt)
                nc.vector.tensor_scalar(out=SC3[:], in0=SC3[:], scalar1=-2.0,
                                        scalar2=1.0, op0=Alu.mult, op1=Alu.add)
                tt(Alu.mult, AT[:], SC2[:], SC3[:])
                nc.vector.tensor_scalar(out=PHI[:], in0=AT[:],
                                        scalar1=-1.0 / 3.0,
                                        scalar2=_PI / 6.0 + _PI / 2.0,
                                        op0=Alu.mult, op1=Alu.add)
                act(SC1[:], PHI[:], Act.Sin)
                tt(Alu.mult, SC1[:], SC1[:], PP[:])
                stt(W2[:], SC1[:], 2.0, Alu.mult, Alu.add, Q[:])
                nc.vector.tensor_scalar(out=PHI[:], in0=AT[:],
                                        scalar1=-1.0 / 3.0,
                                        scalar2=_PI / 6.0 + _PI / 6.0,
                                        op0=Alu.mult, op1=Alu.add)
                act(SC1[:], PHI[:], Act.Sin)
                tt(Alu.mult, SC1[:], SC1[:], PP[:])
                stt(W0[:], SC1[:], -2.0, Alu.mult, Alu.add, Q[:])
                ts(SC1[:], Q[:], 3.0, Alu.mult)
                tt(Alu.subtract, W1[:], SC1[:], W0[:])
                tt(Alu.subtract, W1[:], W1[:], W2[:])

                ts(SC1[:], W2[:], TINY, Alu.max)
                nc.vector.reciprocal(RW2[:], SC1[:])
                tt(Alu.mult, DIRWT[:], W1[:], RW2[:])
                nc.vector.tensor_scalar(out=DIRWT[:], in0=DIRWT[:],
                                        scalar1=-1.0, scalar2=1.0,
                                        op0=Alu.mult, op1=Alu.add)
                for q in range(6):
                    tt(Alu.mult, sl(B6, q), sl(A, q), RW2[:])

                CD = tmp("CD", 3)
                DD = tmp("DD", 3)
                for qi, ai in enumerate((0, 3, 5)):
                    tt(Alu.subtract, sl(CD, qi), sl(A, ai), W0[:])
                    tt(Alu.subtract, sl(DD, qi), sl(A, ai), W1[:])
                M9 = tmp("M9", 9)

                def mcol(colq, dv):
                    crow = [(sl(CD, 0), sl(A, 1), sl(A, 2)),
                            (sl(A, 1), sl(CD, 1), sl(A, 4)),
                            (sl(A, 2), sl(A, 4), sl(CD, 2))]
                    for r in range(3):
                        a0, a1, a2 = crow[r]
                        tt(Alu.mult, SC1[:], a0, dv[0])
                        tt(Alu.mult, SC2[:], a1, dv[1])
                        tt(Alu.add, SC1[:], SC1[:], SC2[:])
                        tt(Alu.mult, SC2[:], a2, dv[2])
                        tt(Alu.add, sl(M9, colq * 3 + r), SC1[:], SC2[:])

                mcol(0, (sl(DD, 0), sl(A, 1), sl(A, 2)))
                mcol(1, (sl(A, 1), sl(DD, 1), sl(A, 4)))
                mcol(2, (sl(A, 2), sl(A, 4), sl(DD, 2)))

                CN = tmp("CN", 3)
                for j in range(3):
                    tt(Alu.mult, sl(CN, j), sl(M9, j * 3), sl(M9, j * 3))
                    tt(Alu.mult, SC1[:], sl(M9, j * 3 + 1), sl(M9, j * 3 + 1))
                    tt(Alu.add, sl(CN, j), sl(CN, j), SC1[:])
                    tt(Alu.mult, SC1[:], sl(M9, j * 3 + 2), sl(M9, j * 3 + 2))
                    tt(Alu.add, sl(CN, j), sl(CN, j), SC1[:])
                NBEST = tmp("NBEST")
                for i in range(3):
                    nc.vector.tensor_copy(out=sl(V3, i), in_=sl(M9, i))
                nc.vector.tensor_copy(out=NBEST[:], in_=sl(CN, 0))
                for j in (1, 2):
                    tt(Alu.is_gt, GT[:], sl(CN, j), NBEST[:])
                    for i in range(3):
                        tt(Alu.subtract, SC1[:], sl(M9, j * 3 + i), sl(V3, i))
                        tt(Alu.mult, SC1[:], SC1[:], GT[:])
                        tt(Alu.add, sl(V3, i), sl(V3, i), SC1[:])
                    tt(Alu.max, NBEST[:], NBEST[:], sl(CN, j))
                ts(SC1[:], NBEST[:], 1e-37, Alu.max)
                act(SC2[:], SC1[:], Act.Sqrt)
                nc.vector.reciprocal(SC2[:], SC2[:])
                for i in range(3):
                    tt(Alu.mult, sl(V3, i), sl(V3, i), SC2[:])
                return DIRWT

            DIRWT = cluster_math()

            # ---------------- pass B (cluster-major, scaled units) ----
            from contextlib import ExitStack
            _pb_stack = ExitStack()
            pbp = _pb_stack.enter_context(tc.tile_pool(name="pbp", bufs=1))
            pb = _pb_stack.enter_context(tc.tile_pool(name="pb", bufs=2))
            CSTR = pbp.tile([P, 3 * Sg], dt.float16, tag="CSTR", name="CSTR")
            nc.sync.dma_start(out=CSTR[:], in_=cstr[:, :])
            TP = pbp.tile([P, Sg], dt.float16, tag="TP", name="TP")

            def cm_plane(i):   # cluster-major input plane i
                return CSTR[:, i * Sg:(i + 1) * Sg]

            # NK = -(c . v0)  (also reused as T0 in sign phase)
            NK = full_tile("NK")
            SCX = full_tile("SCX")
            tt(Alu.mult, NK[:], CEN[:, 0:NT], V3[:, 0:NT])
            tt(Alu.mult, SCX[:], CEN[:, NT:2 * NT], V3[:, NT:2 * NT])
            tt(Alu.add, NK[:], NK[:], SCX[:])
            tt(Alu.mult, SCX[:], CEN[:, 2 * NT:3 * NT], V3[:, 2 * NT:3 * NT])
            tt(Alu.add, NK[:], NK[:], SCX[:])
            ts(NK[:], NK[:], -1.0, Alu.mult)

            for g in range(NG):
                lg = int(Lg[g]); s0 = int(goff[g]); w = 4 * lg
                SQ3 = pb.tile([P, 3 * w], dt.float16, tag="SQ3", name=f"SQ3{g}")
                QA = pb.tile([P, w], dt.float16, tag="QA", name=f"QA{g}")
                QB = pb.tile([P, w], dt.float16, tag="QB", name=f"QB{g}")
                T2 = pb.tile([P, w], dt.float16, tag="T2", name=f"T2{g}")
                R2 = pb.tile([P, w], dt.float16, tag="R2", name=f"R2{g}")
                RPL = pb.tile([P, w], dt.float16, tag="RPL", name=f"RPL{g}")
                for tg in range(4):
                    t = 4 * g + tg
                    sl0 = s0 + tg * lg
                    # centered squares straight from raw x via ACT bias
                    for i in range(3):
                        act(SQ3[:, i * w + tg * lg:i * w + (tg + 1) * lg],
                            cm_plane(i)[:, sl0:sl0 + lg], Act.Square,
                            bias=NCEN[:, i * NT + t:i * NT + t + 1])
                    # T = x*v0x + NK, += y*v0y, += z*v0z
                    nc.vector.tensor_scalar(
                        out=TP[:, sl0:sl0 + lg],
                        in0=cm_plane(0)[:, sl0:sl0 + lg],
                        scalar1=V3[:, 0 * NT + t:0 * NT + t + 1],
                        scalar2=NK[:, t:t + 1],
                        op0=Alu.mult, op1=Alu.add)
                    stt(TP[:, sl0:sl0 + lg], cm_plane(1)[:, sl0:sl0 + lg],
                        V3[:, 1 * NT + t:1 * NT + t + 1], Alu.mult, Alu.add,
                        TP[:, sl0:sl0 + lg])
                    stt(TP[:, sl0:sl0 + lg], cm_plane(2)[:, sl0:sl0 + lg],
                        V3[:, 2 * NT + t:2 * NT + t + 1], Alu.mult, Alu.add,
                        TP[:, sl0:sl0 + lg])
                # q and r per group; first add on gpsimd
                nc.gpsimd.tensor_tensor(out=QA[:], in0=SQ3[:, 0:w],
                                        in1=SQ3[:, w:2 * w], op=Alu.add)
                tt(Alu.add, QB[:], QA[:], SQ3[:, 2 * w:3 * w])
                tt(Alu.mult, T2[:], TP[:, s0:s0 + w], TP[:, s0:s0 + w])
                tt(Alu.subtract, R2[:], QB[:], T2[:])
                ts(R2[:], R2[:], 0.0, Alu.max)
                act(RPL[:], R2[:], Act.Sqrt)
                for tg in range(4):
                    t = 4 * g + tg
                    stt(T2[:, tg * lg:(tg + 1) * lg],
                        TP[:, s0 + tg * lg:s0 + (tg + 1) * lg], 1.0,
                        Alu.mult, Alu.mult,
                        RPL[:, tg * lg:(tg + 1) * lg],
                        accum=SCRAW[:, t:t + 1])

            # ---------------- sign + output --------------------------
            def sign_phase():
                def tmp(tag, k=1):
                    return ret.tile([P, k * NT], dt.float32, tag=tag, name=tag)

                def sl(T, i):
                    return T[:, i * NT:(i + 1) * NT]

                CC = tmp("CC"); R0 = tmp("R0")
                SCV = tmp("SCV"); FAC = tmp("FAC"); SC9 = tmp("SC9")
                GT9 = tmp("GT9"); NPAD = tmp("NPAD")
                T0 = NK
                tt(Alu.mult, CC[:], sl(CEN, 0), sl(CEN, 0))
                tt(Alu.mult, SC9[:], sl(CEN, 1), sl(CEN, 1))
                tt(Alu.add, CC[:], CC[:], SC9[:])
                tt(Alu.mult, SC9[:], sl(CEN, 2), sl(CEN, 2))
                tt(Alu.add, CC[:], CC[:], SC9[:])
                tt(Alu.mult, SC9[:], T0[:], T0[:])
                tt(Alu.subtract, R0[:], CC[:], SC9[:])
                ts(R0[:], R0[:], 0.0, Alu.max)
                act(R0[:], R0[:], Act.Sqrt)
                # padded slots use the group padded length Lg
                for t in range(NT):
                    lg = int(Lg[t // 4])
                    nc.vector.tensor_scalar(
                        out=NPAD[:, t:t + 1],
                        in0=NV[:, t:t + 1], scalar1=-1.0,
                        scalar2=float(lg), op0=Alu.mult, op1=Alu.add)
                tt(Alu.mult, SC9[:], T0[:], R0[:])
                tt(Alu.mult, SC9[:], SC9[:], NPAD[:])
                tt(Alu.subtract, SCV[:], SCRAW[:], SC9[:])
                ts(GT9[:], SCV[:], 0.0, Alu.is_lt)
                nc.vector.tensor_scalar(out=GT9[:], in0=GT9[:], scalar1=-2.0,
                                        scalar2=1.0, op0=Alu.mult, op1=Alu.add)
                tt(Alu.mult, FAC[:], DIRWT[:], GT9[:])
                for i in range(3):
                    tt(Alu.mult, sl(V3, i), sl(V3, i), FAC[:])
                # unscale centers: x16
                for i in range(3):
                    ts(sl(CEN, i), sl(CEN, i), SCL, Alu.mult)
                OUTST = ret.tile([P, 19 * NT], dt.float32, tag="OUTST",
                                 name="OUTST")
                for j, pl in [(0, sl(CEN, 0)), (1, sl(CEN, 1)), (2, sl(CEN, 2)),
                              (3, sl(B6, 0)), (4, sl(B6, 1)), (5, sl(B6, 2)),
                              (6, sl(B6, 1)), (7, sl(B6, 3)), (8, sl(B6, 4)),
                              (9, sl(B6, 2)), (10, sl(B6, 4)), (11, sl(B6, 5)),
                              (12, sl(V3, 0)), (13, sl(V3, 1)), (14, sl(V3, 2)),
                              (15, NV[:]), (16, MEANV[:]), (17, STDV[:]),
                              (18, MODE[:])]:
                    nc.vector.tensor_copy(
                        out=OUTST[:, j * NT:(j + 1) * NT], in_=pl)
                nc.sync.dma_start(out=res[:, :], in_=OUTST[:])

            sign_phase()
            _pb_stack.close()

    nc.compile()
    return nc


_cache = {}
_last = None


def kernel(data, clust_idx, clust_len):
    global N, C, L, NT, NG
    data = np.asarray(data)
    clust_idx = np.asarray(clust_idx)
    N = int(data.shape[0])
    C, L = int(clust_idx.shape[0]), int(clust_idx.shape[1])
    assert C % (P * N_CORES) == 0
    NT = C // (P * N_CORES)
    NG = NT // 4
    meta = _host_prep(data, clust_idx, clust_len)

    key = (tuple(int(x) for x in meta["Lb"]), N, C)
    if key not in _cache:
        _cache[key] = _build_program(meta)
    nc = _cache[key]

    from concourse.bass_utils import run_bass_kernel_spmd
    in_maps = [{"tstr": meta["tstr"][c], "cstr": meta["cstr"][c],
                "nvec": meta["nvecs"][c]} for c in range(N_CORES)]
    global _last
    _last = (nc, in_maps)
    res = run_bass_kernel_spmd(nc, in_maps, list(range(N_CORES)))

    ids = meta["ids"]
    out = np.zeros((C, 19), dtype=f32)
    for core in range(N_CORES):
        r = res.results[core]["res"].reshape(P, 19, NT)
        for t in range(NT):
            out[ids[core, t]] = r[:, :, t]
    return out


# revision 15
# speedup vs baseline: 1.2970x; 1.0133x over previous
"""Trainium2 Bass kernel for nn_ClustGeoNodeEncoder (segment_reduce).

v2 architecture (PE-accelerated moments):
  - Host sorts clusters by length, deals them round-robin to 8 cores x 32
    tiles of 128 clusters (one cluster per partition per tile), and stages
    TWO fp16 streams per core:
      * transposed stream: [128 element-slots, ncols] feature planes
        (x/16, y/16, z/16, v, ca, cb) where each column holds up to 128
        elements of one cluster chunk (2 chunks for tiles padded > 128).
        ca = oh1 + 512*oh2, cb = oh3 + 512*oh4 pack the semantic one-hots
        (exact in fp16; sums stay < 2^24 so fp32 PSUM accumulation is
        exact).  Columns are ordered partition-major so partition p's
        clusters occupy a contiguous 13*NT-column window.
      * cluster-major stream: [128 clusters, S] x/16, y/16, z/16 planes,
        feature-major per group of 4 equal-padded tiles (for pass B).
  - Device pass A: ACT squares the coordinate/value planes, DVE forms the
    three cross-product planes (2x fp16 mode), and the TensorEngine
    reduces all 13 moment planes per cluster with ones-column matmuls:
    a staircase window (ones only in absolute column 128 of a [128, 256]
    buffer) places partition p's sums into PSUM row p; 128 accumulating
    matmuls cover all partitions, long-tile second chunks accumulate into
    the same PSUM columns.  One [128, 13*NT] PSUM->SBUF copy evacuates
    every raw moment.
  - Cluster math on [128, NT] fp32 planes: centers, centered scatter
    matrix A (scale-free in /16 units), closed-form trig eigenvalues,
    principal eigenvector via spectral projector, B = A/w2, dirwt, value
    stats, semantic mode via int32-truncation unpack of ca/cb.
  - Pass B (cluster-major): ts-centering (4x fp16), per-tile stt dot with
    v0, ACT group squares + sqrt, stt-accum orientation statistic sc;
    padded-slot closed-form correction, sign flip, output DMA.
"""

import sys

for _p in ("/opt/trn_rl_repo",):
    if _p not in sys.path:
        sys.path.insert(0, _p)

import numpy as np

N = 2_000_000
C = 32768
L = 256
N_CORES = 8
P = 128
NT = C // (P * N_CORES)  # 32 tiles per core
NG = 8                   # pass-B tile groups (4 tiles each, shared pad)
f32 = np.float32
f16 = np.float16

_PI = float(np.pi)
SCL = 16.0               # coordinate pre-scale (powers of 2 are exact)


def _host_prep(data, clust_idx, clust_len):
    data = np.asarray(data, dtype=f32)
    clust_idx = np.asarray(clust_idx).astype(np.int64)
    lens = np.asarray(clust_len).astype(np.int64)

    # feature table: x/16, y/16, z/16, v, ca, cb ; row N = zeros for padding
    table = np.zeros((N + 1, 6), dtype=f32)
    table[:N, 0:3] = data[:, 0:3] / SCL
    table[:N, 3] = data[:, 4]
    sem = data[:N, 5].astype(np.int32)
    ca = (sem == 1).astype(f32) + 512.0 * (sem == 2)
    cb = (sem == 3).astype(f32) + 512.0 * (sem == 4)
    table[:N, 4] = ca
    table[:N, 5] = cb

    order = np.argsort(lens, kind="stable")
    # rank r: tile t = r // (P*N_CORES); slot s = r % (P*N_CORES)
    # core = s % N_CORES ; partition = s // N_CORES
    Lb = np.zeros(NT, dtype=np.int64)
    for t in range(NT):
        Lb[t] = lens[order[t * P * N_CORES:(t + 1) * P * N_CORES]].max()
    # pass-B groups of 4 tiles share a padded length
    Lg = np.zeros(NG, dtype=np.int64)
    for g in range(NG):
        Lg[g] = Lb[4 * g:4 * g + 4].max()
    Sg = int(Lg.sum() * 4)          # cluster-major columns per partition

    chunks = np.maximum(1, (Lb + 127) // 128)     # 1 or 2 per tile
    n2 = int((chunks == 2).sum())                 # trailing tiles (sorted)
    t2_start = NT - n2
    ncol_p = NT + n2                              # columns per partition
    NCOL = P * ncol_p

    ar = np.arange(L)[None, :]
    idx_pad = np.where(ar < lens[:, None], clust_idx, N)

    ids = np.zeros((N_CORES, NT, P), dtype=np.int64)
    nvecs = np.zeros((N_CORES, P, NT), dtype=f32)
    # transposed stream: [core][128 slots, 6 planes * NCOL] plane-major
    tstr = np.zeros((N_CORES, P, 6 * NCOL), dtype=f16)
    # cluster-major stream: [core][128, 3 * Sg] plane-major, group-padded
    cstr = np.zeros((N_CORES, P, 3 * Sg), dtype=f16)

    goff = np.zeros(NG, dtype=np.int64)
    off = 0
    for g in range(NG):
        goff[g] = off
        off += 4 * int(Lg[g])

    tv = tstr.reshape(N_CORES, P, 6, P, ncol_p)
    cv = cstr.reshape(N_CORES, P, 3, Sg)
    for t in range(NT):
        base = t * P * N_CORES
        g, tg = t // 4, t % 4
        lg = int(Lg[g])
        lb = int(Lb[t])
        c1 = min(lb, 128)
        for core in range(N_CORES):
            sel = order[base + core + N_CORES * np.arange(P)]
            ids[core, t] = sel
            nvecs[core, :, t] = lens[sel]
            feats = table[idx_pad[sel, :lb]]          # [P, lb, 6]
            # transposed: chunk 0 -> col t, chunk 1 -> col NT + (t - t2_start)
            blk = np.zeros((P, 128, 6), dtype=f32)
            blk[:, :c1] = feats[:, :c1]
            tv[core, :, :, :, t] = blk.transpose(1, 2, 0).astype(f16)
            if lb > 128:
                blk2 = np.zeros((P, 128, 6), dtype=f32)
                blk2[:, :lb - 128] = feats[:, 128:lb]
                tv[core, :, :, :, NT + (t - t2_start)] = (
                    blk2.transpose(1, 2, 0).astype(f16))
            # cluster-major x/16,y/16,z/16 planes, group layout
            s0 = int(goff[g]) + tg * lg
            cv[core, :, :, s0:s0 + lb] = (
                feats[:, :, 0:3].transpose(0, 2, 1).astype(f16))
    return dict(tstr=tstr, cstr=cstr, nvecs=nvecs, ids=ids, Lb=Lb, Lg=Lg,
                Sg=Sg, ncol_p=ncol_p, NCOL=NCOL, n2=n2, t2_start=t2_start,
                goff=goff)


def _build_program(meta):
    import concourse.bass as bass
    import concourse.bacc as bacc
    import concourse.mybir as mybir
    from concourse.tile import TileContext

    dt = mybir.dt
    Alu = mybir.AluOpType
    Act = mybir.ActivationFunctionType

    Lb = meta["Lb"]; Lg = meta["Lg"]; Sg = meta["Sg"]
    ncol_p = meta["ncol_p"]; NCOL = meta["NCOL"]
    n2 = meta["n2"]; t2s = meta["t2_start"]; goff = meta["goff"]
    NM = 13                       # moment planes

    nc = bacc.Bacc("TRN2", target_bir_lowering=False, debug=False,
                   enable_asserts=False)
    tstr = nc.dram_tensor("tstr", [P, 6 * NCOL], dt.float16,
                          kind="ExternalInput")
    cstr = nc.dram_tensor("cstr", [P, 3 * Sg], dt.float16,
                          kind="ExternalInput")
    nvec_d = nc.dram_tensor("nvec", [P, NT], dt.float32, kind="ExternalInput")
    res = nc.dram_tensor("res", [P, 19 * NT], dt.float32,
                         kind="ExternalOutput")
    resdbg = nc.dram_tensor("resdbg", [P, 13 * NT], dt.float32,
                            kind="ExternalOutput")

    TINY = 1e-30
    NBLK = 8
    BLK = NCOL // NBLK            # column split for SBUF residency

    with TileContext(nc) as tc:
        with tc.tile_pool(name="ret", bufs=1) as ret, \
             tc.tile_pool(name="ps", bufs=1, space="PSUM") as ps:

            def full_tile(tag, k=1):
                return ret.tile([P, k * NT], dt.float32, tag=tag, name=tag)

            NV = full_tile("NV")
            RN = full_tile("RN")
            RAWM = ret.tile([P, NM * NT], dt.float32, tag="RAWM", name="RAWM")
            SCRAW = full_tile("SCRAW")
            MEANV = full_tile("MEANV"); STDV = full_tile("STDV")
            MODE = full_tile("MODE")
            B6 = full_tile("B6", 6)
            V3 = full_tile("V3", 3)
            CEN = full_tile("CEN", 3)
            NCEN = full_tile("NCEN", 3)
            STAIR = ret.tile([P, 256], dt.float16, tag="STAIR", name="STAIR")

            nc.sync.dma_start(out=NV[:], in_=nvec_d[:, :])
            nc.vector.reciprocal(RN[:], NV[:])
            nc.vector.memset(STAIR[:], 0.0)
            nc.vector.memset(STAIR[:, 128:129], 1.0)

            def tt(op, out, a, b):
                nc.vector.tensor_tensor(out=out, in0=a, in1=b, op=op)

            def ts(out, in0, s, op, s2=None, op1=None):
                kw = {}
                if op1 is not None:
                    kw["op1"] = op1
                nc.vector.tensor_scalar(out=out, in0=in0, scalar1=s,
                                        scalar2=s2, op0=op, **kw)

            def stt(out, in0, s, op0, op1, in1, accum=None):
                nc.vector.scalar_tensor_tensor(out=out, in0=in0, scalar=s,
                                               in1=in1, op0=op0, op1=op1,
                                               accum_out=accum)

            def act(out, in_, func, bias=0.0, scale=1.0, accum=None):
                nc.scalar.activation(out, in_, func, bias=bias, scale=scale,
                                     accum_out=accum)

            # ---------------- Pass A: PE moment sums -----------------
            PS = ps.tile([P, NM * NT], dt.float32, tag="PS", name="PS")
            first = [True]

            def pass_a_blk(h, hp):
                c0, c1 = h * BLK, (h + 1) * BLK
                W = c1 - c0
                # combined tile: 6 raw planes + 7 product planes
                al = hp.tile([P, 13 * W], dt.float16, tag="al",
                             name=f"al{h}")
                alv = al[:].rearrange("k (f c) -> k f c", f=13)
                nc.sync.dma_start(
                    out=alv[:, 0:6, :],
                    in_=tstr[:, :].rearrange("k (f c) -> k f c", f=6)[:, :, c0:c1])
                # squares on ACT, crosses on DVE; chunked so the
                # per-partition matmuls can start early
                NCK = 2
                CW = W // NCK
                for ck in range(NCK):
                    a0, a1 = ck * CW, (ck + 1) * CW
                    act(alv[:, 6:10, a0:a1], alv[:, 0:4, a0:a1], Act.Square)
                    tt(Alu.mult, alv[:, 10, a0:a1], alv[:, 0, a0:a1],
                       alv[:, 1, a0:a1])
                    tt(Alu.mult, alv[:, 11, a0:a1], alv[:, 0, a0:a1],
                       alv[:, 2, a0:a1])
                    tt(Alu.mult, alv[:, 12, a0:a1], alv[:, 1, a0:a1],
                       alv[:, 2, a0:a1])

                # per-partition matmuls (staircase window -> PSUM row p)
                p0, p1 = c0 // ncol_p, c1 // ncol_p
                psv = PS[:].rearrange("p (m t) -> p m t", m=NM)
                for p in range(p0, p1):
                    rb = p * ncol_p - c0
                    r13 = alv[:, :, rb:rb + ncol_p]
                    lhs = STAIR[:, 128 - p:256 - p]
                    st = first[0]
                    first[0] = False
                    last = (p == P - 1)
                    # start resets the WHOLE psum bank: only the very
                    # first matmul may carry start=True
                    nc.tensor.matmul(psv[:, :, 0:NT], lhs, r13[:, :, 0:NT],
                                     start=st, stop=False,
                                     skip_group_check=True)
                    if n2 > 0:
                        nc.tensor.matmul(psv[:, :, t2s:NT], lhs,
                                         r13[:, :, NT:ncol_p],
                                         start=False, stop=last,
                                         skip_group_check=True)
                    elif last:
                        nc.tensor.matmul(psv[:, 0:1, NT - 1:NT],
                                         STAIR[:, 0:1].broadcast_to((P, 1)),
                                         STAIR[:, 0:1], start=False, stop=True)

            with tc.tile_pool(name="blk", bufs=3) as hp:
                for h in range(NBLK):
                    pass_a_blk(h, hp)
            nc.vector.tensor_copy(out=RAWM[:], in_=PS[:])
            nc.sync.dma_start(out=resdbg[:, :], in_=RAWM[:])

            def msl(m):
                return RAWM[:, m * NT:(m + 1) * NT]

            # ---------------- cluster math ----------------------------
            def cluster_math():
                def tmp(tag, k=1):
                    return ret.tile([P, k * NT], dt.float32, tag=tag, name=tag)

                def sl(T, i):
                    return T[:, i * NT:(i + 1) * NT]

                SC1 = tmp("SC1"); SC2 = tmp("SC2"); SC3 = tmp("SC3")
                # centers (scaled units): c' = sum(x')/n
                for i in range(3):
                    tt(Alu.mult, sl(CEN, i), msl(i), RN[:])
                    ts(sl(NCEN, i), sl(CEN, i), -1.0, Alu.mult)
                # A' = prod - cen*sum  (xx,xy,xz,yy,yz,zz in cmap order)
                A = tmp("A", 6)
                # raw plane order: 6=xx,7=yy,8=zz,9=vv,10=xy,11=xz,12=yz
                pmap = [(0, 6, 0, 0), (1, 10, 0, 1), (2, 11, 0, 2),
                        (3, 7, 1, 1), (4, 12, 1, 2), (5, 8, 2, 2)]
                for q, pm, i, j in pmap:
                    tt(Alu.mult, SC1[:], sl(CEN, i), msl(j))
                    tt(Alu.subtract, sl(A, q), msl(pm), SC1[:])

                # value stats: meanv = sum(v)/n ; var = (sum(v^2)-mean*sum)/ (n-1)
                VAR = tmp("VAR"); NM1 = tmp("NM1")
                tt(Alu.mult, MEANV[:], msl(3), RN[:])
                tt(Alu.mult, VAR[:], MEANV[:], msl(3))
                tt(Alu.subtract, VAR[:], msl(9), VAR[:])
                ts(NM1[:], NV[:], 1.0, Alu.subtract)
                nc.vector.reciprocal(SC1[:], NM1[:])
                tt(Alu.mult, VAR[:], VAR[:], SC1[:])
                ts(VAR[:], VAR[:], 0.0, Alu.max)
                act(STDV[:], VAR[:], Act.Sqrt)

                # unpack semantic counts: ca -> c1 + 512*c2, cb -> c3 + 512*c4
                CNT = tmp("CNT", 4)
                HI_I = ret.tile([P, 2 * NT], dt.int32, tag="HI_I", name="HI_I")
                HIF = tmp("HIF", 2)
                for k, src in ((0, msl(4)), (1, msl(5))):
                    ts(sl(HIF, k), src, 1.0 / 512.0, Alu.mult)
                nc.vector.tensor_copy(out=HI_I[:], in_=HIF[:])
                nc.vector.tensor_copy(out=HIF[:], in_=HI_I[:])
                # c2 = floor(ca/512); c1 = ca - 512*c2
                for k, src in ((0, msl(4)), (1, msl(5))):
                    ts(SC1[:], sl(HIF, k), -512.0, Alu.mult)
                    tt(Alu.add, sl(CNT, 2 * k), src, SC1[:])
                    nc.vector.tensor_copy(out=sl(CNT, 2 * k + 1), in_=sl(HIF, k))

                BEST = tmp("BEST"); GT = tmp("GT"); KT = tmp("KT")
                tt(Alu.subtract, BEST[:], NV[:], sl(CNT, 0))
                for k in (1, 2, 3):
                    tt(Alu.subtract, BEST[:], BEST[:], sl(CNT, k))
                nc.vector.memset(MODE[:], 0.0)
                for k in range(1, 5):
                    ck = sl(CNT, k - 1)
                    tt(Alu.is_gt, GT[:], ck, BEST[:])
                    nc.vector.tensor_scalar(out=KT[:], in0=MODE[:],
                                            scalar1=-1.0, scalar2=float(k),
                                            op0=Alu.mult, op1=Alu.add)
                    tt(Alu.mult, KT[:], KT[:], GT[:])
                    tt(Alu.add, MODE[:], MODE[:], KT[:])
                    tt(Alu.max, BEST[:], BEST[:], ck)

                # eigenvalues: trig closed form on A'
                Q = tmp("Q"); P1 = tmp("P1"); P2 = tmp("P2"); PP = tmp("PP")
                RP = tmp("RP"); DET = tmp("DET"); RR = tmp("RR"); SS = tmp("SS")
                AT = tmp("AT"); PHI = tmp("PHI")
                W0 = tmp("W0"); W1 = tmp("W1"); W2 = tmp("W2"); RW2 = tmp("RW2")
                DIRWT = tmp("DIRWT")
                NB = tmp("NB", 6)

                tt(Alu.add, Q[:], sl(A, 0), sl(A, 3))
                tt(Alu.add, Q[:], Q[:], sl(A, 5))
                ts(Q[:], Q[:], 1.0 / 3.0, Alu.mult)

                tt(Alu.mult, P1[:], sl(A, 1), sl(A, 1))
                tt(Alu.mult, SC1[:], sl(A, 2), sl(A, 2))
                tt(Alu.add, P1[:], P1[:], SC1[:])
                tt(Alu.mult, SC1[:], sl(A, 4), sl(A, 4))
                tt(Alu.add, P1[:], P1[:], SC1[:])

                BD = tmp("BD", 3)
                tt(Alu.subtract, sl(BD, 0), sl(A, 0), Q[:])
                tt(Alu.subtract, sl(BD, 1), sl(A, 3), Q[:])
                tt(Alu.subtract, sl(BD, 2), sl(A, 5), Q[:])
                tt(Alu.mult, P2[:], sl(BD, 0), sl(BD, 0))
                tt(Alu.mult, SC1[:], sl(BD, 1), sl(BD, 1))
                tt(Alu.add, P2[:], P2[:], SC1[:])
                tt(Alu.mult, SC1[:], sl(BD, 2), sl(BD, 2))
                tt(Alu.add, P2[:], P2[:], SC1[:])
                stt(P2[:], P1[:], 2.0, Alu.mult, Alu.add, P2[:])
                ts(PP[:], P2[:], 1.0 / 6.0, Alu.mult)
                act(PP[:], PP[:], Act.Sqrt)
                ts(SC1[:], PP[:], TINY, Alu.max)
                nc.vector.reciprocal(RP[:], SC1[:])

                tt(Alu.mult, sl(NB, 0), sl(BD, 0), RP[:])
                tt(Alu.mult, sl(NB, 1), sl(A, 1), RP[:])
                tt(Alu.mult, sl(NB, 2), sl(A, 2), RP[:])
                tt(Alu.mult, sl(NB, 3), sl(BD, 1), RP[:])
                tt(Alu.mult, sl(NB, 4), sl(A, 4), RP[:])
                tt(Alu.mult, sl(NB, 5), sl(BD, 2), RP[:])

                tt(Alu.mult, SC1[:], sl(NB, 3), sl(NB, 5))
                tt(Alu.mult, SC2[:], sl(NB, 4), sl(NB, 4))
                tt(Alu.subtract, SC1[:], SC1[:], SC2[:])
                tt(Alu.mult, DET[:], sl(NB, 0), SC1[:])
                tt(Alu.mult, SC1[:], sl(NB, 1), sl(NB, 5))
                tt(Alu.mult, SC2[:], sl(NB, 4), sl(NB, 2))
                tt(Alu.subtract, SC1[:], SC1[:], SC2[:])
                tt(Alu.mult, SC1[:], sl(NB, 1), SC1[:])
                tt(Alu.subtract, DET[:], DET[:], SC1[:])
                tt(Alu.mult, SC1[:], sl(NB, 1), sl(NB, 4))
                tt(Alu.mult, SC2[:], sl(NB, 3), sl(NB, 2))
                tt(Alu.subtract, SC1[:], SC1[:], SC2[:])
                tt(Alu.mult, SC1[:], sl(NB, 2), SC1[:])
                tt(Alu.add, DET[:], DET[:], SC1[:])

                ts(RR[:], DET[:], 0.5, Alu.mult)
                ts(RR[:], RR[:], -1.0, Alu.max)
                ts(RR[:], RR[:], 1.0, Alu.min)
                tt(Alu.mult, SS[:], RR[:], RR[:])
                nc.vector.tensor_scalar(out=SS[:], in0=SS[:], scalar1=-1.0,
                                        scalar2=1.0, op0=Alu.mult, op1=Alu.add)
                ts(SS[:], SS[:], 0.0, Alu.max)
                act(SS[:], SS[:], Act.Sqrt)
                UA = tmp("UA"); UB = tmp("UB")
                ts(SC1[:], RR[:], -1.0, Alu.mult)
                tt(Alu.max, SC1[:], SC1[:], RR[:])
                ts(SS[:], SS[:], TINY, Alu.max)
                nc.vector.reciprocal(SC2[:], SS[:])
                tt(Alu.mult, UA[:], SC1[:], SC2[:])
                ts(SC1[:], UA[:], TINY, Alu.max)
                nc.vector.reciprocal(UB[:], SC1[:])
                tt(Alu.min, SC2[:], UA[:], UB[:])
                act(SC2[:], SC2[:], Act.Arctan)
                ts(SC1[:], UA[:], 1.0, Alu.is_gt)
                nc.vector.tensor_scalar(out=SC3[:], in0=SC2[:], scalar1=-2.0,
                                        scalar2=_PI / 2.0, op0=Alu.mult,
                                        op1=Alu.add)
                tt(Alu.mult, SC3[:], SC3[:], SC1[:])
                tt(Alu.add, SC2[:], SC2[:], SC3[:])
                ts(SC3[:], RR[:], 0.0, Alu.is_lt)
                nc.vector.tensor_scalar(out=SC3[:], in0=SC3[:], scalar1=-2.0,
                                        scalar2=1.0, op0=Alu.mult, op1=Alu.add)
                tt(Alu.mult, AT[:], SC2[:], SC3[:])
                nc.vector.tensor_scalar(out=PHI[:], in0=AT[:],
                                        scalar1=-1.0 / 3.0,
                                        scalar2=_PI / 6.0 + _PI / 2.0,
                                        op0=Alu.mult, op1=Alu.add)
                act(SC1[:], PHI[:], Act.Sin)
                tt(Alu.mult, SC1[:], SC1[:], PP[:])
                stt(W2[:], SC1[:], 2.0, Alu.mult, Alu.add, Q[:])
                nc.vector.tensor_scalar(out=PHI[:], in0=AT[:],
                                        scalar1=-1.0 / 3.0,
                                        scalar2=_PI / 6.0 + _PI / 6.0,
                                        op0=Alu.mult, op1=Alu.add)
                act(SC1[:], PHI[:], Act.Sin)
                tt(Alu.mult, SC1[:], SC1[:], PP[:])
                stt(W0[:], SC1[:], -2.0, Alu.mult, Alu.add, Q[:])
                ts(SC1[:], Q[:], 3.0, Alu.mult)
                tt(Alu.subtract, W1[:], SC1[:], W0[:])
                tt(Alu.subtract, W1[:], W1[:], W2[:])

                ts(SC1[:], W2[:], TINY, Alu.max)
                nc.vector.reciprocal(RW2[:], SC1[:])
                tt(Alu.mult, DIRWT[:], W1[:], RW2[:])
                nc.vector.tensor_scalar(out=DIRWT[:], in0=DIRWT[:],
                                        scalar1=-1.0, scalar2=1.0,
                                        op0=Alu.mult, op1=Alu.add)
                for q in range(6):
                    tt(Alu.mult, sl(B6, q), sl(A, q), RW2[:])

                CD = tmp("CD", 3)
                DD = tmp("DD", 3)
                for qi, ai in enumerate((0, 3, 5)):
                    tt(Alu.subtract, sl(CD, qi), sl(A, ai), W0[:])
                    tt(Alu.subtract, sl(DD, qi), sl(A, ai), W1[:])
                M9 = tmp("M9", 9)

                def mcol(colq, dv):
                    crow = [(sl(CD, 0), sl(A, 1), sl(A, 2)),
                            (sl(A, 1), sl(CD, 1), sl(A, 4)),
                            (sl(A, 2), sl(A, 4), sl(CD, 2))]
                    for r in range(3):
                        a0, a1, a2 = crow[r]
                        tt(Alu.mult, SC1[:], a0, dv[0])
                        tt(Alu.mult, SC2[:], a1, dv[1])
                        tt(Alu.add, SC1[:], SC1[:], SC2[:])
                        tt(Alu.mult, SC2[:], a2, dv[2])
                        tt(Alu.add, sl(M9, colq * 3 + r), SC1[:], SC2[:])

                mcol(0, (sl(DD, 0), sl(A, 1), sl(A, 2)))
                mcol(1, (sl(A, 1), sl(DD, 1), sl(A, 4)))
                mcol(2, (sl(A, 2), sl(A, 4), sl(DD, 2)))

                CN = tmp("CN", 3)
                for j in range(3):
                    tt(Alu.mult, sl(CN, j), sl(M9, j * 3), sl(M9, j * 3))
                    tt(Alu.mult, SC1[:], sl(M9, j * 3 + 1), sl(M9, j * 3 + 1))
                    tt(Alu.add, sl(CN, j), sl(CN, j), SC1[:])
                    tt(Alu.mult, SC1[:], sl(M9, j * 3 + 2), sl(M9, j * 3 + 2))
                    tt(Alu.add, sl(CN, j), sl(CN, j), SC1[:])
                NBEST = tmp("NBEST")
                for i in range(3):
                    nc.vector.tensor_copy(out=sl(V3, i), in_=sl(M9, i))
                nc.vector.tensor_copy(out=NBEST[:], in_=sl(CN, 0))
                for j in (1, 2):
                    tt(Alu.is_gt, GT[:], sl(CN, j), NBEST[:])
                    for i in range(3):
                        tt(Alu.subtract, SC1[:], sl(M9, j * 3 + i), sl(V3, i))
                        tt(Alu.mult, SC1[:], SC1[:], GT[:])
                        tt(Alu.add, sl(V3, i), sl(V3, i), SC1[:])
                    tt(Alu.max, NBEST[:], NBEST[:], sl(CN, j))
                ts(SC1[:], NBEST[:], 1e-37, Alu.max)
                act(SC2[:], SC1[:], Act.Sqrt)
                nc.vector.reciprocal(SC2[:], SC2[:])
                for i in range(3):
                    tt(Alu.mult, sl(V3, i), sl(V3, i), SC2[:])
                return DIRWT

            DIRWT = cluster_math()

            # ---------------- pass B (cluster-major, scaled units) ----
            from contextlib import ExitStack
            _pb_stack = ExitStack()
            pbp = _pb_stack.enter_context(tc.tile_pool(name="pbp", bufs=1))
            pb = _pb_stack.enter_context(tc.tile_pool(name="pb", bufs=2))
            CSTR = pbp.tile([P, 3 * Sg], dt.float16, tag="CSTR", name="CSTR")
            nc.sync.dma_start(out=CSTR[:], in_=cstr[:, :])
            TP = pbp.tile([P, Sg], dt.float16, tag="TP", name="TP")

            def cm_plane(i):   # cluster-major input plane i
                return CSTR[:, i * Sg:(i + 1) * Sg]

            # NK = -(c . v0)  (also reused as T0 in sign phase)
            NK = full_tile("NK")
            SCX = full_tile("SCX")
            tt(Alu.mult, NK[:], CEN[:, 0:NT], V3[:, 0:NT])
            tt(Alu.mult, SCX[:], CEN[:, NT:2 * NT], V3[:, NT:2 * NT])
            tt(Alu.add, NK[:], NK[:], SCX[:])
            tt(Alu.mult, SCX[:], CEN[:, 2 * NT:3 * NT], V3[:, 2 * NT:3 * NT])
            tt(Alu.add, NK[:], NK[:], SCX[:])
            ts(NK[:], NK[:], -1.0, Alu.mult)

            for g in range(NG):
                lg = int(Lg[g]); s0 = int(goff[g]); w = 4 * lg
                SQ3 = pb.tile([P, 3 * w], dt.float16, tag="SQ3", name=f"SQ3{g}")
                QA = pb.tile([P, w], dt.float16, tag="QA", name=f"QA{g}")
                QB = pb.tile([P, w], dt.float16, tag="QB", name=f"QB{g}")
                T2 = pb.tile([P, w], dt.float16, tag="T2", name=f"T2{g}")
                R2 = pb.tile([P, w], dt.float16, tag="R2", name=f"R2{g}")
                RPL = pb.tile([P, w], dt.float16, tag="RPL", name=f"RPL{g}")
                for tg in range(4):
                    t = 4 * g + tg
                    sl0 = s0 + tg * lg
                    # centered squares straight from raw x via ACT bias
                    for i in range(3):
                        act(SQ3[:, i * w + tg * lg:i * w + (tg + 1) * lg],
                            cm_plane(i)[:, sl0:sl0 + lg], Act.Square,
                            bias=NCEN[:, i * NT + t:i * NT + t + 1])
                    # T = x*v0x + NK, += y*v0y, += z*v0z
                    nc.vector.tensor_scalar(
                        out=TP[:, sl0:sl0 + lg],
                        in0=cm_plane(0)[:, sl0:sl0 + lg],
                        scalar1=V3[:, 0 * NT + t:0 * NT + t + 1],
                        scalar2=NK[:, t:t + 1],
                        op0=Alu.mult, op1=Alu.add)
                    stt(TP[:, sl0:sl0 + lg], cm_plane(1)[:, sl0:sl0 + lg],
                        V3[:, 1 * NT + t:1 * NT + t + 1], Alu.mult, Alu.add,
                        TP[:, sl0:sl0 + lg])
                    stt(TP[:, sl0:sl0 + lg], cm_plane(2)[:, sl0:sl0 + lg],
                        V3[:, 2 * NT + t:2 * NT + t + 1], Alu.mult, Alu.add,
                        TP[:, sl0:sl0 + lg])
                # q and r per group; first add on gpsimd
                nc.gpsimd.tensor_tensor(out=QA[:], in0=SQ3[:, 0:w],
                                        in1=SQ3[:, w:2 * w], op=Alu.add)
                tt(Alu.add, QB[:], QA[:], SQ3[:, 2 * w:3 * w])
                tt(Alu.mult, T2[:], TP[:, s0:s0 + w], TP[:, s0:s0 + w])
                tt(Alu.subtract, R2[:], QB[:], T2[:])
                ts(R2[:], R2[:], 0.0, Alu.max)
                act(RPL[:], R2[:], Act.Sqrt)
                for tg in range(4):
                    t = 4 * g + tg
                    stt(T2[:, tg * lg:(tg + 1) * lg],
                        TP[:, s0 + tg * lg:s0 + (tg + 1) * lg], 1.0,
                        Alu.mult, Alu.mult,
                        RPL[:, tg * lg:(tg + 1) * lg],
                        accum=SCRAW[:, t:t + 1])

            # ---------------- sign + output --------------------------
            def sign_phase():
                def tmp(tag, k=1):
                    return ret.tile([P, k * NT], dt.float32, tag=tag, name=tag)

                def sl(T, i):
                    return T[:, i * NT:(i + 1) * NT]

                CC = tmp("CC"); R0 = tmp("R0")
                SCV = tmp("SCV"); FAC = tmp("FAC"); SC9 = tmp("SC9")
                GT9 = tmp("GT9"); NPAD = tmp("NPAD")
                T0 = NK
                tt(Alu.mult, CC[:], sl(CEN, 0), sl(CEN, 0))
                tt(Alu.mult, SC9[:], sl(CEN, 1), sl(CEN, 1))
                tt(Alu.add, CC[:], CC[:], SC9[:])
                tt(Alu.mult, SC9[:], sl(CEN, 2), sl(CEN, 2))
                tt(Alu.add, CC[:], CC[:], SC9[:])
                tt(Alu.mult, SC9[:], T0[:], T0[:])
                tt(Alu.subtract, R0[:], CC[:], SC9[:])
                ts(R0[:], R0[:], 0.0, Alu.max)
                act(R0[:], R0[:], Act.Sqrt)
                # padded slots use the group padded length Lg
                for t in range(NT):
                    lg = int(Lg[t // 4])
                    nc.vector.tensor_scalar(
                        out=NPAD[:, t:t + 1],
                        in0=NV[:, t:t + 1], scalar1=-1.0,
                        scalar2=float(lg), op0=Alu.mult, op1=Alu.add)
                tt(Alu.mult, SC9[:], T0[:], R0[:])
                tt(Alu.mult, SC9[:], SC9[:], NPAD[:])
                tt(Alu.subtract, SCV[:], SCRAW[:], SC9[:])
                ts(GT9[:], SCV[:], 0.0, Alu.is_lt)
                nc.vector.tensor_scalar(out=GT9[:], in0=GT9[:], scalar1=-2.0,
                                        scalar2=1.0, op0=Alu.mult, op1=Alu.add)
                tt(Alu.mult, FAC[:], DIRWT[:], GT9[:])
                for i in range(3):
                    tt(Alu.mult, sl(V3, i), sl(V3, i), FAC[:])
                # unscale centers: x16
                for i in range(3):
                    ts(sl(CEN, i), sl(CEN, i), SCL, Alu.mult)
                OUTST = ret.tile([P, 19 * NT], dt.float32, tag="OUTST",
                                 name="OUTST")
                for j, pl in [(0, sl(CEN, 0)), (1, sl(CEN, 1)), (2, sl(CEN, 2)),
                              (3, sl(B6, 0)), (4, sl(B6, 1)), (5, sl(B6, 2)),
                              (6, sl(B6, 1)), (7, sl(B6, 3)), (8, sl(B6, 4)),
                              (9, sl(B6, 2)), (10, sl(B6, 4)), (11, sl(B6, 5)),
                              (12, sl(V3, 0)), (13, sl(V3, 1)), (14, sl(V3, 2)),
                              (15, NV[:]), (16, MEANV[:]), (17, STDV[:]),
                              (18, MODE[:])]:
                    nc.vector.tensor_copy(
                        out=OUTST[:, j * NT:(j + 1) * NT], in_=pl)
                nc.sync.dma_start(out=res[:, :], in_=OUTST[:])

            sign_phase()
            _pb_stack.close()

    nc.compile()
    return nc


_cache = {}
_last = None


def kernel(data, clust_idx, clust_len):
    global N, C, L, NT, NG
    data = np.asarray(data)
    clust_idx = np.asarray(clust_idx)
    N = int(data.shape[0])
    C, L = int(clust_idx.shape[0]), int(clust_idx.shape[1])
    assert C % (P * N_CORES) == 0
    NT = C // (P * N_CORES)
    NG = NT // 4
    meta = _host_prep(data, clust_idx, clust_len)

    key = (tuple(int(x) for x in meta["Lb"]), N, C)
    if key not in _cache:
        _cache[key] = _build_program(meta)
    nc = _cache[key]

    from concourse.bass_utils import run_bass_kernel_spmd
    in_maps = [{"tstr": meta["tstr"][c], "cstr": meta["cstr"][c],
                "nvec": meta["nvecs"][c]} for c in range(N_CORES)]
    global _last
    _last = (nc, in_maps)
    res = run_bass_kernel_spmd(nc, in_maps, list(range(N_CORES)))

    ids = meta["ids"]
    out = np.zeros((C, 19), dtype=f32)
    for core in range(N_CORES):
        r = res.results[core]["res"].reshape(P, 19, NT)
        for t in range(NT):
            out[ids[core, t]] = r[:, :, t]
    return out


# revision 17
# speedup vs baseline: 1.3236x; 1.0205x over previous
"""Trainium2 Bass kernel for nn_ClustGeoNodeEncoder (segment_reduce).

v2 architecture (PE-accelerated moments):
  - Host sorts clusters by length, deals them round-robin to 8 cores x 32
    tiles of 128 clusters (one cluster per partition per tile), and stages
    TWO fp16 streams per core:
      * transposed stream: [128 element-slots, ncols] feature planes
        (x/16, y/16, z/16, v, ca, cb) where each column holds up to 128
        elements of one cluster chunk (2 chunks for tiles padded > 128).
        ca = oh1 + 512*oh2, cb = oh3 + 512*oh4 pack the semantic one-hots
        (exact in fp16; sums stay < 2^24 so fp32 PSUM accumulation is
        exact).  Columns are ordered partition-major so partition p's
        clusters occupy a contiguous 13*NT-column window.
      * cluster-major stream: [128 clusters, S] x/16, y/16, z/16 planes,
        feature-major per group of 4 equal-padded tiles (for pass B).
  - Device pass A: ACT squares the coordinate/value planes, DVE forms the
    three cross-product planes (2x fp16 mode), and the TensorEngine
    reduces all 13 moment planes per cluster with ones-column matmuls:
    a staircase window (ones only in absolute column 128 of a [128, 256]
    buffer) places partition p's sums into PSUM row p; 128 accumulating
    matmuls cover all partitions, long-tile second chunks accumulate into
    the same PSUM columns.  One [128, 13*NT] PSUM->SBUF copy evacuates
    every raw moment.
  - Cluster math on [128, NT] fp32 planes: centers, centered scatter
    matrix A (scale-free in /16 units), closed-form trig eigenvalues,
    principal eigenvector via spectral projector, B = A/w2, dirwt, value
    stats, semantic mode via int32-truncation unpack of ca/cb.
  - Pass B (cluster-major): ts-centering (4x fp16), per-tile stt dot with
    v0, ACT group squares + sqrt, stt-accum orientation statistic sc;
    padded-slot closed-form correction, sign flip, output DMA.
"""

import sys

for _p in ("/opt/trn_rl_repo",):
    if _p not in sys.path:
        sys.path.insert(0, _p)

import numpy as np

N = 2_000_000
C = 32768
L = 256
N_CORES = 8
P = 128
NT = C // (P * N_CORES)  # 32 tiles per core
NG = 8                   # pass-B tile groups (4 tiles each, shared pad)
f32 = np.float32
f16 = np.float16

_PI = float(np.pi)
SCL = 16.0               # coordinate pre-scale (powers of 2 are exact)


def _host_prep(data, clust_idx, clust_len):
    data = np.asarray(data, dtype=f32)
    clust_idx = np.asarray(clust_idx).astype(np.int64)
    lens = np.asarray(clust_len).astype(np.int64)

    # feature table: x/16, y/16, z/16, v, ca, cb ; row N = zeros for padding
    table = np.zeros((N + 1, 6), dtype=f32)
    table[:N, 0:3] = data[:, 0:3] / SCL
    table[:N, 3] = data[:, 4]
    sem = data[:N, 5].astype(np.int32)
    ca = (sem == 1).astype(f32) + 512.0 * (sem == 2)
    cb = (sem == 3).astype(f32) + 512.0 * (sem == 4)
    table[:N, 4] = ca
    table[:N, 5] = cb

    order = np.argsort(lens, kind="stable")
    # rank r: tile t = r // (P*N_CORES); slot s = r % (P*N_CORES)
    # core = s % N_CORES ; partition = s // N_CORES
    Lb = np.zeros(NT, dtype=np.int64)
    for t in range(NT):
        Lb[t] = lens[order[t * P * N_CORES:(t + 1) * P * N_CORES]].max()
    # pass-B groups of 4 tiles share a padded length
    Lg = np.zeros(NG, dtype=np.int64)
    for g in range(NG):
        Lg[g] = Lb[4 * g:4 * g + 4].max()
    Sg = int(Lg.sum() * 4)          # cluster-major columns per partition

    chunks = np.maximum(1, (Lb + 127) // 128)     # 1 or 2 per tile
    n2 = int((chunks == 2).sum())                 # trailing tiles (sorted)
    t2_start = NT - n2
    ncol_p = NT + n2                              # columns per partition
    NCOL = P * ncol_p

    ar = np.arange(L)[None, :]
    idx_pad = np.where(ar < lens[:, None], clust_idx, N)

    ids = np.zeros((N_CORES, NT, P), dtype=np.int64)
    nvecs = np.zeros((N_CORES, P, NT), dtype=f32)
    # transposed stream: [core][128 slots, 6 planes * NCOL] plane-major
    tstr = np.zeros((N_CORES, P, 6 * NCOL), dtype=f16)
    # cluster-major stream: [core][128, 3 * Sg] plane-major, group-padded
    cstr = np.zeros((N_CORES, P, 3 * Sg), dtype=f16)

    goff = np.zeros(NG, dtype=np.int64)
    off = 0
    for g in range(NG):
        goff[g] = off
        off += 4 * int(Lg[g])

    tv = tstr.reshape(N_CORES, P, 6, P, ncol_p)
    cv = cstr.reshape(N_CORES, P, 3, Sg)
    for t in range(NT):
        base = t * P * N_CORES
        g, tg = t // 4, t % 4
        lg = int(Lg[g])
        lb = int(Lb[t])
        c1 = min(lb, 128)
        for core in range(N_CORES):
            sel = order[base + core + N_CORES * np.arange(P)]
            ids[core, t] = sel
            nvecs[core, :, t] = lens[sel]
            feats = table[idx_pad[sel, :lb]]          # [P, lb, 6]
            # transposed: chunk 0 -> col t, chunk 1 -> col NT + (t - t2_start)
            blk = np.zeros((P, 128, 6), dtype=f32)
            blk[:, :c1] = feats[:, :c1]
            tv[core, :, :, :, t] = blk.transpose(1, 2, 0).astype(f16)
            if lb > 128:
                blk2 = np.zeros((P, 128, 6), dtype=f32)
                blk2[:, :lb - 128] = feats[:, 128:lb]
                tv[core, :, :, :, NT + (t - t2_start)] = (
                    blk2.transpose(1, 2, 0).astype(f16))
            # cluster-major x/16,y/16,z/16 planes, group layout
            s0 = int(goff[g]) + tg * lg
            cv[core, :, :, s0:s0 + lb] = (
                feats[:, :, 0:3].transpose(0, 2, 1).astype(f16))
    return dict(tstr=tstr, cstr=cstr, nvecs=nvecs, ids=ids, Lb=Lb, Lg=Lg,
                Sg=Sg, ncol_p=ncol_p, NCOL=NCOL, n2=n2, t2_start=t2_start,
                goff=goff)


def _build_program(meta):
    import concourse.bass as bass
    import concourse.bacc as bacc
    import concourse.mybir as mybir
    from concourse.tile import TileContext

    dt = mybir.dt
    Alu = mybir.AluOpType
    Act = mybir.ActivationFunctionType

    Lb = meta["Lb"]; Lg = meta["Lg"]; Sg = meta["Sg"]
    ncol_p = meta["ncol_p"]; NCOL = meta["NCOL"]
    n2 = meta["n2"]; t2s = meta["t2_start"]; goff = meta["goff"]
    NM = 13                       # moment planes

    nc = bacc.Bacc("TRN2", target_bir_lowering=False, debug=False,
                   enable_asserts=False)
    tstr = nc.dram_tensor("tstr", [P, 6 * NCOL], dt.float16,
                          kind="ExternalInput")
    cstr = nc.dram_tensor("cstr", [P, 3 * Sg], dt.float16,
                          kind="ExternalInput")
    nvec_d = nc.dram_tensor("nvec", [P, NT], dt.float32, kind="ExternalInput")
    res = nc.dram_tensor("res", [P, 19 * NT], dt.float32,
                         kind="ExternalOutput")

    TINY = 1e-30
    NBLK = 8
    BLK = NCOL // NBLK            # column split for SBUF residency

    with TileContext(nc) as tc:
        with tc.tile_pool(name="ret", bufs=1) as ret, \
             tc.tile_pool(name="ps", bufs=1, space="PSUM") as ps:

            def full_tile(tag, k=1):
                return ret.tile([P, k * NT], dt.float32, tag=tag, name=tag)

            NV = full_tile("NV")
            RN = full_tile("RN")
            RAWM = ret.tile([P, NM * NT], dt.float32, tag="RAWM", name="RAWM")
            SCRAW = full_tile("SCRAW")
            MEANV = full_tile("MEANV"); STDV = full_tile("STDV")
            MODE = full_tile("MODE")
            B6 = full_tile("B6", 6)
            V3 = full_tile("V3", 3)
            CEN = full_tile("CEN", 3)
            NCEN = full_tile("NCEN", 3)
            STAIR = ret.tile([P, 256], dt.float16, tag="STAIR", name="STAIR")

            nc.sync.dma_start(out=NV[:], in_=nvec_d[:, :])
            nc.vector.reciprocal(RN[:], NV[:])
            nc.vector.memset(STAIR[:], 0.0)
            nc.vector.memset(STAIR[:, 128:129], 1.0)

            def tt(op, out, a, b):
                nc.vector.tensor_tensor(out=out, in0=a, in1=b, op=op)

            def ts(out, in0, s, op, s2=None, op1=None):
                kw = {}
                if op1 is not None:
                    kw["op1"] = op1
                nc.vector.tensor_scalar(out=out, in0=in0, scalar1=s,
                                        scalar2=s2, op0=op, **kw)

            def stt(out, in0, s, op0, op1, in1, accum=None):
                nc.vector.scalar_tensor_tensor(out=out, in0=in0, scalar=s,
                                               in1=in1, op0=op0, op1=op1,
                                               accum_out=accum)

            def act(out, in_, func, bias=0.0, scale=1.0, accum=None):
                nc.scalar.activation(out, in_, func, bias=bias, scale=scale,
                                     accum_out=accum)

            # ---------------- Pass A: PE moment sums -----------------
            PS = ps.tile([P, NM * NT], dt.float32, tag="PS", name="PS")
            first = [True]

            def pass_a_blk(h, hp):
                c0, c1 = h * BLK, (h + 1) * BLK
                W = c1 - c0
                # combined tile: 6 raw planes + 7 product planes
                al = hp.tile([P, 13 * W], dt.float16, tag="al",
                             name=f"al{h}")
                alv = al[:].rearrange("k (f c) -> k f c", f=13)
                nc.sync.dma_start(
                    out=alv[:, 0:6, :],
                    in_=tstr[:, :].rearrange("k (f c) -> k f c", f=6)[:, :, c0:c1])
                # squares on ACT, crosses on DVE; chunked so the
                # per-partition matmuls can start early
                NCK = 2
                CW = W // NCK
                for ck in range(NCK):
                    a0, a1 = ck * CW, (ck + 1) * CW
                    act(alv[:, 6:10, a0:a1], alv[:, 0:4, a0:a1], Act.Square)
                    tt(Alu.mult, alv[:, 10, a0:a1], alv[:, 0, a0:a1],
                       alv[:, 1, a0:a1])
                    tt(Alu.mult, alv[:, 11, a0:a1], alv[:, 0, a0:a1],
                       alv[:, 2, a0:a1])
                    tt(Alu.mult, alv[:, 12, a0:a1], alv[:, 1, a0:a1],
                       alv[:, 2, a0:a1])

                # per-partition matmuls (staircase window -> PSUM row p)
                p0, p1 = c0 // ncol_p, c1 // ncol_p
                psv = PS[:].rearrange("p (m t) -> p m t", m=NM)
                for p in range(p0, p1):
                    rb = p * ncol_p - c0
                    r13 = alv[:, :, rb:rb + ncol_p]
                    lhs = STAIR[:, 128 - p:256 - p]
                    st = first[0]
                    first[0] = False
                    last = (p == P - 1)
                    # start resets the WHOLE psum bank: only the very
                    # first matmul may carry start=True
                    nc.tensor.matmul(psv[:, :, 0:NT], lhs, r13[:, :, 0:NT],
                                     start=st, stop=False,
                                     skip_group_check=True)
                    if n2 > 0:
                        nc.tensor.matmul(psv[:, :, t2s:NT], lhs,
                                         r13[:, :, NT:ncol_p],
                                         start=False, stop=last,
                                         skip_group_check=True)
                    elif last:
                        nc.tensor.matmul(psv[:, 0:1, NT - 1:NT],
                                         STAIR[:, 0:1].broadcast_to((P, 1)),
                                         STAIR[:, 0:1], start=False, stop=True)

            with tc.tile_pool(name="blk", bufs=3) as hp:
                for h in range(NBLK):
                    pass_a_blk(h, hp)
            nc.vector.tensor_copy(out=RAWM[:], in_=PS[:])

            def msl(m):
                return RAWM[:, m * NT:(m + 1) * NT]

            # ---------------- cluster math ----------------------------
            def cluster_math():
                def tmp(tag, k=1):
                    return ret.tile([P, k * NT], dt.float32, tag=tag, name=tag)

                def sl(T, i):
                    return T[:, i * NT:(i + 1) * NT]

                SC1 = tmp("SC1"); SC2 = tmp("SC2"); SC3 = tmp("SC3")
                # centers (scaled units): c' = sum(x')/n
                for i in range(3):
                    tt(Alu.mult, sl(CEN, i), msl(i), RN[:])
                    ts(sl(NCEN, i), sl(CEN, i), -1.0, Alu.mult)
                # A' = prod - cen*sum  (xx,xy,xz,yy,yz,zz in cmap order)
                A = tmp("A", 6)
                # raw plane order: 6=xx,7=yy,8=zz,9=vv,10=xy,11=xz,12=yz
                pmap = [(0, 6, 0, 0), (1, 10, 0, 1), (2, 11, 0, 2),
                        (3, 7, 1, 1), (4, 12, 1, 2), (5, 8, 2, 2)]
                for q, pm, i, j in pmap:
                    tt(Alu.mult, SC1[:], sl(CEN, i), msl(j))
                    tt(Alu.subtract, sl(A, q), msl(pm), SC1[:])

                # value stats: meanv = sum(v)/n ; var = (sum(v^2)-mean*sum)/ (n-1)
                VAR = tmp("VAR"); NM1 = tmp("NM1")
                tt(Alu.mult, MEANV[:], msl(3), RN[:])
                tt(Alu.mult, VAR[:], MEANV[:], msl(3))
                tt(Alu.subtract, VAR[:], msl(9), VAR[:])
                ts(NM1[:], NV[:], 1.0, Alu.subtract)
                nc.vector.reciprocal(SC1[:], NM1[:])
                tt(Alu.mult, VAR[:], VAR[:], SC1[:])
                ts(VAR[:], VAR[:], 0.0, Alu.max)
                act(STDV[:], VAR[:], Act.Sqrt)

                # unpack semantic counts: ca -> c1 + 512*c2, cb -> c3 + 512*c4
                CNT = tmp("CNT", 4)
                HI_I = ret.tile([P, 2 * NT], dt.int32, tag="HI_I", name="HI_I")
                HIF = tmp("HIF", 2)
                for k, src in ((0, msl(4)), (1, msl(5))):
                    ts(sl(HIF, k), src, 1.0 / 512.0, Alu.mult)
                nc.vector.tensor_copy(out=HI_I[:], in_=HIF[:])
                nc.vector.tensor_copy(out=HIF[:], in_=HI_I[:])
                # c2 = floor(ca/512); c1 = ca - 512*c2
                for k, src in ((0, msl(4)), (1, msl(5))):
                    ts(SC1[:], sl(HIF, k), -512.0, Alu.mult)
                    tt(Alu.add, sl(CNT, 2 * k), src, SC1[:])
                    nc.vector.tensor_copy(out=sl(CNT, 2 * k + 1), in_=sl(HIF, k))

                BEST = tmp("BEST"); GT = tmp("GT"); KT = tmp("KT")
                tt(Alu.subtract, BEST[:], NV[:], sl(CNT, 0))
                for k in (1, 2, 3):
                    tt(Alu.subtract, BEST[:], BEST[:], sl(CNT, k))
                nc.vector.memset(MODE[:], 0.0)
                for k in range(1, 5):
                    ck = sl(CNT, k - 1)
                    tt(Alu.is_gt, GT[:], ck, BEST[:])
                    nc.vector.tensor_scalar(out=KT[:], in0=MODE[:],
                                            scalar1=-1.0, scalar2=float(k),
                                            op0=Alu.mult, op1=Alu.add)
                    tt(Alu.mult, KT[:], KT[:], GT[:])
                    tt(Alu.add, MODE[:], MODE[:], KT[:])
                    tt(Alu.max, BEST[:], BEST[:], ck)

                # eigenvalues: trig closed form on A'
                Q = tmp("Q"); P1 = tmp("P1"); P2 = tmp("P2"); PP = tmp("PP")
                RP = tmp("RP"); DET = tmp("DET"); RR = tmp("RR"); SS = tmp("SS")
                AT = tmp("AT"); PHI = tmp("PHI")
                W0 = tmp("W0"); W1 = tmp("W1"); W2 = tmp("W2"); RW2 = tmp("RW2")
                DIRWT = tmp("DIRWT")
                NB = tmp("NB", 6)

                tt(Alu.add, Q[:], sl(A, 0), sl(A, 3))
                tt(Alu.add, Q[:], Q[:], sl(A, 5))
                ts(Q[:], Q[:], 1.0 / 3.0, Alu.mult)

                tt(Alu.mult, P1[:], sl(A, 1), sl(A, 1))
                tt(Alu.mult, SC1[:], sl(A, 2), sl(A, 2))
                tt(Alu.add, P1[:], P1[:], SC1[:])
                tt(Alu.mult, SC1[:], sl(A, 4), sl(A, 4))
                tt(Alu.add, P1[:], P1[:], SC1[:])

                BD = tmp("BD", 3)
                tt(Alu.subtract, sl(BD, 0), sl(A, 0), Q[:])
                tt(Alu.subtract, sl(BD, 1), sl(A, 3), Q[:])
                tt(Alu.subtract, sl(BD, 2), sl(A, 5), Q[:])
                tt(Alu.mult, P2[:], sl(BD, 0), sl(BD, 0))
                tt(Alu.mult, SC1[:], sl(BD, 1), sl(BD, 1))
                tt(Alu.add, P2[:], P2[:], SC1[:])
                tt(Alu.mult, SC1[:], sl(BD, 2), sl(BD, 2))
                tt(Alu.add, P2[:], P2[:], SC1[:])
                stt(P2[:], P1[:], 2.0, Alu.mult, Alu.add, P2[:])
                ts(PP[:], P2[:], 1.0 / 6.0, Alu.mult)
                act(PP[:], PP[:], Act.Sqrt)
                ts(SC1[:], PP[:], TINY, Alu.max)
                nc.vector.reciprocal(RP[:], SC1[:])

                tt(Alu.mult, sl(NB, 0), sl(BD, 0), RP[:])
                tt(Alu.mult, sl(NB, 1), sl(A, 1), RP[:])
                tt(Alu.mult, sl(NB, 2), sl(A, 2), RP[:])
                tt(Alu.mult, sl(NB, 3), sl(BD, 1), RP[:])
                tt(Alu.mult, sl(NB, 4), sl(A, 4), RP[:])
                tt(Alu.mult, sl(NB, 5), sl(BD, 2), RP[:])

                tt(Alu.mult, SC1[:], sl(NB, 3), sl(NB, 5))
                tt(Alu.mult, SC2[:], sl(NB, 4), sl(NB, 4))
                tt(Alu.subtract, SC1[:], SC1[:], SC2[:])
                tt(Alu.mult, DET[:], sl(NB, 0), SC1[:])
                tt(Alu.mult, SC1[:], sl(NB, 1), sl(NB, 5))
                tt(Alu.mult, SC2[:], sl(NB, 4), sl(NB, 2))
                tt(Alu.subtract, SC1[:], SC1[:], SC2[:])
                tt(Alu.mult, SC1[:], sl(NB, 1), SC1[:])
                tt(Alu.subtract, DET[:], DET[:], SC1[:])
                tt(Alu.mult, SC1[:], sl(NB, 1), sl(NB, 4))
                tt(Alu.mult, SC2[:], sl(NB, 3), sl(NB, 2))
                tt(Alu.subtract, SC1[:], SC1[:], SC2[:])
                tt(Alu.mult, SC1[:], sl(NB, 2), SC1[:])
                tt(Alu.add, DET[:], DET[:], SC1[:])

                ts(RR[:], DET[:], 0.5, Alu.mult)
                ts(RR[:], RR[:], -1.0, Alu.max)
                ts(RR[:], RR[:], 1.0, Alu.min)
                tt(Alu.mult, SS[:], RR[:], RR[:])
                nc.vector.tensor_scalar(out=SS[:], in0=SS[:], scalar1=-1.0,
                                        scalar2=1.0, op0=Alu.mult, op1=Alu.add)
                ts(SS[:], SS[:], 0.0, Alu.max)
                act(SS[:], SS[:], Act.Sqrt)
                UA = tmp("UA"); UB = tmp("UB")
                ts(SC1[:], RR[:], -1.0, Alu.mult)
                tt(Alu.max, SC1[:], SC1[:], RR[:])
                ts(SS[:], SS[:], TINY, Alu.max)
                nc.vector.reciprocal(SC2[:], SS[:])
                tt(Alu.mult, UA[:], SC1[:], SC2[:])
                ts(SC1[:], UA[:], TINY, Alu.max)
                nc.vector.reciprocal(UB[:], SC1[:])
                tt(Alu.min, SC2[:], UA[:], UB[:])
                act(SC2[:], SC2[:], Act.Arctan)
                ts(SC1[:], UA[:], 1.0, Alu.is_gt)
                nc.vector.tensor_scalar(out=SC3[:], in0=SC2[:], scalar1=-2.0,
                                        scalar2=_PI / 2.0, op0=Alu.mult,
                                        op1=Alu.add)
                tt(Alu.mult, SC3[:], SC3[:], SC1[:])
                tt(Alu.add, SC2[:], SC2[:], SC3[:])
                ts(SC3[:], RR[:], 0.0, Alu.is_lt)
                nc.vector.tensor_scalar(out=SC3[:], in0=SC3[:], scalar1=-2.0,
                                        scalar2=1.0, op0=Alu.mult, op1=Alu.add)
                tt(Alu.mult, AT[:], SC2[:], SC3[:])
                nc.vector.tensor_scalar(out=PHI[:], in0=AT[:],
                                        scalar1=-1.0 / 3.0,
                                        scalar2=_PI / 6.0 + _PI / 2.0,
                                        op0=Alu.mult, op1=Alu.add)
                act(SC1[:], PHI[:], Act.Sin)
                tt(Alu.mult, SC1[:], SC1[:], PP[:])
                stt(W2[:], SC1[:], 2.0, Alu.mult, Alu.add, Q[:])
                nc.vector.tensor_scalar(out=PHI[:], in0=AT[:],
                                        scalar1=-1.0 / 3.0,
                                        scalar2=_PI / 6.0 + _PI / 6.0,
                                        op0=Alu.mult, op1=Alu.add)
                act(SC1[:], PHI[:], Act.Sin)
                tt(Alu.mult, SC1[:], SC1[:], PP[:])
                stt(W0[:], SC1[:], -2.0, Alu.mult, Alu.add, Q[:])
                ts(SC1[:], Q[:], 3.0, Alu.mult)
                tt(Alu.subtract, W1[:], SC1[:], W0[:])
                tt(Alu.subtract, W1[:], W1[:], W2[:])

                ts(SC1[:], W2[:], TINY, Alu.max)
                nc.vector.reciprocal(RW2[:], SC1[:])
                tt(Alu.mult, DIRWT[:], W1[:], RW2[:])
                nc.vector.tensor_scalar(out=DIRWT[:], in0=DIRWT[:],
                                        scalar1=-1.0, scalar2=1.0,
                                        op0=Alu.mult, op1=Alu.add)
                tt(Alu.mult,
                   B6[:].rearrange("p (q t) -> p q t", q=6),
                   A[:].rearrange("p (q t) -> p q t", q=6),
                   RW2[:].unsqueeze(1).broadcast_to((P, 6, NT)))

                CD = tmp("CD", 3)
                DD = tmp("DD", 3)
                for qi, ai in enumerate((0, 3, 5)):
                    tt(Alu.subtract, sl(CD, qi), sl(A, ai), W0[:])
                    tt(Alu.subtract, sl(DD, qi), sl(A, ai), W1[:])
                M9 = tmp("M9", 9)

                def mcol(colq, dv):
                    crow = [(sl(CD, 0), sl(A, 1), sl(A, 2)),
                            (sl(A, 1), sl(CD, 1), sl(A, 4)),
                            (sl(A, 2), sl(A, 4), sl(CD, 2))]
                    for r in range(3):
                        a0, a1, a2 = crow[r]
                        tt(Alu.mult, SC1[:], a0, dv[0])
                        tt(Alu.mult, SC2[:], a1, dv[1])
                        tt(Alu.add, SC1[:], SC1[:], SC2[:])
                        tt(Alu.mult, SC2[:], a2, dv[2])
                        tt(Alu.add, sl(M9, colq * 3 + r), SC1[:], SC2[:])

                mcol(0, (sl(DD, 0), sl(A, 1), sl(A, 2)))
                mcol(1, (sl(A, 1), sl(DD, 1), sl(A, 4)))
                mcol(2, (sl(A, 2), sl(A, 4), sl(DD, 2)))

                CN = tmp("CN", 3)
                for j in range(3):
                    tt(Alu.mult, sl(CN, j), sl(M9, j * 3), sl(M9, j * 3))
                    tt(Alu.mult, SC1[:], sl(M9, j * 3 + 1), sl(M9, j * 3 + 1))
                    tt(Alu.add, sl(CN, j), sl(CN, j), SC1[:])
                    tt(Alu.mult, SC1[:], sl(M9, j * 3 + 2), sl(M9, j * 3 + 2))
                    tt(Alu.add, sl(CN, j), sl(CN, j), SC1[:])
                NBEST = tmp("NBEST")
                SC3W = tmp("SC3W", 3)
                nc.vector.tensor_copy(out=V3[:], in_=M9[:, 0:3 * NT])
                nc.vector.tensor_copy(out=NBEST[:], in_=sl(CN, 0))
                for j in (1, 2):
                    tt(Alu.is_gt, GT[:], sl(CN, j), NBEST[:])
                    gtb = GT[:].unsqueeze(1).broadcast_to((P, 3, NT))
                    v3v = V3[:].rearrange("p (i t) -> p i t", i=3)
                    sc3 = SC3W[:].rearrange("p (i t) -> p i t", i=3)
                    tt(Alu.subtract, sc3,
                       M9[:, j * 3 * NT:(j + 1) * 3 * NT].rearrange(
                           "p (i t) -> p i t", i=3), v3v)
                    tt(Alu.mult, sc3, sc3, gtb)
                    tt(Alu.add, v3v, v3v, sc3)
                    tt(Alu.max, NBEST[:], NBEST[:], sl(CN, j))
                ts(SC1[:], NBEST[:], 1e-37, Alu.max)
                act(SC2[:], SC1[:], Act.Sqrt)
                nc.vector.reciprocal(SC2[:], SC2[:])
                tt(Alu.mult, V3[:].rearrange("p (i t) -> p i t", i=3),
                   V3[:].rearrange("p (i t) -> p i t", i=3),
                   SC2[:].unsqueeze(1).broadcast_to((P, 3, NT)))
                return DIRWT

            DIRWT = cluster_math()

            # ---------------- pass B (cluster-major, scaled units) ----
            from contextlib import ExitStack
            _pb_stack = ExitStack()
            pbp = _pb_stack.enter_context(tc.tile_pool(name="pbp", bufs=1))
            pb = _pb_stack.enter_context(tc.tile_pool(name="pb", bufs=2))
            CSTR = pbp.tile([P, 3 * Sg], dt.float16, tag="CSTR", name="CSTR")
            nc.sync.dma_start(out=CSTR[:], in_=cstr[:, :])
            TP = pbp.tile([P, Sg], dt.float16, tag="TP", name="TP")

            def cm_plane(i):   # cluster-major input plane i
                return CSTR[:, i * Sg:(i + 1) * Sg]

            # NK = -(c . v0)  (also reused as T0 in sign phase)
            NK = full_tile("NK")
            SCX = full_tile("SCX")
            tt(Alu.mult, NK[:], CEN[:, 0:NT], V3[:, 0:NT])
            tt(Alu.mult, SCX[:], CEN[:, NT:2 * NT], V3[:, NT:2 * NT])
            tt(Alu.add, NK[:], NK[:], SCX[:])
            tt(Alu.mult, SCX[:], CEN[:, 2 * NT:3 * NT], V3[:, 2 * NT:3 * NT])
            tt(Alu.add, NK[:], NK[:], SCX[:])
            # ---------------- sign + output --------------------------
            OUTST = ret.tile([P, 19 * NT], dt.float32, tag="OUTST",
                             name="OUTST")
            COR = full_tile("COR")

            def sl_(T, i):
                return T[:, i * NT:(i + 1) * NT]

            def sign_prep():
                CC = full_tile("CC"); R0 = full_tile("R0")
                SC9 = full_tile("SC9"); NPAD = full_tile("NPAD")
                T0 = NK
                tt(Alu.mult, CC[:], sl_(CEN, 0), sl_(CEN, 0))
                tt(Alu.mult, SC9[:], sl_(CEN, 1), sl_(CEN, 1))
                tt(Alu.add, CC[:], CC[:], SC9[:])
                tt(Alu.mult, SC9[:], sl_(CEN, 2), sl_(CEN, 2))
                tt(Alu.add, CC[:], CC[:], SC9[:])
                tt(Alu.mult, SC9[:], T0[:], T0[:])
                tt(Alu.subtract, R0[:], CC[:], SC9[:])
                ts(R0[:], R0[:], 0.0, Alu.max)
                act(R0[:], R0[:], Act.Sqrt)
                for t in range(NT):
                    lg = int(Lg[t // 4])
                    nc.vector.tensor_scalar(
                        out=NPAD[:, t:t + 1],
                        in0=NV[:, t:t + 1], scalar1=-1.0,
                        scalar2=float(lg), op0=Alu.mult, op1=Alu.add)
                tt(Alu.mult, COR[:], T0[:], R0[:])
                tt(Alu.mult, COR[:], COR[:], NPAD[:])
                # unscale centers: x16
                for i in range(3):
                    ts(sl_(CEN, i), sl_(CEN, i), SCL, Alu.mult)
                for j, pl in [(0, sl_(CEN, 0)), (1, sl_(CEN, 1)),
                              (2, sl_(CEN, 2)),
                              (3, sl_(B6, 0)), (4, sl_(B6, 1)), (5, sl_(B6, 2)),
                              (6, sl_(B6, 1)), (7, sl_(B6, 3)), (8, sl_(B6, 4)),
                              (9, sl_(B6, 2)), (10, sl_(B6, 4)),
                              (11, sl_(B6, 5)),
                              (15, NV[:]), (16, MEANV[:]), (17, STDV[:]),
                              (18, MODE[:])]:
                    nc.vector.tensor_copy(
                        out=OUTST[:, j * NT:(j + 1) * NT], in_=pl)


            ts(NK[:], NK[:], -1.0, Alu.mult)
            sign_prep()

            for g in range(NG):
                lg = int(Lg[g]); s0 = int(goff[g]); w = 4 * lg
                SQ3 = pb.tile([P, 3 * w], dt.float16, tag="SQ3", name=f"SQ3{g}")
                QA = pb.tile([P, w], dt.float16, tag="QA", name=f"QA{g}")
                QB = pb.tile([P, w], dt.float16, tag="QB", name=f"QB{g}")
                T2 = pb.tile([P, w], dt.float16, tag="T2", name=f"T2{g}")
                R2 = pb.tile([P, w], dt.float16, tag="R2", name=f"R2{g}")
                RPL = pb.tile([P, w], dt.float16, tag="RPL", name=f"RPL{g}")
                for tg in range(4):
                    t = 4 * g + tg
                    sl0 = s0 + tg * lg
                    # centered squares straight from raw x via ACT bias
                    for i in range(3):
                        act(SQ3[:, i * w + tg * lg:i * w + (tg + 1) * lg],
                            cm_plane(i)[:, sl0:sl0 + lg], Act.Square,
                            bias=NCEN[:, i * NT + t:i * NT + t + 1])
                    # T = x*v0x + NK, += y*v0y, += z*v0z
                    nc.vector.tensor_scalar(
                        out=TP[:, sl0:sl0 + lg],
                        in0=cm_plane(0)[:, sl0:sl0 + lg],
                        scalar1=V3[:, 0 * NT + t:0 * NT + t + 1],
                        scalar2=NK[:, t:t + 1],
                        op0=Alu.mult, op1=Alu.add)
                    stt(TP[:, sl0:sl0 + lg], cm_plane(1)[:, sl0:sl0 + lg],
                        V3[:, 1 * NT + t:1 * NT + t + 1], Alu.mult, Alu.add,
                        TP[:, sl0:sl0 + lg])
                    stt(TP[:, sl0:sl0 + lg], cm_plane(2)[:, sl0:sl0 + lg],
                        V3[:, 2 * NT + t:2 * NT + t + 1], Alu.mult, Alu.add,
                        TP[:, sl0:sl0 + lg])
                # q and r per group; first add on gpsimd
                nc.gpsimd.tensor_tensor(out=QA[:], in0=SQ3[:, 0:w],
                                        in1=SQ3[:, w:2 * w], op=Alu.add)
                tt(Alu.add, QB[:], QA[:], SQ3[:, 2 * w:3 * w])
                tt(Alu.mult, T2[:], TP[:, s0:s0 + w], TP[:, s0:s0 + w])
                tt(Alu.subtract, R2[:], QB[:], T2[:])
                ts(R2[:], R2[:], 0.0, Alu.max)
                act(RPL[:], R2[:], Act.Sqrt)
                for tg in range(4):
                    t = 4 * g + tg
                    stt(T2[:, tg * lg:(tg + 1) * lg],
                        TP[:, s0 + tg * lg:s0 + (tg + 1) * lg], 1.0,
                        Alu.mult, Alu.mult,
                        RPL[:, tg * lg:(tg + 1) * lg],
                        accum=SCRAW[:, t:t + 1])

            def sign_final():
                SCV = full_tile("SCV"); FAC = full_tile("FAC")
                GT9 = full_tile("GT9")
                tt(Alu.subtract, SCV[:], SCRAW[:], COR[:])
                ts(GT9[:], SCV[:], 0.0, Alu.is_lt)
                nc.vector.tensor_scalar(out=GT9[:], in0=GT9[:], scalar1=-2.0,
                                        scalar2=1.0, op0=Alu.mult, op1=Alu.add)
                tt(Alu.mult, FAC[:], DIRWT[:], GT9[:])
                tt(Alu.mult,
                   OUTST[:, 12 * NT:15 * NT].rearrange("p (i t) -> p i t", i=3),
                   V3[:].rearrange("p (i t) -> p i t", i=3),
                   FAC[:].unsqueeze(1).broadcast_to((P, 3, NT)))
                nc.sync.dma_start(out=res[:, :], in_=OUTST[:])

            sign_final()
            _pb_stack.close()

    nc.compile()
    return nc


_cache = {}
_last = None


def kernel(data, clust_idx, clust_len):
    global N, C, L, NT, NG
    data = np.asarray(data)
    clust_idx = np.asarray(clust_idx)
    N = int(data.shape[0])
    C, L = int(clust_idx.shape[0]), int(clust_idx.shape[1])
    assert C % (P * N_CORES) == 0
    NT = C // (P * N_CORES)
    NG = NT // 4
    meta = _host_prep(data, clust_idx, clust_len)

    key = (tuple(int(x) for x in meta["Lb"]), N, C)
    if key not in _cache:
        _cache[key] = _build_program(meta)
    nc = _cache[key]

    from concourse.bass_utils import run_bass_kernel_spmd
    in_maps = [{"tstr": meta["tstr"][c], "cstr": meta["cstr"][c],
                "nvec": meta["nvecs"][c]} for c in range(N_CORES)]
    global _last
    _last = (nc, in_maps)
    res = run_bass_kernel_spmd(nc, in_maps, list(range(N_CORES)))

    ids = meta["ids"]
    out = np.zeros((C, 19), dtype=f32)
    for core in range(N_CORES):
        r = res.results[core]["res"].reshape(P, 19, NT)
        for t in range(NT):
            out[ids[core, t]] = r[:, :, t]
    return out
